# revision 24
# baseline (speedup 1.0000x reference)
"""Trainium2 Bass kernel for nn_FC_LSTM (FC-LSTM encoder-decoder).

Strategy:
  - Data-parallel over batch: 256 samples -> 8 cores x 32 samples.
  - Feature-major layout on chip: activations stored [feature(part), sample(free)],
    weights host-transposed to [in_feat, out_feat] so every matmul is
    out[feat_out, samples] = W_T.T @ act with contraction on partitions.
  - All matmuls in fp8e4 (e4m3) with MatmulPerfMode.DoubleRow: each
    instruction contracts a 256-wide K (two 128 k-tiles packed in dim1 of
    both operands), doubling PE throughput and halving PE instruction
    count vs bf16. Weights are host-quantized with power-of-2 scales
    (max|w|*s = 128 < 240); the descale folds into the activation
    instruction's input scale (out = func(scale*psum + bias)).
  - Encoder FC stack (4096->1024->256) batched over all 20 timesteps (640
    samples per core). en1's input-side gate matmul (Wih @ z_t) is also
    precomputed densely for all t at gate scale 2^11.
  - LSTM recurrence: per step the layer-2 cell at t and the layer-1 cell at
    t+1 depend only on the previous step's pair state, so they are emitted
    as a PAIR: two PSUM banks (a shared bank trips psum start=True
    whole-bank zeroing), but one [128, 512] SBUF gates tile and
    double-width elementwise (sig/tanh ACT per bank, then pair-wide
    mul/add on DVE + Pool, one tanh(c) ACT, one fp8 h-mul). Gate psum col
    m*32+s = gate-feature 128m+p of sample s (scale 2^11); gate order
    host-permuted to [i, f, o, g]. Cell biases (x2^11) are injected with a
    single K=128 bf16 matmul against a constant indicator matrix; FC biases
    likewise ride in the psum via a bias-in-row-0 bf16 matmul against a
    ones column, so FC relu+descale is a single 2-op DVE tensor_scalar
    (no ACT involvement outside the cells and the output tanh).
    h state is stored fp8, c state fp32.
  - Decoder FC stack (256->1024->4096) batched over all future steps.
  - The FC phases are emitted as generators whose matmul blocks interleave
    into the recurrence as PE gap fillers; DMAs are emitted in first-use
    order (small bias/ones constants first: they gate FC psum-group
    starts).
  - measure_hw_time uses a block design: bursts of same-program calls
    (per-call NEFF swap costs otherwise contaminate the estimate),
    alternating blocks of the 1-rep and 9-rep programs, difference of
    median block-medians.
"""

import time
from contextlib import ExitStack

import ml_dtypes
import numpy as np

import concourse.bass as bass
import concourse.mybir as mybir
import concourse.tile as tile

BF16NP = ml_dtypes.bfloat16
F8NP = ml_dtypes.float8_e4m3
AF = mybir.ActivationFunctionType
DT = mybir.dt
DR = mybir.MatmulPerfMode.DoubleRow

S = 20          # encoder sequence length
B = 256         # global batch
NCORES = 8
BL = B // NCORES  # 32 samples per core
H = 256         # LSTM hidden
G = 4 * H       # 1024 gate features
D = 4096        # input feature dim (64*64)
HID = 1024      # FC hidden
SB = S * BL     # 640 encoder samples per core

# power-of-2 quant scales: max|w| * SW = 128 (e4m3 max finite = 240)
SW1 = 2.0 ** 13    # fc_en1 w ~ U(+-2^-6)
SW2 = 2.0 ** 12    # fc_en2 w ~ U(+-2^-5)
SWC = 2.0 ** 11    # cell weights ~ U(+-2^-4); gate psum scale
SWD1 = 2.0 ** 11   # fc_de1 w ~ U(+-2^-4)
SWD2 = 2.0 ** 12   # fc_de2 w ~ U(+-2^-5)

VERBOSE = True


def _log(*a):
    if VERBOSE:
        print("[kernel]", *a, flush=True)


# ---------------------------------------------------------------------------
# Workaround: walrus CoreV3 setupSyncWait allows only 1 sync wait on the
# TileContext exit Drain. Split its waits across multiple drain instructions.
# ---------------------------------------------------------------------------
def _patched_drain_and_barrier(self, tick_clock, wait_clock):
    nc = self.nc
    drain_inst = nc.sync.drain()
    wait_clock.add_sem_waits(
        drain_inst.ins, tile.ScopedClock({None: tick_clock.global_clock})
    )
    inst = drain_inst.ins
    si = inst.sync_info
    waits = list(si.on_wait) if si is not None and si.on_wait else []
    MAXW = 1
    if len(waits) > MAXW:
        si.on_wait = waits[:MAXW]
        for i in range(MAXW, len(waits), MAXW):
            d2 = nc.sync.drain()
            i2 = d2.ins
            si2 = i2.sync_info
            if si2 is None:
                i2.sync_info = type(si)(on_wait=waits[i : i + MAXW], on_update=[])
            else:
                si2.on_wait = list(si2.on_wait or []) + waits[i : i + MAXW]

    nc.all_engine_barrier()
    assert self.sems is not None
    popped = nc._tile_sem_poison_stack.pop()
    assert popped is self._sem_poison
    nc.clear_and_free_semaphores(list(self.sems.allocated().values()))
    nc.all_engine_barrier()


tile.TileContext._drain_and_barrier = _patched_drain_and_barrier


def _split_sync_waits(nc, limit=1):
    """walrus setupSyncWait rejects >2 sem waits per instruction: move excess
    waits onto same-engine NoOps spliced just before the instruction."""
    ctr = [0]
    SyncInfo = None
    for f in nc.m.functions:
        for bb in f.blocks:
            out = []
            for inst in bb.instructions:
                si = inst.sync_info
                waits = list(si.on_wait) if si is not None and si.on_wait else []
                if len(waits) > limit:
                    if SyncInfo is None:
                        SyncInfo = type(si)
                    extras = waits[: len(waits) - limit]
                    si.on_wait = waits[len(waits) - limit:]
                    for i in range(0, len(extras), limit):
                        ctr[0] += 1
                        nop = mybir.InstNoOp(name=f"ws_{ctr[0]}", ins=[], outs=[])
                        nop.engine = inst.engine
                        nop.sync_info = SyncInfo(
                            on_wait=extras[i : i + limit], on_update=[]
                        )
                        out.append(nop)
                out.append(inst)
            bb.instructions[:] = out
    return ctr[0]


# ---------------------------------------------------------------------------
# Program builder
# ---------------------------------------------------------------------------
CELLS = ["en1", "en2", "en3", "de1", "de2", "de3"]
DJ = D // 256       # 16 k-tile-pairs of the 4096 input dim
HJ = HID // 256     # 4 k-tile-pairs of the 1024 hidden dim


def build_program(F: int, nrep: int = 1) -> bass.Bass:
    FB = F * BL  # decoder samples per core
    assert FB <= 512 and FB % 2 == 0
    nc = bass.Bass()

    # --- DRAM tensors (all fp8 weight layouts are DoubleRow-paired:
    #     row j*128+p, col i*M+m holds W[256j+128i+p, m]) ---
    xT = nc.dram_tensor("xT", [DJ * 128, 2 * SB], DT.float8e4, kind="ExternalInput")
    w1T = nc.dram_tensor("w1T", [DJ * 128, 2 * HID], DT.float8e4, kind="ExternalInput")
    b1B = nc.dram_tensor("b1B", [128, HID], DT.bfloat16, kind="ExternalInput")
    w2T = nc.dram_tensor("w2T", [HJ * 128, 2 * H], DT.float8e4, kind="ExternalInput")
    b2B = nc.dram_tensor("b2B", [128, H], DT.bfloat16, kind="ExternalInput")
    cellT = {}
    for nm in CELLS:
        ins = {}
        if nm != "de1":
            ins["wih"] = nc.dram_tensor(f"{nm}_wihT", [128, 2 * G], DT.float8e4,
                                        kind="ExternalInput")
        ins["whh"] = nc.dram_tensor(f"{nm}_whhT", [128, 2 * G], DT.float8e4,
                                    kind="ExternalInput")
        if nm == "en1":
            ins["bv"] = nc.dram_tensor("en1_bv", [128, G // 128], DT.float32,
                                       kind="ExternalInput")
        else:
            ins["bB"] = nc.dram_tensor(f"{nm}_bB", [128, 128], DT.bfloat16,
                                       kind="ExternalInput")
        cellT[nm] = ins
    Em = nc.dram_tensor("Em", [128, 256], DT.bfloat16, kind="ExternalInput")
    # decoder head stays bf16: its pre-tanh output is tiny relative to the
    # summand magnitudes (heavy cancellation), so fp8 there blows the error
    # budget (measured: fp8 fc_de2 alone -> 3.3e-2 rel err).
    wd1T = nc.dram_tensor("wd1T", [H, HID], DT.bfloat16, kind="ExternalInput")
    bd1B = nc.dram_tensor("bd1B", [128, HID], DT.bfloat16, kind="ExternalInput")
    wd2T = nc.dram_tensor("wd2T", [HID, D], DT.bfloat16, kind="ExternalInput")
    bd2v = nc.dram_tensor("bd2v", [128, D // 128], DT.float32, kind="ExternalInput")
    yT = nc.dram_tensor("yT", [D, FB], DT.float32, kind="ExternalOutput")

    NCHA = 4                      # phase A chunks (5 encoder steps each)
    CHA = SB // NCHA              # 160 samples
    SPC = S // NCHA               # steps per chunk
    NCHD = 2                      # phase D chunks
    CHD = FB // NCHD

    def pair(ap):
        """[128, 2*N] fp8 AP -> [128, 2, N] DoubleRow operand view."""
        return ap.rearrange("p (i n) -> p i n", i=2)

    with tile.TileContext(nc) as tc:
     for rep in range(nrep):
      with ExitStack() as ctx:
        const = ctx.enter_context(tc.tile_pool(name="const", bufs=1))
        gi1p = ctx.enter_context(tc.tile_pool(name="gi1p", bufs=1))
        state = ctx.enter_context(tc.tile_pool(name="state", bufs=3))
        gates = ctx.enter_context(tc.tile_pool(name="gates", bufs=4))
        outp = ctx.enter_context(tc.tile_pool(name="outp", bufs=4))
        psum = ctx.enter_context(tc.tile_pool(name="psum", bufs=8, space="PSUM"))

        uid = [0]

        def PS():
            uid[0] += 1
            return psum.tile([128, 512], DT.float32, tag="ps",
                             name=f"ps{uid[0]}")

        def dma_in2(pool, dram, tag):
            t = pool.tile(list(dram.shape), dram.dtype, tag=tag)
            nc.sync.dma_start(t[:], dram[:])
            return t

        def gi_ap(t):
            c, o = divmod(t, SPC)
            return gi1[c][:, :, o * BL:(o + 1) * BL]

        # ========== Phase A generator: per-k-tile weight/x DMAs, chunked ===
        pA_ctx = ExitStack()
        pA = pA_ctx.enter_context(tc.tile_pool(name="phaseA", bufs=1))

        # DMA emission ordered by first use: w1 + x chunk 0 feed the FC chain
        # immediately; encoder cell weights arrive next; remaining x chunks
        # stream during the early recurrence; decoder weights last.
        # tiny constants first: the FC1 bias matmuls (group starts) need
        # b1/ones immediately; don't queue them behind 5MB of w1/x stream
        b1_sb = const.tile([128, HID // 128, 128], DT.bfloat16, tag="b1B")
        nc.sync.dma_start(b1_sb[:], b1B.rearrange("p (m q) -> p m q", q=128))
        ones_sb = const.tile([128, 160], DT.bfloat16, tag="ones")
        nc.vector.memset(ones_sb[:], 1.0)
        b2_sb = const.tile([128, H // 128, 128], DT.bfloat16, tag="b2B")
        nc.sync.dma_start(b2_sb[:], b2B.rearrange("p (m q) -> p m q", q=128))
        x_kc = [[None] * NCHA for _ in range(DJ)]
        w1_k = []
        for j in range(DJ):
            wk = pA.tile([128, 2 * HID], DT.float8e4, tag=f"w1_{j}",
                         name=f"w1_{j}")
            nc.sync.dma_start(wk[:], w1T[j * 128:(j + 1) * 128, :])
            w1_k.append(wk)
            xk = pA.tile([128, 2 * CHA], DT.float8e4, tag=f"x{j}_0",
                         name=f"x{j}_0")
            nc.sync.dma_start(xk[:], xT[j * 128:(j + 1) * 128, 0:2 * CHA])
            x_kc[j][0] = xk

        # encoder-phase constants
        w2_sb = const.tile([128, HJ, 2 * H], DT.float8e4, tag="w2")
        nc.sync.dma_start(w2_sb[:], w2T.rearrange("(j p) c -> p j c", p=128))
        cell_sb = {}
        for nm in ["en1", "en2", "en3"]:
            e = {}
            e["wih"] = dma_in2(const, cellT[nm]["wih"], f"{nm}_wih")
            e["whh"] = dma_in2(const, cellT[nm]["whh"], f"{nm}_whh")
            if nm == "en1":
                e["bv"] = dma_in2(const, cellT[nm]["bv"], "en1_bv")
            else:
                e["bB"] = dma_in2(const, cellT[nm]["bB"], f"{nm}_bB")
            cell_sb[nm] = e
        E_sb = dma_in2(const, Em, "Em")

        # remaining x chunks
        for c in range(1, NCHA):
            for j in range(DJ):
                xk = pA.tile([128, 2 * CHA], DT.float8e4, tag=f"x{j}_{c}",
                             name=f"x{j}_{c}")
                nc.sync.dma_start(
                    xk[:], xT[j * 128:(j + 1) * 128, c * 2 * CHA:(c + 1) * 2 * CHA])
                x_kc[j][c] = xk

        # decoder-phase constants
        for nm in ["de1", "de2", "de3"]:
            e = {}
            if "wih" in cellT[nm]:
                e["wih"] = dma_in2(const, cellT[nm]["wih"], f"{nm}_wih")
            e["whh"] = dma_in2(const, cellT[nm]["whh"], f"{nm}_whh")
            e["bB"] = dma_in2(const, cellT[nm]["bB"], f"{nm}_bB")
            cell_sb[nm] = e
        wd1_sb = const.tile([128, H // 128, HID], DT.bfloat16, tag="wd1")
        nc.sync.dma_start(wd1_sb[:], wd1T.rearrange("(o p) m -> p o m", p=128))
        bd1_sb = const.tile([128, HID // 128, 128], DT.bfloat16, tag="bd1B")
        nc.sync.dma_start(bd1_sb[:], bd1B.rearrange("p (m q) -> p m q", q=128))
        bd2_sb = dma_in2(const, bd2v, "bd2v")
        zh = const.tile([128, 2 * BL], DT.float8e4, tag="zh")
        zc = const.tile([128, 2 * BL], DT.float32, tag="zc")
        nc.vector.memset(zh[:], 0.0)
        nc.vector.memset(zc[:], 0.0)
        h3all = [const.tile([128, H // 128, CHD], DT.bfloat16, tag=f"h3all{c}",
                            name=f"h3all{c}")
                 for c in range(NCHD)]
        gi1 = [gi1p.tile([128, G // 128, CHA], DT.float32, tag=f"gi1_{c}",
                         name=f"gi1_{c}")
               for c in range(NCHA)]

        def phaseA_gen():
            """Emits all of phase A; yields chunks_done after each MM block."""
            for c in range(NCHA):
                z1c = pA.tile([128, HID // 128, CHA], DT.float8e4,
                              tag=f"z1_{c}", name=f"z1_{c}")
                zc_ = pA.tile([128, 2, CHA], DT.float8e4,
                              tag=f"z_{c}", name=f"z_{c}")
                if c == 0:
                    # chunk 0 runs while x/w1 DMAs land: j-outer so each
                    # arriving k-pair tile is consumed immediately
                    for half in range(2):
                        ms = range(half * 4, half * 4 + 4)
                        pss = [PS()[:, :CHA] for _ in ms]
                        for mi, m in enumerate(ms):
                            nc.tensor.matmul(
                                pss[mi], b1_sb[:, m, :], ones_sb[:, :CHA],
                                start=True, stop=False, skip_group_check=True)
                        for j in range(DJ):
                            wv = pair(w1_k[j][:])
                            xv = pair(x_kc[j][0][:])
                            for mi, m in enumerate(ms):
                                nc.tensor.matmul(
                                    pss[mi], wv[:, :, m * 128:(m + 1) * 128],
                                    xv,
                                    start=False, stop=(j == DJ - 1),
                                    perf_mode=DR,
                                )
                            if j % 4 == 3:
                                yield c
                        for mi, m in enumerate(ms):
                            nc.vector.tensor_scalar(
                                z1c[:, m, :], pss[mi], 1.0 / SW1, 0.0,
                                mybir.AluOpType.mult, mybir.AluOpType.max)
                else:
                    for m in range(HID // 128):
                        ps = PS()[:, :CHA]
                        nc.tensor.matmul(
                            ps, b1_sb[:, m, :], ones_sb[:, :CHA],
                            start=True, stop=False, skip_group_check=True)
                        for j0 in range(0, DJ, 4):
                            for j in range(j0, j0 + 4):
                                nc.tensor.matmul(
                                    ps,
                                    pair(w1_k[j][:])[:, :, m * 128:(m + 1) * 128],
                                    pair(x_kc[j][c][:]),
                                    start=False, stop=(j == DJ - 1),
                                    perf_mode=DR,
                                )
                            yield c
                        nc.vector.tensor_scalar(
                            z1c[:, m, :], ps, 1.0 / SW1, 0.0,
                            mybir.AluOpType.mult, mybir.AluOpType.max)
                for m in range(H // 128):
                    ps = PS()[:, :CHA]
                    nc.tensor.matmul(
                        ps, b2_sb[:, m, :], ones_sb[:, :CHA],
                        start=True, stop=False, skip_group_check=True)
                    for j in range(HJ):
                        nc.tensor.matmul(
                            ps,
                            pair(w2_sb[:, j, :])[:, :, m * 128:(m + 1) * 128],
                            z1c[:, 2 * j:2 * j + 2, :],
                            start=False, stop=(j == HJ - 1),
                            perf_mode=DR,
                        )
                    nc.vector.tensor_scalar(
                        zc_[:, m, :], ps, 1.0 / SW2, 0.0,
                        mybir.AluOpType.mult, mybir.AluOpType.max)
                    yield c
                wihv = pair(cell_sb["en1"]["wih"][:])
                for m in range(G // 128):
                    ps = PS()[:, :CHA]
                    nc.tensor.matmul(
                        ps, wihv[:, :, m * 128:(m + 1) * 128], zc_[:],
                        start=True, stop=True, perf_mode=DR,
                    )
                    nc.vector.tensor_scalar_add(
                        gi1[c][:, m, :], ps, cell_sb["en1"]["bv"][:, m:m + 1])
                    if m % 2 == 1:
                        yield c + (m == G // 128 - 1)
            while True:
                yield NCHA + 1

        genA = phaseA_gen()
        a_done = [0]

        def fillA(n=1):
            for _ in range(n):
                a_done[0] = max(a_done[0], next(genA))

        def needA(chunks):
            while a_done[0] < chunks + 1:
                fillA()

        # ========== LSTM cell (single) ====================================
        def lstm_cell(nm, x_in, h_prev, c_prev, gi, htag, ctag,
                      h_out=None, c_out=None):
            e = cell_sb[nm]
            ps = PS()[:, :256]
            groups = []
            if gi is None:
                nc.tensor.matmul(ps, e["bB"][:], E_sb[:], start=True, stop=False)
                started = True
            else:
                started = False
            if x_in is not None:
                groups.append((pair(e["wih"][:]), pair(x_in)))
            groups.append((pair(e["whh"][:]), pair(h_prev)))
            ng = len(groups)
            for gidx, (wv, rv) in enumerate(groups):
                for m in range(8):
                    nc.tensor.matmul(
                        ps[:, m * 32:(m + 1) * 32],
                        wv[:, :, m * 128:(m + 1) * 128],
                        rv,
                        start=(not started and gidx == 0),
                        stop=(gidx == ng - 1),
                        perf_mode=DR,
                        skip_group_check=True,
                    )
            if gi is not None:
                pv = ps.rearrange("p (m s) -> p m s", s=32)
                nc.vector.tensor_add(pv, pv, gi)
            g = gates.tile([128, 256], DT.float32, tag="g", name=f"g{uid[0]}")
            nc.scalar.activation(g[:, 0:192], ps[:, 0:192], AF.Sigmoid,
                                 scale=1.0 / SWC)
            nc.scalar.activation(g[:, 192:256], ps[:, 192:256], AF.Tanh,
                                 scale=1.0 / SWC)
            # packed: i: 0..63, f: 64..127, o: 128..191, g: 192..255
            t1 = gates.tile([128, 64], DT.float32, tag="t1", name=f"t1{uid[0]}")
            nc.gpsimd.tensor_mul(t1[:], g[:, 0:64], g[:, 192:256])
            c2 = gates.tile([128, 64], DT.float32, tag="c2", name=f"c2{uid[0]}")
            nc.vector.tensor_mul(c2[:], g[:, 64:128], c_prev)
            if c_out is None:
                c_new = state.tile([128, 64], DT.float32, tag=ctag,
                                   name=f"{ctag}{uid[0]}")
                c_out = c_new[:]
            else:
                c_new = None
            nc.vector.tensor_add(c_out, c2[:], t1[:])
            th = gates.tile([128, 64], DT.float32, tag="th", name=f"th{uid[0]}")
            nc.scalar.activation(th[:], c_out, AF.Tanh)
            if h_out is None:
                h_new = state.tile([128, 64], DT.float8e4, tag=htag,
                                   name=f"{htag}{uid[0]}")
                h_out = h_new[:]
            else:
                h_new = None
            nc.vector.tensor_mul(h_out, g[:, 128:192], th[:])
            return h_out, c_out

        # ========== Paired LSTM cells (A = layer-2 cell at t, B = layer-1
        # cell at t+1; both read only pair_prev, so their 512-col gate psums
        # share one bank and the elementwise chain runs at double width) ====
        def lstm_pair(nmA, nmB, hp_prev, cp_prev, giB, htag, ctag):
            eA, eB = cell_sb[nmA], cell_sb[nmB]
            psA = PS()[:, :256]
            psB = PS()[:, :256]
            nc.tensor.matmul(psA, eA["bB"][:], E_sb[:],
                             start=True, stop=False, skip_group_check=True)
            bstart = False
            if giB is None:
                nc.tensor.matmul(psB, eB["bB"][:], E_sb[:],
                                 start=True, stop=False, skip_group_check=True)
            else:
                bstart = True
            xA = pair(hp_prev[:, 64:128])   # h of layer-1 cell at t
            hA = pair(hp_prev[:, 0:64])     # layer-2 cell's own h at t-1
            hB = pair(hp_prev[:, 64:128])   # layer-1 cell's own h at t
            for m in range(8):
                sl = slice(m * 32, (m + 1) * 32)
                nc.tensor.matmul(psA[:, sl],
                                 pair(eA["wih"][:])[:, :, m * 128:(m + 1) * 128],
                                 xA, start=False, stop=False,
                                 perf_mode=DR, skip_group_check=True)
            for m in range(8):
                sl = slice(m * 32, (m + 1) * 32)
                nc.tensor.matmul(psA[:, sl],
                                 pair(eA["whh"][:])[:, :, m * 128:(m + 1) * 128],
                                 hA, start=False, stop=True,
                                 perf_mode=DR, skip_group_check=True)
            for m in range(8):
                sl = slice(m * 32, (m + 1) * 32)
                nc.tensor.matmul(psB[:, sl],
                                 pair(eB["whh"][:])[:, :, m * 128:(m + 1) * 128],
                                 hB, start=bstart, stop=True,
                                 perf_mode=DR, skip_group_check=True)
            if giB is not None:
                pv = psB.rearrange("p (m s) -> p m s", s=32)
                nc.vector.tensor_add(pv, pv, giB)
            # gates tile packs both cells: [ifo|g] at 0 (A) and 256 (B)
            g = gates.tile([128, 512], DT.float32, tag="gp", name=f"gp{uid[0]}")
            cp_new = state.tile([128, 128], DT.float32, tag=ctag,
                                name=f"{ctag}{uid[0]}")
            th = gates.tile([128, 128], DT.float32, tag="thp",
                            name=f"thp{uid[0]}")
            hp_new = state.tile([128, 128], DT.float8e4, tag=htag,
                                name=f"{htag}{uid[0]}")
            for o0, psx in ((0, psA), (256, psB)):
                nc.scalar.activation(g[:, o0:o0 + 192], psx[:, 0:192],
                                     AF.Sigmoid, scale=1.0 / SWC)
                nc.scalar.activation(g[:, o0 + 192:o0 + 256], psx[:, 192:256],
                                     AF.Tanh, scale=1.0 / SWC)
            # pair-wide elementwise on SBUF (strided [128, 2, 64] views)
            gv = g.rearrange("p (two x) -> p two x", two=2)
            cpv = cp_prev.rearrange("p (two s) -> p two s", two=2)
            t1 = gates.tile([128, 2, 64], DT.float32, tag="t1p",
                            name=f"t1p{uid[0]}")
            nc.gpsimd.tensor_mul(t1[:], gv[:, :, 0:64], gv[:, :, 192:256])
            c2 = gates.tile([128, 2, 64], DT.float32, tag="c2p",
                            name=f"c2p{uid[0]}")
            nc.vector.tensor_mul(c2[:], gv[:, :, 64:128], cpv)
            cnv = cp_new.rearrange("p (two s) -> p two s", two=2)
            nc.vector.tensor_add(cnv, c2[:], t1[:])
            nc.scalar.activation(th[:], cp_new[:], AF.Tanh)
            hnv = hp_new.rearrange("p (two s) -> p two s", two=2)
            nc.vector.tensor_mul(hnv, gv[:, :, 128:192],
                                 th.rearrange("p (two s) -> p two s", two=2))
            return hp_new, cp_new

        # ========== Phase D transition (callable mid-encoder) =============
        wd2_k = []
        pD_box = []

        def ensure_pD():
            if pD_box:
                return
            pA_ctx.close()  # free phase A SBUF before loading decoder weights
            pD = ctx.enter_context(tc.tile_pool(name="phaseD", bufs=1))
            pD_box.append(pD)
            for k in range(HID // 128):
                wk = pD.tile([128, D], DT.bfloat16, tag=f"wd2_{k}",
                             name=f"wd2_{k}")
                nc.sync.dma_start(wk[:], wd2T[k * 128:(k + 1) * 128, :])
                wd2_k.append(wk)

        # ========== Encoder recurrence, pipelined w/ phase A fillers ======
        # pair state: hp = [h2(t-1), h1(t)] (fp8), cp likewise (fp32)
        needA(0)
        h3s, c3s = [None] * S, [None] * S
        hp = state.tile([128, 128], DT.float8e4, tag="hp", name="hp_init")
        cp = state.tile([128, 128], DT.float32, tag="cp", name="cp_init")
        nc.vector.memset(hp[:, 0:64], 0.0)
        nc.vector.memset(cp[:, 0:64], 0.0)
        lstm_cell("en1", None, zh[:], zc[:], gi_ap(0), "h1", "c1",
                  h_out=hp[:, 64:128], c_out=cp[:, 64:128])
        for t in range(S):
            fillA(3)
            if t + 1 < S:
                needA((t + 1) // SPC)
                hp_new, cp_new = lstm_pair("en2", "en1", hp, cp,
                                           gi_ap(t + 1), "hp", "cp")
                h2t = hp_new[:, 0:64]
            else:
                h2t, _ = lstm_cell("en2", hp[:, 64:128], hp[:, 0:64],
                                   cp[:, 0:64], None, "h2", "c2")
            fillA(3)
            h3p = h3s[t - 1] if t else zh[:]
            c3p = c3s[t - 1] if t else zc[:]
            h3s[t], c3s[t] = lstm_cell("en3", h2t, h3p, c3p, None, "h3", "c3")
            if t + 1 < S:
                hp, cp = hp_new, cp_new
            if t == S - 5:
                needA(NCHA)  # drain phase A now so decoder weights can load
                ensure_pD()

        ensure_pD()
        pD = pD_box[0]

        def phaseD_gen(c):
            y1c = pD.tile([128, HID // 128, CHD], DT.bfloat16,
                          tag=f"y1_{c}", name=f"y1_{c}")
            for m in range(HID // 128):
                ps = PS()[:, :CHD]
                nc.tensor.matmul(
                    ps, bd1_sb[:, m, :], ones_sb[:, :CHD],
                    start=True, stop=False, skip_group_check=True)
                for k in range(H // 128):
                    nc.tensor.matmul(
                        ps, wd1_sb[:, k, m * 128:(m + 1) * 128],
                        h3all[c][:, k, :],
                        start=False, stop=(k == H // 128 - 1),
                    )
                nc.vector.tensor_scalar(
                    y1c[:, m, :], ps, 1.0, 0.0,
                    mybir.AluOpType.mult, mybir.AluOpType.max)
                if m % 2 == 1:
                    yield
            for m in range(D // 128):
                ps = PS()[:, :CHD]
                for k in range(HID // 128):
                    nc.tensor.matmul(
                        ps, wd2_k[k][:, m * 128:(m + 1) * 128],
                        y1c[:, k, :],
                        start=(k == 0), stop=(k == HID // 128 - 1),
                    )
                o_sb = outp.tile([128, CHD], DT.float32, tag="o",
                                 name=f"o{uid[0]}")
                nc.scalar.activation(o_sb[:], ps, AF.Tanh,
                                     bias=bd2_sb[:, m:m + 1])
                nc.sync.dma_start(
                    yT[m * 128:(m + 1) * 128, c * CHD:(c + 1) * CHD],
                    o_sb[:])
                yield

        genDs = [phaseD_gen(c) for c in range(NCHD)]
        d_ready = [0]   # decoder chunks whose h3all is complete

        def fillD(n=1):
            for _ in range(n):
                for c in range(d_ready[0]):
                    if next(genDs[c], None) is not None:
                        break

        # ========== Decoder recurrence, pipelined w/ phase D fillers ======
        d3s, f3s = [None] * F, [None] * F
        dp = state.tile([128, 128], DT.float8e4, tag="dp", name="dp_init")
        fp = state.tile([128, 128], DT.float32, tag="fp", name="fp_init")
        nc.vector.memset(dp[:, 0:64], 0.0)
        nc.vector.memset(fp[:, 0:64], 0.0)
        lstm_cell("de1", None, h3s[S - 1], zc[:], None, "d1", "e1",
                  h_out=dp[:, 64:128], c_out=fp[:, 64:128])
        for t in range(F):
            fillD(4)
            if t + 1 < F:
                dp_new, fp_new = lstm_pair("de2", "de1", dp, fp, None,
                                           "dp", "fp")
                d2t = dp_new[:, 0:64]
            else:
                d2t, _ = lstm_cell("de2", dp[:, 64:128], dp[:, 0:64],
                                   fp[:, 0:64], None, "d2", "e2")
            fillD(4)
            d3p = d3s[t - 1] if t else zh[:]
            f3p = f3s[t - 1] if t else zc[:]
            d3s[t], f3s[t] = lstm_cell("de3", d2t, d3p, f3p, None, "d3", "e3")
            if t + 1 < F:
                dp, fp = dp_new, fp_new
            c, o = divmod(t, CHD // BL)
            nc.vector.tensor_copy(
                h3all[c][:, :, o * BL:(o + 1) * BL],
                d3s[t].rearrange("p (k s) -> p k s", s=BL),
            )
            if o == CHD // BL - 1:
                d_ready[0] = c + 1
        # drain remaining phase D work
        for gd in genDs:
            for _ in gd:
                pass

    nsplit = _split_sync_waits(nc, limit=1)
    _log(f"split {nsplit} over-limit sync waits")
    return nc

# ---------------------------------------------------------------------------
# Host-side input prep
# ---------------------------------------------------------------------------
GATE_PERM = np.concatenate([
    np.arange(0, 2 * H),          # i, f
    np.arange(3 * H, 4 * H),      # o
    np.arange(2 * H, 3 * H),      # g
])


def _dr256(wT, scale):
    """[K, M] f32 (K = 256) -> DoubleRow fp8 layout [128, 2*M]:
    col i*M+m holds wT[128i+p, m] * scale."""
    K, M = wT.shape
    assert K == 256
    a = (wT * scale).reshape(2, 128, M).transpose(1, 0, 2).reshape(128, 2 * M)
    return np.ascontiguousarray(a).astype(F8NP)


def _drK(wT, scale):
    """[K, M] f32 (K = 256*J) -> [J*128, 2*M] fp8: row j*128+p,
    col i*M+m holds wT[256j+128i+p, m] * scale."""
    K, M = wT.shape
    J = K // 256
    a = (wT * scale).reshape(J, 2, 128, M).transpose(0, 2, 1, 3)
    return np.ascontiguousarray(a.reshape(J * 128, 2 * M)).astype(F8NP)


def prep_inputs(inputs):
    f32 = np.float32
    g = {k: np.asarray(v) for k, v in inputs.items()}
    F = int(np.asarray(g["future_step"]))

    shared = {}
    shared["w1T"] = _drK(np.asarray(g["fc_en1_w"].T, f32), SW1)
    b1B = np.zeros((128, HID // 128, 128), f32)
    b1B[0] = g["fc_en1_b"].astype(f32).reshape(HID // 128, 128) * SW1
    shared["b1B"] = b1B.reshape(128, HID).astype(BF16NP)
    shared["w2T"] = _drK(np.asarray(g["fc_en2_w"].T, f32), SW2)
    b2B = np.zeros((128, H // 128, 128), f32)
    b2B[0] = g["fc_en2_b"].astype(f32).reshape(H // 128, 128) * SW2
    shared["b2B"] = b2B.reshape(128, H).astype(BF16NP)
    for nm in CELLS:
        wih = g[nm + "_wih"][GATE_PERM]
        whh = g[nm + "_whh"][GATE_PERM]
        bsum = (g[nm + "_bih"] + g[nm + "_bhh"])[GATE_PERM].astype(f32) * SWC
        if nm != "de1":
            shared[nm + "_wihT"] = _dr256(np.asarray(wih.T, f32), SWC)
        shared[nm + "_whhT"] = _dr256(np.asarray(whh.T, f32), SWC)
        if nm == "en1":
            shared["en1_bv"] = np.ascontiguousarray(
                bsum.reshape(G // 128, 128).T)
        else:
            bB = np.zeros((128, 128), f32)
            bB[:G // 128, :] = bsum.reshape(G // 128, 128)
            shared[nm + "_bB"] = bB.astype(BF16NP)
    E = np.zeros((128, 256), f32)
    for j in range(8):
        E[j, j * 32:(j + 1) * 32] = 1.0
    shared["Em"] = E.astype(BF16NP)
    shared["wd1T"] = np.ascontiguousarray(g["fc_de1_w"].T).astype(BF16NP)
    bd1B = np.zeros((128, HID // 128, 128), f32)
    bd1B[0] = g["fc_de1_b"].astype(f32).reshape(HID // 128, 128)
    shared["bd1B"] = bd1B.reshape(128, HID).astype(BF16NP)
    shared["wd2T"] = np.ascontiguousarray(g["fc_de2_w"].T).astype(BF16NP)
    shared["bd2v"] = np.ascontiguousarray(
        g["fc_de2_b"].astype(f32).reshape(D // 128, 128).T)

    x = g["x"].astype(f32).reshape(S, B, D)
    NCHA, CHA = 4, SB // 4
    in_maps = []
    for c in range(NCORES):
        xc = x[:, c * BL:(c + 1) * BL, :].reshape(SB, D)   # row = t*BL + b
        # DoubleRow chunked layout: row j*128+p, col ch*2*CHA + i*CHA + s
        # holds x[feature 256j+128i+p, sample ch*CHA+s]
        xT = xc.T.reshape(DJ, 2, 128, NCHA, CHA).transpose(0, 2, 3, 1, 4)
        m = dict(shared)
        m["xT"] = np.ascontiguousarray(
            xT.reshape(DJ * 128, 2 * SB)).astype(F8NP)
        in_maps.append(m)
    return in_maps, F


# ---------------------------------------------------------------------------
# Execution via PJRT (axon), modeled on bass2jax.run_bass_via_pjrt
# ---------------------------------------------------------------------------
def run_spmd(nc, in_maps, n_timing=0):
    import jax
    from jax.experimental.shard_map import shard_map
    from jax.sharding import Mesh, NamedSharding, PartitionSpec

    from concourse import bass2jax

    bass2jax.install_neuronx_cc_hook()
    n_cores = len(in_maps)
    partition_name = nc.partition_id_tensor.name if nc.partition_id_tensor else None
    in_names, out_names, out_avals, zero_outs = [], [], [], []
    for alloc in nc.m.functions[0].allocations:
        if not isinstance(alloc, mybir.MemoryLocationSet):
            continue
        name = alloc.memorylocations[0].name
        if alloc.kind == "ExternalInput":
            if name != partition_name:
                in_names.append(name)
        elif alloc.kind == "ExternalOutput":
            out_names.append(name)
            shape = tuple(alloc.tensor_shape)
            dtype = mybir.dt.np(alloc.dtype)
            out_avals.append(jax.core.ShapedArray(shape, dtype))
            zero_outs.append(np.zeros(shape, dtype))
    n_params = len(in_names)
    all_in = in_names + out_names
    if partition_name is not None:
        all_in = all_in + [partition_name]
    all_in = tuple(all_in)

    def _bind(args):
        operands = list(args)
        if partition_name is not None:
            operands.append(bass2jax.partition_id_tensor())
        return bass2jax._bass_exec_p.bind(
            *operands,
            out_avals=tuple(out_avals),
            in_names=all_in,
            out_names=tuple(out_names),
            lowering_input_output_aliases=(),
            sim_require_finite=False,
            sim_require_nnan=False,
            nc=nc,
        )

    def _body(*args):
        return tuple(_bind(args))

    devices = jax.devices()[:n_cores]
    mesh = Mesh(np.asarray(devices), ("core",))
    pspec = PartitionSpec("core")
    in_specs = (pspec,) * (n_params + len(out_names))
    out_specs = (pspec,) * len(out_names)

    f1 = jax.jit(shard_map(_body, mesh=mesh, in_specs=in_specs,
                           out_specs=out_specs, check_rep=False))
    concat = [
        np.concatenate([np.asarray(in_maps[c][nm]) for c in range(n_cores)], axis=0)
        for nm in in_names
    ]
    concat += [np.concatenate([z] * n_cores, axis=0) for z in zero_outs]

    sharding = NamedSharding(mesh, pspec)
    t0 = time.perf_counter()
    dev_in = [jax.device_put(a, sharding) for a in concat]
    jax.block_until_ready(dev_in)
    _log(f"upload {sum(a.nbytes for a in concat)/1e6:.1f} MB in "
         f"{time.perf_counter()-t0:.2f}s")

    t0 = time.perf_counter()
    outs = jax.block_until_ready(f1(*dev_in))
    _log(f"first run (incl compile) {time.perf_counter()-t0:.1f}s")

    results = []
    np_outs = [np.asarray(o) for o in outs]
    for c in range(n_cores):
        r = {}
        for i, nm in enumerate(out_names):
            sh0 = out_avals[i].shape[0]
            r[nm] = np_outs[i][c * sh0:(c + 1) * sh0]
        results.append(r)

    wall = None
    if n_timing:
        ts = []
        for _ in range(n_timing):
            t0 = time.perf_counter()
            jax.block_until_ready(f1(*dev_in))
            ts.append(time.perf_counter() - t0)
        wall = min(ts)
        _log("wall per call ms: " + " ".join(f"{t*1e3:.2f}" for t in ts))
    return results, wall, (f1, dev_in)


def measure_hw_time(F, in_maps, nrep=9, reps=14, nblocks=6, block=14):
    """HW exec estimate: block design. Same-program bursts (no per-call NEFF
    swap inside a block), alternating blocks between the 1-rep and nrep-rep
    programs to cancel slow drift of the ~85ms RPC floor. Per-iter =
    (median over blocks of block-median wall) diff / (nrep - 1). The older
    interleaved-min protocol swaps NEFFs every call, which adds a
    program-size-dependent cost and ~200us of noise to the estimate."""
    import jax

    import numpy as _np

    nc1 = build_program(F, nrep=1)
    _, _, (f1, dev1) = run_spmd(nc1, in_maps)
    ncN = build_program(F, nrep=nrep)
    _, _, (fN, devN) = run_spmd(ncN, in_maps)
    meds = {1: [], nrep: []}
    for blk in range(nblocks):
        for key, f, dev in ((1, f1, dev1), (nrep, fN, devN)):
            ts = []
            for i in range(block):
                t0 = time.perf_counter()
                jax.block_until_ready(f(*dev))
                ts.append(time.perf_counter() - t0)
            meds[key].append(float(_np.median(ts[2:])))
    w1 = float(_np.median(meds[1]))
    wN = float(_np.median(meds[nrep]))
    per_iter = max((wN - w1) / (nrep - 1), 0.0)
    _log("block medians 1: " + " ".join(f"{t*1e3:.2f}" for t in meds[1]))
    _log(f"block medians {nrep}: " + " ".join(f"{t*1e3:.2f}" for t in meds[nrep]))
    _log(f"measure: w1={w1*1e3:.3f}ms w{nrep}={wN*1e3:.3f}ms -> "
         f"{per_iter*1e6:.1f}us/iter")
    return per_iter * 1e9


_LAST_TIMING = None


def kernel(**inputs) -> np.ndarray:
    t0 = time.perf_counter()
    in_maps, F = prep_inputs(inputs)
    _log(f"host prep {time.perf_counter()-t0:.2f}s")
    t0 = time.perf_counter()
    nc = build_program(F)
    _log(f"build+tile {time.perf_counter()-t0:.1f}s")
    results, _, _ = run_spmd(nc, in_maps)
    out = np.empty((F, B, 64, 64), np.float32)
    for c in range(NCORES):
        yT = results[c]["yT"]                      # [4096, F*32]
        y = yT.T.reshape(F, BL, 64, 64)
        out[:, c * BL:(c + 1) * BL] = y
    return out


# revision 25
# speedup vs baseline: 1.5181x; 1.5181x over previous
"""Trainium2 Bass kernel for nn_FC_LSTM (FC-LSTM encoder-decoder).

Strategy:
  - Data-parallel over batch: 256 samples -> 8 cores x 32 samples.
  - Feature-major layout on chip: activations stored [feature(part), sample(free)],
    weights host-transposed to [in_feat, out_feat] so every matmul is
    out[feat_out, samples] = W_T.T @ act with contraction on partitions.
  - All matmuls in fp8e4 (e4m3) with MatmulPerfMode.DoubleRow: each
    instruction contracts a 256-wide K (two 128 k-tiles packed in dim1 of
    both operands), doubling PE throughput and halving PE instruction
    count vs bf16. Weights are host-quantized with power-of-2 scales
    (max|w|*s = 128 < 240); the descale folds into the activation
    instruction's input scale (out = func(scale*psum + bias)).
  - Encoder FC stack (4096->1024->256) batched over all 20 timesteps (640
    samples per core). en1's input-side gate matmul (Wih @ z_t) is also
    precomputed densely for all t at gate scale 2^11.
  - LSTM recurrence: per step the layer-2 cell at t and the layer-1 cell at
    t+1 depend only on the previous step's pair state, so they are emitted
    as a PAIR: two PSUM banks (a shared bank trips psum start=True
    whole-bank zeroing), but one [128, 512] SBUF gates tile and
    double-width elementwise (sig/tanh ACT per bank, then pair-wide
    mul/add on DVE + Pool, one tanh(c) ACT, one fp8 h-mul). Gate psum col
    m*32+s = gate-feature 128m+p of sample s (scale 2^11); gate order
    host-permuted to [i, f, o, g]. Cell biases (x2^11) are injected with a
    single K=128 bf16 matmul against a constant indicator matrix; FC biases
    likewise ride in the psum via a bias-in-row-0 bf16 matmul against a
    ones column, so FC relu+descale is a single 2-op DVE tensor_scalar
    (no ACT involvement outside the cells and the output tanh).
    h state is stored fp8, c state fp32.
  - Decoder FC stack (256->1024->4096) batched over all future steps.
  - The FC phases are emitted as generators whose matmul blocks interleave
    into the recurrence as PE gap fillers; DMAs are emitted in first-use
    order (small bias/ones constants first: they gate FC psum-group
    starts).
  - measure_hw_time uses a block design: bursts of same-program calls
    (per-call NEFF swap costs otherwise contaminate the estimate),
    alternating blocks of the 1-rep and 9-rep programs, difference of
    median block-medians.
"""

import time
from contextlib import ExitStack

import ml_dtypes
import numpy as np

import concourse.bass as bass
import concourse.mybir as mybir
import concourse.tile as tile

BF16NP = ml_dtypes.bfloat16
F8NP = ml_dtypes.float8_e4m3
AF = mybir.ActivationFunctionType
DT = mybir.dt
DR = mybir.MatmulPerfMode.DoubleRow

S = 20          # encoder sequence length
B = 256         # global batch
NCORES = 8
BL = B // NCORES  # 32 samples per core
H = 256         # LSTM hidden
G = 4 * H       # 1024 gate features
D = 4096        # input feature dim (64*64)
HID = 1024      # FC hidden
SB = S * BL     # 640 encoder samples per core

# power-of-2 quant scales: max|w| * SW = 128 (e4m3 max finite = 240)
SW1 = 2.0 ** 13    # fc_en1 w ~ U(+-2^-6)
SW2 = 2.0 ** 12    # fc_en2 w ~ U(+-2^-5)
SWC = 2.0 ** 11    # cell weights ~ U(+-2^-4); gate psum scale
SWD1 = 2.0 ** 11   # fc_de1 w ~ U(+-2^-4)
SWD2 = 2.0 ** 12   # fc_de2 w ~ U(+-2^-5)

VERBOSE = True


def _log(*a):
    if VERBOSE:
        print("[kernel]", *a, flush=True)


# ---------------------------------------------------------------------------
# Workaround: walrus CoreV3 setupSyncWait allows only 1 sync wait on the
# TileContext exit Drain. Split its waits across multiple drain instructions.
# ---------------------------------------------------------------------------
def _patched_drain_and_barrier(self, tick_clock, wait_clock):
    nc = self.nc
    drain_inst = nc.sync.drain()
    wait_clock.add_sem_waits(
        drain_inst.ins, tile.ScopedClock({None: tick_clock.global_clock})
    )
    inst = drain_inst.ins
    si = inst.sync_info
    waits = list(si.on_wait) if si is not None and si.on_wait else []
    MAXW = 1
    if len(waits) > MAXW:
        si.on_wait = waits[:MAXW]
        for i in range(MAXW, len(waits), MAXW):
            d2 = nc.sync.drain()
            i2 = d2.ins
            si2 = i2.sync_info
            if si2 is None:
                i2.sync_info = type(si)(on_wait=waits[i : i + MAXW], on_update=[])
            else:
                si2.on_wait = list(si2.on_wait or []) + waits[i : i + MAXW]

    nc.all_engine_barrier()
    assert self.sems is not None
    popped = nc._tile_sem_poison_stack.pop()
    assert popped is self._sem_poison
    nc.clear_and_free_semaphores(list(self.sems.allocated().values()))
    nc.all_engine_barrier()


tile.TileContext._drain_and_barrier = _patched_drain_and_barrier


def _split_sync_waits(nc, limit=1):
    """walrus setupSyncWait rejects >2 sem waits per instruction: move excess
    waits onto same-engine NoOps spliced just before the instruction."""
    ctr = [0]
    SyncInfo = None
    for f in nc.m.functions:
        for bb in f.blocks:
            out = []
            for inst in bb.instructions:
                si = inst.sync_info
                waits = list(si.on_wait) if si is not None and si.on_wait else []
                if len(waits) > limit:
                    if SyncInfo is None:
                        SyncInfo = type(si)
                    extras = waits[: len(waits) - limit]
                    si.on_wait = waits[len(waits) - limit:]
                    for i in range(0, len(extras), limit):
                        ctr[0] += 1
                        nop = mybir.InstNoOp(name=f"ws_{ctr[0]}", ins=[], outs=[])
                        nop.engine = inst.engine
                        nop.sync_info = SyncInfo(
                            on_wait=extras[i : i + limit], on_update=[]
                        )
                        out.append(nop)
                out.append(inst)
            bb.instructions[:] = out
    return ctr[0]


# ---------------------------------------------------------------------------
# Program builder
# ---------------------------------------------------------------------------
CELLS = ["en1", "en2", "en3", "de1", "de2", "de3"]
DJ = D // 256       # 16 k-tile-pairs of the 4096 input dim
HJ = HID // 256     # 4 k-tile-pairs of the 1024 hidden dim


def build_program(F: int, nrep: int = 1) -> bass.Bass:
    FB = F * BL  # decoder samples per core
    assert FB <= 512 and FB % 2 == 0
    nc = bass.Bass()

    # --- DRAM tensors (all fp8 weight layouts are DoubleRow-paired:
    #     row j*128+p, col i*M+m holds W[256j+128i+p, m]) ---
    xT = nc.dram_tensor("xT", [DJ * 128, 2 * SB], DT.float8e4, kind="ExternalInput")
    w1T = nc.dram_tensor("w1T", [DJ * 128, 2 * HID], DT.float8e4, kind="ExternalInput")
    b1B = nc.dram_tensor("b1B", [128, HID], DT.bfloat16, kind="ExternalInput")
    w2T = nc.dram_tensor("w2T", [HJ * 128, 2 * H], DT.float8e4, kind="ExternalInput")
    b2B = nc.dram_tensor("b2B", [128, H], DT.bfloat16, kind="ExternalInput")
    cellT = {}
    for nm in CELLS:
        ins = {}
        if nm != "de1":
            ins["wih"] = nc.dram_tensor(f"{nm}_wihT", [128, 2 * G], DT.float8e4,
                                        kind="ExternalInput")
        ins["whh"] = nc.dram_tensor(f"{nm}_whhT", [128, 2 * G], DT.float8e4,
                                    kind="ExternalInput")
        if nm == "en1":
            ins["bv"] = nc.dram_tensor("en1_bv", [128, G // 128], DT.float32,
                                       kind="ExternalInput")
        else:
            ins["bB"] = nc.dram_tensor(f"{nm}_bB", [128, 128], DT.bfloat16,
                                       kind="ExternalInput")
        cellT[nm] = ins
    Em = nc.dram_tensor("Em", [128, 256], DT.bfloat16, kind="ExternalInput")
    # decoder head stays bf16: its pre-tanh output is tiny relative to the
    # summand magnitudes (heavy cancellation), so fp8 there blows the error
    # budget (measured: fp8 fc_de2 alone -> 3.3e-2 rel err).
    wd1T = nc.dram_tensor("wd1T", [H, HID], DT.bfloat16, kind="ExternalInput")
    bd1B = nc.dram_tensor("bd1B", [128, HID], DT.bfloat16, kind="ExternalInput")
    wd2T = nc.dram_tensor("wd2T", [HID, D], DT.bfloat16, kind="ExternalInput")
    bd2v = nc.dram_tensor("bd2v", [128, D // 128], DT.float32, kind="ExternalInput")
    yT = nc.dram_tensor("yT", [D, FB], DT.float32, kind="ExternalOutput")

    NCHA = 4                      # phase A chunks (5 encoder steps each)
    CHA = SB // NCHA              # 160 samples
    SPC = S // NCHA               # steps per chunk
    NCHD = 2                      # phase D chunks
    CHD = FB // NCHD

    def pair(ap):
        """[128, 2*N] fp8 AP -> [128, 2, N] DoubleRow operand view."""
        return ap.rearrange("p (i n) -> p i n", i=2)

    with tile.TileContext(nc) as tc:
     for rep in range(nrep):
      with ExitStack() as ctx:
        const = ctx.enter_context(tc.tile_pool(name="const", bufs=1))
        gi1p = ctx.enter_context(tc.tile_pool(name="gi1p", bufs=1))
        state = ctx.enter_context(tc.tile_pool(name="state", bufs=3))
        gates = ctx.enter_context(tc.tile_pool(name="gates", bufs=4))
        outp = ctx.enter_context(tc.tile_pool(name="outp", bufs=4))
        psum = ctx.enter_context(tc.tile_pool(name="psum", bufs=8, space="PSUM"))

        uid = [0]

        def PS():
            uid[0] += 1
            return psum.tile([128, 512], DT.float32, tag="ps",
                             name=f"ps{uid[0]}")

        def dma_in2(pool, dram, tag):
            t = pool.tile(list(dram.shape), dram.dtype, tag=tag)
            nc.sync.dma_start(t[:], dram[:])
            return t

        def gi_ap(t):
            c, o = divmod(t, SPC)
            return gi1[c][:, :, o * BL:(o + 1) * BL]

        # ========== Phase A generator: per-k-tile weight/x DMAs, chunked ===
        pA_ctx = ExitStack()
        pA = pA_ctx.enter_context(tc.tile_pool(name="phaseA", bufs=1))

        # DMA emission ordered by first use: w1 + x chunk 0 feed the FC chain
        # immediately; encoder cell weights arrive next; remaining x chunks
        # stream during the early recurrence; decoder weights last.
        # tiny constants first: the FC1 bias matmuls (group starts) need
        # b1/ones immediately; don't queue them behind 5MB of w1/x stream
        b1_sb = const.tile([128, HID // 128, 128], DT.bfloat16, tag="b1B")
        nc.sync.dma_start(b1_sb[:], b1B.rearrange("p (m q) -> p m q", q=128))
        ones_sb = const.tile([128, 160], DT.bfloat16, tag="ones")
        nc.vector.memset(ones_sb[:], 1.0)
        b2_sb = const.tile([128, H // 128, 128], DT.bfloat16, tag="b2B")
        nc.sync.dma_start(b2_sb[:], b2B.rearrange("p (m q) -> p m q", q=128))
        x_kc = [[None] * NCHA for _ in range(DJ)]
        w1_k = []
        for j in range(DJ):
            wk = pA.tile([128, 2 * HID], DT.float8e4, tag=f"w1_{j}",
                         name=f"w1_{j}")
            nc.sync.dma_start(wk[:], w1T[j * 128:(j + 1) * 128, :])
            w1_k.append(wk)
            xk = pA.tile([128, 2 * CHA], DT.float8e4, tag=f"x{j}_0",
                         name=f"x{j}_0")
            nc.sync.dma_start(xk[:], xT[j * 128:(j + 1) * 128, 0:2 * CHA])
            x_kc[j][0] = xk

        # encoder-phase constants
        w2_sb = const.tile([128, HJ, 2 * H], DT.float8e4, tag="w2")
        nc.sync.dma_start(w2_sb[:], w2T.rearrange("(j p) c -> p j c", p=128))
        cell_sb = {}
        for nm in ["en1", "en2", "en3"]:
            e = {}
            e["wih"] = dma_in2(const, cellT[nm]["wih"], f"{nm}_wih")
            e["whh"] = dma_in2(const, cellT[nm]["whh"], f"{nm}_whh")
            if nm == "en1":
                e["bv"] = dma_in2(const, cellT[nm]["bv"], "en1_bv")
            else:
                e["bB"] = dma_in2(const, cellT[nm]["bB"], f"{nm}_bB")
            cell_sb[nm] = e
        E_sb = dma_in2(const, Em, "Em")

        # remaining x chunks
        for c in range(1, NCHA):
            for j in range(DJ):
                xk = pA.tile([128, 2 * CHA], DT.float8e4, tag=f"x{j}_{c}",
                             name=f"x{j}_{c}")
                nc.sync.dma_start(
                    xk[:], xT[j * 128:(j + 1) * 128, c * 2 * CHA:(c + 1) * 2 * CHA])
                x_kc[j][c] = xk

        # decoder-phase constants
        for nm in ["de1", "de2", "de3"]:
            e = {}
            if "wih" in cellT[nm]:
                e["wih"] = dma_in2(const, cellT[nm]["wih"], f"{nm}_wih")
            e["whh"] = dma_in2(const, cellT[nm]["whh"], f"{nm}_whh")
            e["bB"] = dma_in2(const, cellT[nm]["bB"], f"{nm}_bB")
            cell_sb[nm] = e
        wd1_sb = const.tile([128, H // 128, HID], DT.bfloat16, tag="wd1")
        nc.sync.dma_start(wd1_sb[:], wd1T.rearrange("(o p) m -> p o m", p=128))
        bd1_sb = const.tile([128, HID // 128, 128], DT.bfloat16, tag="bd1B")
        nc.sync.dma_start(bd1_sb[:], bd1B.rearrange("p (m q) -> p m q", q=128))
        bd2_sb = dma_in2(const, bd2v, "bd2v")
        zh = const.tile([128, 2 * BL], DT.float8e4, tag="zh")
        zc = const.tile([128, 2 * BL], DT.float32, tag="zc")
        nc.vector.memset(zh[:], 0.0)
        nc.vector.memset(zc[:], 0.0)
        h3all = [const.tile([128, H // 128, CHD], DT.bfloat16, tag=f"h3all{c}",
                            name=f"h3all{c}")
                 for c in range(NCHD)]
        gi1 = [gi1p.tile([128, G // 128, CHA], DT.float32, tag=f"gi1_{c}",
                         name=f"gi1_{c}")
               for c in range(NCHA)]

        def phaseA_gen():
            """Emits all of phase A; yields chunks_done after each MM block."""
            for c in range(NCHA):
                z1c = pA.tile([128, HID // 128, CHA], DT.float8e4,
                              tag=f"z1_{c}", name=f"z1_{c}")
                zc_ = pA.tile([128, 2, CHA], DT.float8e4,
                              tag=f"z_{c}", name=f"z_{c}")
                if c == 0:
                    # chunk 0 runs while x/w1 DMAs land: j-outer so each
                    # arriving k-pair tile is consumed immediately
                    for half in range(2):
                        ms = range(half * 4, half * 4 + 4)
                        pss = [PS()[:, :CHA] for _ in ms]
                        for mi, m in enumerate(ms):
                            nc.tensor.matmul(
                                pss[mi], b1_sb[:, m, :], ones_sb[:, :CHA],
                                start=True, stop=False, skip_group_check=True)
                        for j in range(DJ):
                            wv = pair(w1_k[j][:])
                            xv = pair(x_kc[j][0][:])
                            for mi, m in enumerate(ms):
                                nc.tensor.matmul(
                                    pss[mi], wv[:, :, m * 128:(m + 1) * 128],
                                    xv,
                                    start=False, stop=(j == DJ - 1),
                                    perf_mode=DR,
                                )
                            if j % 4 == 3:
                                yield c
                        for mi, m in enumerate(ms):
                            nc.vector.tensor_scalar(
                                z1c[:, m, :], pss[mi], 1.0 / SW1, 0.0,
                                mybir.AluOpType.mult, mybir.AluOpType.max)
                else:
                    for m in range(HID // 128):
                        ps = PS()[:, :CHA]
                        nc.tensor.matmul(
                            ps, b1_sb[:, m, :], ones_sb[:, :CHA],
                            start=True, stop=False, skip_group_check=True)
                        for j0 in range(0, DJ, 4):
                            for j in range(j0, j0 + 4):
                                nc.tensor.matmul(
                                    ps,
                                    pair(w1_k[j][:])[:, :, m * 128:(m + 1) * 128],
                                    pair(x_kc[j][c][:]),
                                    start=False, stop=(j == DJ - 1),
                                    perf_mode=DR,
                                )
                            yield c
                        nc.vector.tensor_scalar(
                            z1c[:, m, :], ps, 1.0 / SW1, 0.0,
                            mybir.AluOpType.mult, mybir.AluOpType.max)
                for m in range(H // 128):
                    ps = PS()[:, :CHA]
                    nc.tensor.matmul(
                        ps, b2_sb[:, m, :], ones_sb[:, :CHA],
                        start=True, stop=False, skip_group_check=True)
                    for j in range(HJ):
                        nc.tensor.matmul(
                            ps,
                            pair(w2_sb[:, j, :])[:, :, m * 128:(m + 1) * 128],
                            z1c[:, 2 * j:2 * j + 2, :],
                            start=False, stop=(j == HJ - 1),
                            perf_mode=DR,
                        )
                    nc.vector.tensor_scalar(
                        zc_[:, m, :], ps, 1.0 / SW2, 0.0,
                        mybir.AluOpType.mult, mybir.AluOpType.max)
                    yield c
                wihv = pair(cell_sb["en1"]["wih"][:])
                for m in range(G // 128):
                    ps = PS()[:, :CHA]
                    nc.tensor.matmul(
                        ps, wihv[:, :, m * 128:(m + 1) * 128], zc_[:],
                        start=True, stop=True, perf_mode=DR,
                    )
                    nc.vector.tensor_scalar_add(
                        gi1[c][:, m, :], ps, cell_sb["en1"]["bv"][:, m:m + 1])
                    if m % 2 == 1:
                        yield c + (m == G // 128 - 1)
            while True:
                yield NCHA + 1

        genA = phaseA_gen()
        a_done = [0]

        def fillA(n=1):
            for _ in range(n):
                a_done[0] = max(a_done[0], next(genA))

        def needA(chunks):
            while a_done[0] < chunks + 1:
                fillA()

        # ========== LSTM cell (single) ====================================
        def lstm_cell(nm, x_in, h_prev, c_prev, gi, htag, ctag,
                      h_out=None, c_out=None):
            e = cell_sb[nm]
            ps = PS()[:, :256]
            groups = []
            if gi is None:
                nc.tensor.matmul(ps, e["bB"][:], E_sb[:], start=True, stop=False)
                started = True
            else:
                started = False
            if x_in is not None:
                groups.append((pair(e["wih"][:]), pair(x_in)))
            groups.append((pair(e["whh"][:]), pair(h_prev)))
            ng = len(groups)
            for gidx, (wv, rv) in enumerate(groups):
                for m in range(8):
                    nc.tensor.matmul(
                        ps[:, m * 32:(m + 1) * 32],
                        wv[:, :, m * 128:(m + 1) * 128],
                        rv,
                        start=(not started and gidx == 0),
                        stop=(gidx == ng - 1),
                        perf_mode=DR,
                        skip_group_check=True,
                    )
            if gi is not None:
                pv = ps.rearrange("p (m s) -> p m s", s=32)
                nc.vector.tensor_add(pv, pv, gi)
            g = gates.tile([128, 256], DT.float32, tag="g", name=f"g{uid[0]}")
            nc.scalar.activation(g[:, 0:192], ps[:, 0:192], AF.Sigmoid,
                                 scale=1.0 / SWC)
            nc.scalar.activation(g[:, 192:256], ps[:, 192:256], AF.Tanh,
                                 scale=1.0 / SWC)
            # packed: i: 0..63, f: 64..127, o: 128..191, g: 192..255
            t1 = gates.tile([128, 64], DT.float32, tag="t1", name=f"t1{uid[0]}")
            nc.gpsimd.tensor_mul(t1[:], g[:, 0:64], g[:, 192:256])
            c2 = gates.tile([128, 64], DT.float32, tag="c2", name=f"c2{uid[0]}")
            nc.vector.tensor_mul(c2[:], g[:, 64:128], c_prev)
            if c_out is None:
                c_new = state.tile([128, 64], DT.float32, tag=ctag,
                                   name=f"{ctag}{uid[0]}")
                c_out = c_new[:]
            else:
                c_new = None
            nc.vector.tensor_add(c_out, c2[:], t1[:])
            th = gates.tile([128, 64], DT.float32, tag="th", name=f"th{uid[0]}")
            nc.scalar.activation(th[:], c_out, AF.Tanh)
            if h_out is None:
                h_new = state.tile([128, 64], DT.float8e4, tag=htag,
                                   name=f"{htag}{uid[0]}")
                h_out = h_new[:]
            else:
                h_new = None
            nc.vector.tensor_mul(h_out, g[:, 128:192], th[:])
            return h_out, c_out

        # ========== Paired LSTM cells (A = layer-2 cell at t, B = layer-1
        # cell at t+1; both read only pair_prev, so their 512-col gate psums
        # share one bank and the elementwise chain runs at double width) ====
        def lstm_pair(nmA, nmB, hp_prev, cp_prev, giB, htag, ctag):
            eA, eB = cell_sb[nmA], cell_sb[nmB]
            psA = PS()[:, :256]
            psB = PS()[:, :256]
            nc.tensor.matmul(psA, eA["bB"][:], E_sb[:],
                             start=True, stop=False, skip_group_check=True)
            bstart = False
            if giB is None:
                nc.tensor.matmul(psB, eB["bB"][:], E_sb[:],
                                 start=True, stop=False, skip_group_check=True)
            else:
                bstart = True
            xA = pair(hp_prev[:, 64:128])   # h of layer-1 cell at t
            hA = pair(hp_prev[:, 0:64])     # layer-2 cell's own h at t-1
            hB = pair(hp_prev[:, 64:128])   # layer-1 cell's own h at t
            for m in range(8):
                sl = slice(m * 32, (m + 1) * 32)
                nc.tensor.matmul(psA[:, sl],
                                 pair(eA["wih"][:])[:, :, m * 128:(m + 1) * 128],
                                 xA, start=False, stop=False,
                                 perf_mode=DR, skip_group_check=True)
            for m in range(8):
                sl = slice(m * 32, (m + 1) * 32)
                nc.tensor.matmul(psA[:, sl],
                                 pair(eA["whh"][:])[:, :, m * 128:(m + 1) * 128],
                                 hA, start=False, stop=True,
                                 perf_mode=DR, skip_group_check=True)
            for m in range(8):
                sl = slice(m * 32, (m + 1) * 32)
                nc.tensor.matmul(psB[:, sl],
                                 pair(eB["whh"][:])[:, :, m * 128:(m + 1) * 128],
                                 hB, start=bstart, stop=True,
                                 perf_mode=DR, skip_group_check=True)
            if giB is not None:
                pv = psB.rearrange("p (m s) -> p m s", s=32)
                nc.vector.tensor_add(pv, pv, giB)
            # gates tile packs both cells: [ifo|g] at 0 (A) and 256 (B)
            g = gates.tile([128, 512], DT.float32, tag="gp", name=f"gp{uid[0]}")
            cp_new = state.tile([128, 128], DT.float32, tag=ctag,
                                name=f"{ctag}{uid[0]}")
            th = gates.tile([128, 128], DT.float32, tag="thp",
                            name=f"thp{uid[0]}")
            hp_new = state.tile([128, 128], DT.float8e4, tag=htag,
                                name=f"{htag}{uid[0]}")
            for o0, psx in ((0, psA), (256, psB)):
                nc.scalar.activation(g[:, o0:o0 + 192], psx[:, 0:192],
                                     AF.Sigmoid, scale=1.0 / SWC)
                nc.scalar.activation(g[:, o0 + 192:o0 + 256], psx[:, 192:256],
                                     AF.Tanh, scale=1.0 / SWC)
            # pair-wide elementwise on SBUF (strided [128, 2, 64] views)
            gv = g.rearrange("p (two x) -> p two x", two=2)
            cpv = cp_prev.rearrange("p (two s) -> p two s", two=2)
            t1 = gates.tile([128, 2, 64], DT.float32, tag="t1p",
                            name=f"t1p{uid[0]}")
            nc.gpsimd.tensor_mul(t1[:], gv[:, :, 0:64], gv[:, :, 192:256])
            c2 = gates.tile([128, 2, 64], DT.float32, tag="c2p",
                            name=f"c2p{uid[0]}")
            nc.vector.tensor_mul(c2[:], gv[:, :, 64:128], cpv)
            cnv = cp_new.rearrange("p (two s) -> p two s", two=2)
            nc.vector.tensor_add(cnv, c2[:], t1[:])
            nc.scalar.activation(th[:], cp_new[:], AF.Tanh)
            hnv = hp_new.rearrange("p (two s) -> p two s", two=2)
            nc.vector.tensor_mul(hnv, gv[:, :, 128:192],
                                 th.rearrange("p (two s) -> p two s", two=2))
            return hp_new, cp_new

        # ========== Phase D transition (callable mid-encoder) =============
        wd2_k = []
        pD_box = []

        def ensure_pD():
            if pD_box:
                return
            pA_ctx.close()  # free phase A SBUF before loading decoder weights
            pD = ctx.enter_context(tc.tile_pool(name="phaseD", bufs=1))
            pD_box.append(pD)
            for k in range(HID // 128):
                wk = pD.tile([128, D], DT.bfloat16, tag=f"wd2_{k}",
                             name=f"wd2_{k}")
                nc.sync.dma_start(wk[:], wd2T[k * 128:(k + 1) * 128, :])
                wd2_k.append(wk)

        # ========== Encoder recurrence, pipelined w/ phase A fillers ======
        # pair state: hp = [h2(t-1), h1(t)] (fp8), cp likewise (fp32)
        needA(0)
        h3s, c3s = [None] * S, [None] * S
        hp = state.tile([128, 128], DT.float8e4, tag="hp", name="hp_init")
        cp = state.tile([128, 128], DT.float32, tag="cp", name="cp_init")
        nc.vector.memset(hp[:, 0:64], 0.0)
        nc.vector.memset(cp[:, 0:64], 0.0)
        lstm_cell("en1", None, zh[:], zc[:], gi_ap(0), "h1", "c1",
                  h_out=hp[:, 64:128], c_out=cp[:, 64:128])
        for t in range(S):
            fillA(3)
            if t + 1 < S:
                needA((t + 1) // SPC)
                hp_new, cp_new = lstm_pair("en2", "en1", hp, cp,
                                           gi_ap(t + 1), "hp", "cp")
                h2t = hp_new[:, 0:64]
            else:
                h2t, _ = lstm_cell("en2", hp[:, 64:128], hp[:, 0:64],
                                   cp[:, 0:64], None, "h2", "c2")
            fillA(3)
            h3p = h3s[t - 1] if t else zh[:]
            c3p = c3s[t - 1] if t else zc[:]
            h3s[t], c3s[t] = lstm_cell("en3", h2t, h3p, c3p, None, "h3", "c3")
            if t + 1 < S:
                hp, cp = hp_new, cp_new
            if t == S - 5:
                needA(NCHA)  # drain phase A now so decoder weights can load
                ensure_pD()

        ensure_pD()
        pD = pD_box[0]

        def phaseD_gen(c):
            y1c = pD.tile([128, HID // 128, CHD], DT.bfloat16,
                          tag=f"y1_{c}", name=f"y1_{c}")
            for m in range(HID // 128):
                ps = PS()[:, :CHD]
                nc.tensor.matmul(
                    ps, bd1_sb[:, m, :], ones_sb[:, :CHD],
                    start=True, stop=False, skip_group_check=True)
                for k in range(H // 128):
                    nc.tensor.matmul(
                        ps, wd1_sb[:, k, m * 128:(m + 1) * 128],
                        h3all[c][:, k, :],
                        start=False, stop=(k == H // 128 - 1),
                    )
                nc.vector.tensor_scalar(
                    y1c[:, m, :], ps, 1.0, 0.0,
                    mybir.AluOpType.mult, mybir.AluOpType.max)
                if m % 2 == 1:
                    yield
            for m in range(D // 128):
                ps = PS()[:, :CHD]
                for k in range(HID // 128):
                    nc.tensor.matmul(
                        ps, wd2_k[k][:, m * 128:(m + 1) * 128],
                        y1c[:, k, :],
                        start=(k == 0), stop=(k == HID // 128 - 1),
                    )
                o_sb = outp.tile([128, CHD], DT.float32, tag="o",
                                 name=f"o{uid[0]}")
                nc.scalar.activation(o_sb[:], ps, AF.Tanh,
                                     bias=bd2_sb[:, m:m + 1])
                nc.sync.dma_start(
                    yT[m * 128:(m + 1) * 128, c * CHD:(c + 1) * CHD],
                    o_sb[:])
                yield

        genDs = [phaseD_gen(c) for c in range(NCHD)]
        d_ready = [0]   # decoder chunks whose h3all is complete

        def fillD(n=1):
            for _ in range(n):
                for c in range(d_ready[0]):
                    if next(genDs[c], None) is not None:
                        break

        # ========== Decoder recurrence, pipelined w/ phase D fillers ======
        d3s, f3s = [None] * F, [None] * F
        dp = state.tile([128, 128], DT.float8e4, tag="dp", name="dp_init")
        fp = state.tile([128, 128], DT.float32, tag="fp", name="fp_init")
        nc.vector.memset(dp[:, 0:64], 0.0)
        nc.vector.memset(fp[:, 0:64], 0.0)
        lstm_cell("de1", None, h3s[S - 1], zc[:], None, "d1", "e1",
                  h_out=dp[:, 64:128], c_out=fp[:, 64:128])
        for t in range(F):
            fillD(4)
            if t + 1 < F:
                dp_new, fp_new = lstm_pair("de2", "de1", dp, fp, None,
                                           "dp", "fp")
                d2t = dp_new[:, 0:64]
            else:
                d2t, _ = lstm_cell("de2", dp[:, 64:128], dp[:, 0:64],
                                   fp[:, 0:64], None, "d2", "e2")
            fillD(4)
            d3p = d3s[t - 1] if t else zh[:]
            f3p = f3s[t - 1] if t else zc[:]
            d3s[t], f3s[t] = lstm_cell("de3", d2t, d3p, f3p, None, "d3", "e3")
            if t + 1 < F:
                dp, fp = dp_new, fp_new
            c, o = divmod(t, CHD // BL)
            nc.vector.tensor_copy(
                h3all[c][:, :, o * BL:(o + 1) * BL],
                d3s[t].rearrange("p (k s) -> p k s", s=BL),
            )
            if o == CHD // BL - 1:
                d_ready[0] = c + 1
        # drain remaining phase D work
        for gd in genDs:
            for _ in gd:
                pass

    nsplit = _split_sync_waits(nc, limit=1)
    _log(f"split {nsplit} over-limit sync waits")
    return nc

# ---------------------------------------------------------------------------
# Host-side input prep
# ---------------------------------------------------------------------------
GATE_PERM = np.concatenate([
    np.arange(0, 2 * H),          # i, f
    np.arange(3 * H, 4 * H),      # o
    np.arange(2 * H, 3 * H),      # g
])


def _dr256(wT, scale):
    """[K, M] f32 (K = 256) -> DoubleRow fp8 layout [128, 2*M]:
    col i*M+m holds wT[128i+p, m] * scale."""
    K, M = wT.shape
    assert K == 256
    a = (wT * scale).reshape(2, 128, M).transpose(1, 0, 2).reshape(128, 2 * M)
    return np.ascontiguousarray(a).astype(F8NP)


def _drK(wT, scale):
    """[K, M] f32 (K = 256*J) -> [J*128, 2*M] fp8: row j*128+p,
    col i*M+m holds wT[256j+128i+p, m] * scale."""
    K, M = wT.shape
    J = K // 256
    a = (wT * scale).reshape(J, 2, 128, M).transpose(0, 2, 1, 3)
    return np.ascontiguousarray(a.reshape(J * 128, 2 * M)).astype(F8NP)


def prep_inputs(inputs):
    f32 = np.float32
    g = {k: np.asarray(v) for k, v in inputs.items()}
    F = int(np.asarray(g["future_step"]))

    shared = {}
    shared["w1T"] = _drK(np.asarray(g["fc_en1_w"].T, f32), SW1)
    b1B = np.zeros((128, HID // 128, 128), f32)
    b1B[0] = g["fc_en1_b"].astype(f32).reshape(HID // 128, 128) * SW1
    shared["b1B"] = b1B.reshape(128, HID).astype(BF16NP)
    shared["w2T"] = _drK(np.asarray(g["fc_en2_w"].T, f32), SW2)
    b2B = np.zeros((128, H // 128, 128), f32)
    b2B[0] = g["fc_en2_b"].astype(f32).reshape(H // 128, 128) * SW2
    shared["b2B"] = b2B.reshape(128, H).astype(BF16NP)
    for nm in CELLS:
        wih = g[nm + "_wih"][GATE_PERM]
        whh = g[nm + "_whh"][GATE_PERM]
        bsum = (g[nm + "_bih"] + g[nm + "_bhh"])[GATE_PERM].astype(f32) * SWC
        if nm != "de1":
            shared[nm + "_wihT"] = _dr256(np.asarray(wih.T, f32), SWC)
        shared[nm + "_whhT"] = _dr256(np.asarray(whh.T, f32), SWC)
        if nm == "en1":
            shared["en1_bv"] = np.ascontiguousarray(
                bsum.reshape(G // 128, 128).T)
        else:
            bB = np.zeros((128, 128), f32)
            bB[:G // 128, :] = bsum.reshape(G // 128, 128)
            shared[nm + "_bB"] = bB.astype(BF16NP)
    E = np.zeros((128, 256), f32)
    for j in range(8):
        E[j, j * 32:(j + 1) * 32] = 1.0
    shared["Em"] = E.astype(BF16NP)
    shared["wd1T"] = np.ascontiguousarray(g["fc_de1_w"].T).astype(BF16NP)
    bd1B = np.zeros((128, HID // 128, 128), f32)
    bd1B[0] = g["fc_de1_b"].astype(f32).reshape(HID // 128, 128)
    shared["bd1B"] = bd1B.reshape(128, HID).astype(BF16NP)
    shared["wd2T"] = np.ascontiguousarray(g["fc_de2_w"].T).astype(BF16NP)
    shared["bd2v"] = np.ascontiguousarray(
        g["fc_de2_b"].astype(f32).reshape(D // 128, 128).T)

    x = g["x"].astype(f32).reshape(S, B, D)
    NCHA, CHA = 4, SB // 4
    in_maps = []
    for c in range(NCORES):
        xc = x[:, c * BL:(c + 1) * BL, :].reshape(SB, D)   # row = t*BL + b
        # DoubleRow chunked layout: row j*128+p, col ch*2*CHA + i*CHA + s
        # holds x[feature 256j+128i+p, sample ch*CHA+s]
        xT = xc.T.reshape(DJ, 2, 128, NCHA, CHA).transpose(0, 2, 3, 1, 4)
        m = dict(shared)
        m["xT"] = np.ascontiguousarray(
            xT.reshape(DJ * 128, 2 * SB)).astype(F8NP)
        in_maps.append(m)
    return in_maps, F


# ---------------------------------------------------------------------------
# Execution via PJRT (axon), modeled on bass2jax.run_bass_via_pjrt
# ---------------------------------------------------------------------------
def run_spmd(nc, in_maps, n_timing=0):
    import jax
    from jax.experimental.shard_map import shard_map
    from jax.sharding import Mesh, NamedSharding, PartitionSpec

    from concourse import bass2jax

    bass2jax.install_neuronx_cc_hook()
    n_cores = len(in_maps)
    partition_name = nc.partition_id_tensor.name if nc.partition_id_tensor else None
    in_names, out_names, out_avals, zero_outs = [], [], [], []
    for alloc in nc.m.functions[0].allocations:
        if not isinstance(alloc, mybir.MemoryLocationSet):
            continue
        name = alloc.memorylocations[0].name
        if alloc.kind == "ExternalInput":
            if name != partition_name:
                in_names.append(name)
        elif alloc.kind == "ExternalOutput":
            out_names.append(name)
            shape = tuple(alloc.tensor_shape)
            dtype = mybir.dt.np(alloc.dtype)
            out_avals.append(jax.core.ShapedArray(shape, dtype))
            zero_outs.append(np.zeros(shape, dtype))
    n_params = len(in_names)
    all_in = in_names + out_names
    if partition_name is not None:
        all_in = all_in + [partition_name]
    all_in = tuple(all_in)

    def _bind(args):
        operands = list(args)
        if partition_name is not None:
            operands.append(bass2jax.partition_id_tensor())
        return bass2jax._bass_exec_p.bind(
            *operands,
            out_avals=tuple(out_avals),
            in_names=all_in,
            out_names=tuple(out_names),
            lowering_input_output_aliases=(),
            sim_require_finite=False,
            sim_require_nnan=False,
            nc=nc,
        )

    def _body(*args):
        return tuple(_bind(args))

    devices = jax.devices()[:n_cores]
    mesh = Mesh(np.asarray(devices), ("core",))
    pspec = PartitionSpec("core")
    in_specs = (pspec,) * (n_params + len(out_names))
    out_specs = (pspec,) * len(out_names)

    f1 = jax.jit(shard_map(_body, mesh=mesh, in_specs=in_specs,
                           out_specs=out_specs, check_rep=False))
    concat = [
        np.concatenate([np.asarray(in_maps[c][nm]) for c in range(n_cores)], axis=0)
        for nm in in_names
    ]
    concat += [np.concatenate([z] * n_cores, axis=0) for z in zero_outs]

    sharding = NamedSharding(mesh, pspec)
    t0 = time.perf_counter()
    dev_in = [jax.device_put(a, sharding) for a in concat]
    jax.block_until_ready(dev_in)
    _log(f"upload {sum(a.nbytes for a in concat)/1e6:.1f} MB in "
         f"{time.perf_counter()-t0:.2f}s")

    t0 = time.perf_counter()
    outs = jax.block_until_ready(f1(*dev_in))
    _log(f"first run (incl compile) {time.perf_counter()-t0:.1f}s")

    results = []
    np_outs = [np.asarray(o) for o in outs]
    for c in range(n_cores):
        r = {}
        for i, nm in enumerate(out_names):
            sh0 = out_avals[i].shape[0]
            r[nm] = np_outs[i][c * sh0:(c + 1) * sh0]
        results.append(r)

    wall = None
    if n_timing:
        ts = []
        for _ in range(n_timing):
            t0 = time.perf_counter()
            jax.block_until_ready(f1(*dev_in))
            ts.append(time.perf_counter() - t0)
        wall = min(ts)
        _log("wall per call ms: " + " ".join(f"{t*1e3:.2f}" for t in ts))
    return results, wall, (f1, dev_in)


def measure_hw_time(F, in_maps, nrep=9, reps=14, nblocks=6, block=14):
    """HW exec estimate: block design. Same-program bursts (no per-call NEFF
    swap inside a block), alternating blocks between the 1-rep and nrep-rep
    programs to cancel slow drift of the ~85ms RPC floor. Per-iter =
    (median over blocks of block-median wall) diff / (nrep - 1). The older
    interleaved-min protocol swaps NEFFs every call, which adds a
    program-size-dependent cost and ~200us of noise to the estimate."""
    import jax

    import numpy as _np

    nc1 = build_program(F, nrep=1)
    _, _, (f1, dev1) = run_spmd(nc1, in_maps)
    ncN = build_program(F, nrep=nrep)
    _, _, (fN, devN) = run_spmd(ncN, in_maps)
    meds = {1: [], nrep: []}
    for blk in range(nblocks):
        for key, f, dev in ((1, f1, dev1), (nrep, fN, devN)):
            ts = []
            for i in range(block):
                t0 = time.perf_counter()
                jax.block_until_ready(f(*dev))
                ts.append(time.perf_counter() - t0)
            meds[key].append(float(_np.median(ts[2:])))
    # adjacent 1-rep / nrep-rep blocks share slow drift: difference them
    # pairwise, then take the median over pairs
    diffs = [(bN - b1) / (nrep - 1)
             for b1, bN in zip(meds[1], meds[nrep])]
    per_iter = max(float(_np.median(diffs)), 0.0)
    _log("block medians 1: " + " ".join(f"{t*1e3:.2f}" for t in meds[1]))
    _log(f"block medians {nrep}: " + " ".join(f"{t*1e3:.2f}" for t in meds[nrep]))
    _log("paired us/iter: " + " ".join(f"{d*1e6:.0f}" for d in diffs))
    _log(f"measure: -> {per_iter*1e6:.1f}us/iter")
    return per_iter * 1e9


_LAST_TIMING = None


def kernel(**inputs) -> np.ndarray:
    t0 = time.perf_counter()
    in_maps, F = prep_inputs(inputs)
    _log(f"host prep {time.perf_counter()-t0:.2f}s")
    t0 = time.perf_counter()
    nc = build_program(F)
    _log(f"build+tile {time.perf_counter()-t0:.1f}s")
    results, _, _ = run_spmd(nc, in_maps)
    out = np.empty((F, B, 64, 64), np.float32)
    for c in range(NCORES):
        yT = results[c]["yT"]                      # [4096, F*32]
        y = yT.T.reshape(F, BL, 64, 64)
        out[:, c * BL:(c + 1) * BL] = y
    return out


# revision 26
# speedup vs baseline: 1.9323x; 1.2729x over previous
"""Trainium2 Bass kernel for nn_FC_LSTM (FC-LSTM encoder-decoder).

Strategy:
  - Data-parallel over batch: 256 samples -> 8 cores x 32 samples.
  - Feature-major layout on chip: activations stored [feature(part), sample(free)],
    weights host-transposed to [in_feat, out_feat] so every matmul is
    out[feat_out, samples] = W_T.T @ act with contraction on partitions.
  - All matmuls in fp8e4 (e4m3) with MatmulPerfMode.DoubleRow: each
    instruction contracts a 256-wide K (two 128 k-tiles packed in dim1 of
    both operands), doubling PE throughput and halving PE instruction
    count vs bf16. Weights are host-quantized with power-of-2 scales
    (max|w|*s = 128 < 240); the descale folds into the activation
    instruction's input scale (out = func(scale*psum + bias)).
  - Encoder FC stack (4096->1024->256) batched over all 20 timesteps (640
    samples per core). en1's input-side gate matmul (Wih @ z_t) is also
    precomputed densely for all t at gate scale 2^11.
  - LSTM recurrence: per step the layer-2 cell at t and the layer-1 cell at
    t+1 depend only on the previous step's pair state, so they are emitted
    as a PAIR: two PSUM banks (a shared bank trips psum start=True
    whole-bank zeroing), but one [128, 512] SBUF gates tile and
    double-width elementwise (sig/tanh ACT per bank, then pair-wide
    mul/add on DVE + Pool, one tanh(c) ACT, one fp8 h-mul). Gate psum col
    m*32+s = gate-feature 128m+p of sample s (scale 2^11); gate order
    host-permuted to [i, f, o, g]. Cell biases (x2^11) are injected with a
    single K=128 bf16 matmul against a constant indicator matrix; FC biases
    likewise ride in the psum via a bias-in-row-0 bf16 matmul against a
    ones column, so FC relu+descale is a single 2-op DVE tensor_scalar
    (no ACT involvement outside the cells and the output tanh).
    h state is stored fp8, c state fp32.
  - Decoder FC stack (256->1024->4096) batched over all future steps.
  - The FC phases are emitted as generators whose matmul blocks interleave
    into the recurrence as PE gap fillers; DMAs are emitted in first-use
    order (small bias/ones constants first: they gate FC psum-group
    starts).
  - measure_hw_time uses a block design: bursts of same-program calls
    (per-call NEFF swap costs otherwise contaminate the estimate),
    alternating blocks of the 1-rep and 9-rep programs, difference of
    median block-medians.
"""

import time
from contextlib import ExitStack

import ml_dtypes
import numpy as np

import concourse.bass as bass
import concourse.mybir as mybir
import concourse.tile as tile

BF16NP = ml_dtypes.bfloat16
F8NP = ml_dtypes.float8_e4m3
AF = mybir.ActivationFunctionType
DT = mybir.dt
DR = mybir.MatmulPerfMode.DoubleRow

S = 20          # encoder sequence length
B = 256         # global batch
NCORES = 8
BL = B // NCORES  # 32 samples per core
H = 256         # LSTM hidden
G = 4 * H       # 1024 gate features
D = 4096        # input feature dim (64*64)
HID = 1024      # FC hidden
SB = S * BL     # 640 encoder samples per core

# power-of-2 quant scales: max|w| * SW = 128 (e4m3 max finite = 240)
SW1 = 2.0 ** 13    # fc_en1 w ~ U(+-2^-6)
SW2 = 2.0 ** 12    # fc_en2 w ~ U(+-2^-5)
SWC = 2.0 ** 11    # cell weights ~ U(+-2^-4); gate psum scale
SWD1 = 2.0 ** 11   # fc_de1 w ~ U(+-2^-4)
SWD2 = 2.0 ** 12   # fc_de2 w ~ U(+-2^-5)

VERBOSE = True


def _log(*a):
    if VERBOSE:
        print("[kernel]", *a, flush=True)


# ---------------------------------------------------------------------------
# Workaround: walrus CoreV3 setupSyncWait allows only 1 sync wait on the
# TileContext exit Drain. Split its waits across multiple drain instructions.
# ---------------------------------------------------------------------------
def _patched_drain_and_barrier(self, tick_clock, wait_clock):
    nc = self.nc
    drain_inst = nc.sync.drain()
    wait_clock.add_sem_waits(
        drain_inst.ins, tile.ScopedClock({None: tick_clock.global_clock})
    )
    inst = drain_inst.ins
    si = inst.sync_info
    waits = list(si.on_wait) if si is not None and si.on_wait else []
    MAXW = 1
    if len(waits) > MAXW:
        si.on_wait = waits[:MAXW]
        for i in range(MAXW, len(waits), MAXW):
            d2 = nc.sync.drain()
            i2 = d2.ins
            si2 = i2.sync_info
            if si2 is None:
                i2.sync_info = type(si)(on_wait=waits[i : i + MAXW], on_update=[])
            else:
                si2.on_wait = list(si2.on_wait or []) + waits[i : i + MAXW]

    nc.all_engine_barrier()
    assert self.sems is not None
    popped = nc._tile_sem_poison_stack.pop()
    assert popped is self._sem_poison
    nc.clear_and_free_semaphores(list(self.sems.allocated().values()))
    nc.all_engine_barrier()


tile.TileContext._drain_and_barrier = _patched_drain_and_barrier


def _split_sync_waits(nc, limit=1):
    """walrus setupSyncWait rejects >2 sem waits per instruction: move excess
    waits onto same-engine NoOps spliced just before the instruction."""
    ctr = [0]
    SyncInfo = None
    for f in nc.m.functions:
        for bb in f.blocks:
            out = []
            for inst in bb.instructions:
                si = inst.sync_info
                waits = list(si.on_wait) if si is not None and si.on_wait else []
                if len(waits) > limit:
                    if SyncInfo is None:
                        SyncInfo = type(si)
                    extras = waits[: len(waits) - limit]
                    si.on_wait = waits[len(waits) - limit:]
                    for i in range(0, len(extras), limit):
                        ctr[0] += 1
                        nop = mybir.InstNoOp(name=f"ws_{ctr[0]}", ins=[], outs=[])
                        nop.engine = inst.engine
                        nop.sync_info = SyncInfo(
                            on_wait=extras[i : i + limit], on_update=[]
                        )
                        out.append(nop)
                out.append(inst)
            bb.instructions[:] = out
    return ctr[0]


# ---------------------------------------------------------------------------
# Program builder
# ---------------------------------------------------------------------------
CELLS = ["en1", "en2", "en3", "de1", "de2", "de3"]
DJ = D // 256       # 16 k-tile-pairs of the 4096 input dim
HJ = HID // 256     # 4 k-tile-pairs of the 1024 hidden dim


def build_program(F: int, nrep: int = 1) -> bass.Bass:
    FB = F * BL  # decoder samples per core
    assert FB <= 512 and FB % 2 == 0
    nc = bass.Bass()

    # --- DRAM tensors (all fp8 weight layouts are DoubleRow-paired:
    #     row j*128+p, col i*M+m holds W[256j+128i+p, m]) ---
    xT = nc.dram_tensor("xT", [DJ * 128, 2 * SB], DT.float8e4, kind="ExternalInput")
    w1T = nc.dram_tensor("w1T", [DJ * 128, 2 * HID], DT.float8e4, kind="ExternalInput")
    b1B = nc.dram_tensor("b1B", [128, HID], DT.bfloat16, kind="ExternalInput")
    w2T = nc.dram_tensor("w2T", [HJ * 128, 2 * H], DT.float8e4, kind="ExternalInput")
    b2B = nc.dram_tensor("b2B", [128, H], DT.bfloat16, kind="ExternalInput")
    cellT = {}
    for nm in CELLS:
        ins = {}
        if nm != "de1":
            ins["wih"] = nc.dram_tensor(f"{nm}_wihT", [128, 2 * G], DT.float8e4,
                                        kind="ExternalInput")
        ins["whh"] = nc.dram_tensor(f"{nm}_whhT", [128, 2 * G], DT.float8e4,
                                    kind="ExternalInput")
        if nm == "en1":
            ins["bv"] = nc.dram_tensor("en1_bv", [128, G // 128], DT.float32,
                                       kind="ExternalInput")
        else:
            ins["bB"] = nc.dram_tensor(f"{nm}_bB", [128, 128], DT.bfloat16,
                                       kind="ExternalInput")
        cellT[nm] = ins
    Em = nc.dram_tensor("Em", [128, 256], DT.bfloat16, kind="ExternalInput")
    # decoder head stays bf16: its pre-tanh output is tiny relative to the
    # summand magnitudes (heavy cancellation), so fp8 there blows the error
    # budget (measured: fp8 fc_de2 alone -> 3.3e-2 rel err).
    wd1T = nc.dram_tensor("wd1T", [H, HID], DT.bfloat16, kind="ExternalInput")
    bd1B = nc.dram_tensor("bd1B", [128, HID], DT.bfloat16, kind="ExternalInput")
    wd2T = nc.dram_tensor("wd2T", [HID, D], DT.bfloat16, kind="ExternalInput")
    bd2v = nc.dram_tensor("bd2v", [128, D // 128], DT.float32, kind="ExternalInput")
    yT = nc.dram_tensor("yT", [D, FB], DT.float32, kind="ExternalOutput")

    NCHA = 4                      # phase A chunks (5 encoder steps each)
    CHA = SB // NCHA              # 160 samples
    SPC = S // NCHA               # steps per chunk
    NCHD = 2                      # phase D chunks
    CHD = FB // NCHD

    def pair(ap):
        """[128, 2*N] fp8 AP -> [128, 2, N] DoubleRow operand view."""
        return ap.rearrange("p (i n) -> p i n", i=2)

    with tile.TileContext(nc) as tc:
     for rep in range(nrep):
      with ExitStack() as ctx:
        const = ctx.enter_context(tc.tile_pool(name="const", bufs=1))
        gi1p = ctx.enter_context(tc.tile_pool(name="gi1p", bufs=1))
        state = ctx.enter_context(tc.tile_pool(name="state", bufs=3))
        gates = ctx.enter_context(tc.tile_pool(name="gates", bufs=4))
        outp = ctx.enter_context(tc.tile_pool(name="outp", bufs=4))
        psum = ctx.enter_context(tc.tile_pool(name="psum", bufs=6, space="PSUM"))
        psum2 = ctx.enter_context(tc.tile_pool(name="psum2", bufs=1,
                                               space="PSUM"))

        uid = [0]

        def PS():
            uid[0] += 1
            return psum.tile([128, 512], DT.float32, tag="ps",
                             name=f"ps{uid[0]}")

        def dma_in2(pool, dram, tag):
            t = pool.tile(list(dram.shape), dram.dtype, tag=tag)
            nc.sync.dma_start(t[:], dram[:])
            return t

        def gi_ap(t):
            c, o = divmod(t, SPC)
            return gi1[c][:, :, o * BL:(o + 1) * BL]

        # ========== Phase A generator: per-k-tile weight/x DMAs, chunked ===
        pA_ctx = ExitStack()
        pA = pA_ctx.enter_context(tc.tile_pool(name="phaseA", bufs=1))

        # DMA emission ordered by first use: w1 + x chunk 0 feed the FC chain
        # immediately; encoder cell weights arrive next; remaining x chunks
        # stream during the early recurrence; decoder weights last.
        # tiny constants first: the FC1 bias matmuls (group starts) need
        # b1/ones immediately; don't queue them behind 5MB of w1/x stream
        b1_sb = const.tile([128, HID // 128, 128], DT.bfloat16, tag="b1B")
        nc.sync.dma_start(b1_sb[:], b1B.rearrange("p (m q) -> p m q", q=128))
        ones_sb = const.tile([128, 160], DT.bfloat16, tag="ones")
        nc.vector.memset(ones_sb[:], 1.0)
        b2_sb = const.tile([128, H // 128, 128], DT.bfloat16, tag="b2B")
        nc.sync.dma_start(b2_sb[:], b2B.rearrange("p (m q) -> p m q", q=128))
        x_kc = [[None] * NCHA for _ in range(DJ)]
        w1_k = []
        for j in range(DJ):
            wk = pA.tile([128, 2 * HID], DT.float8e4, tag=f"w1_{j}",
                         name=f"w1_{j}")
            nc.sync.dma_start(wk[:], w1T[j * 128:(j + 1) * 128, :])
            w1_k.append(wk)
            xk = pA.tile([128, 2 * CHA], DT.float8e4, tag=f"x{j}_0",
                         name=f"x{j}_0")
            nc.sync.dma_start(xk[:], xT[j * 128:(j + 1) * 128, 0:2 * CHA])
            x_kc[j][0] = xk

        # encoder-phase constants
        w2_sb = const.tile([128, HJ, 2 * H], DT.float8e4, tag="w2")
        nc.sync.dma_start(w2_sb[:], w2T.rearrange("(j p) c -> p j c", p=128))
        cell_sb = {}
        for nm in ["en1", "en2", "en3"]:
            e = {}
            e["wih"] = dma_in2(const, cellT[nm]["wih"], f"{nm}_wih")
            e["whh"] = dma_in2(const, cellT[nm]["whh"], f"{nm}_whh")
            if nm == "en1":
                e["bv"] = dma_in2(const, cellT[nm]["bv"], "en1_bv")
            else:
                e["bB"] = dma_in2(const, cellT[nm]["bB"], f"{nm}_bB")
            cell_sb[nm] = e
        E_sb = dma_in2(const, Em, "Em")

        # remaining x chunks
        for c in range(1, NCHA):
            for j in range(DJ):
                xk = pA.tile([128, 2 * CHA], DT.float8e4, tag=f"x{j}_{c}",
                             name=f"x{j}_{c}")
                nc.sync.dma_start(
                    xk[:], xT[j * 128:(j + 1) * 128, c * 2 * CHA:(c + 1) * 2 * CHA])
                x_kc[j][c] = xk

        # decoder-phase constants
        for nm in ["de1", "de2", "de3"]:
            e = {}
            if "wih" in cellT[nm]:
                e["wih"] = dma_in2(const, cellT[nm]["wih"], f"{nm}_wih")
            e["whh"] = dma_in2(const, cellT[nm]["whh"], f"{nm}_whh")
            e["bB"] = dma_in2(const, cellT[nm]["bB"], f"{nm}_bB")
            cell_sb[nm] = e
        wd1_sb = const.tile([128, H // 128, HID], DT.bfloat16, tag="wd1")
        nc.sync.dma_start(wd1_sb[:], wd1T.rearrange("(o p) m -> p o m", p=128))
        bd1_sb = const.tile([128, HID // 128, 128], DT.bfloat16, tag="bd1B")
        nc.sync.dma_start(bd1_sb[:], bd1B.rearrange("p (m q) -> p m q", q=128))
        bd2_sb = dma_in2(const, bd2v, "bd2v")
        zh = const.tile([128, 2 * BL], DT.float8e4, tag="zh")
        zc = const.tile([128, 2 * BL], DT.float32, tag="zc")
        nc.vector.memset(zh[:], 0.0)
        nc.vector.memset(zc[:], 0.0)
        h3all = [const.tile([128, H // 128, CHD], DT.bfloat16, tag=f"h3all{c}",
                            name=f"h3all{c}")
                 for c in range(NCHD)]
        gi1 = [gi1p.tile([128, G // 128, CHA], DT.float32, tag=f"gi1_{c}",
                         name=f"gi1_{c}")
               for c in range(NCHA)]

        def phaseA_gen():
            """Emits all of phase A; yields chunks_done after each MM block."""
            for c in range(NCHA):
                z1c = pA.tile([128, HID // 128, CHA], DT.float8e4,
                              tag=f"z1_{c}", name=f"z1_{c}")
                zc_ = pA.tile([128, 2, CHA], DT.float8e4,
                              tag=f"z_{c}", name=f"z_{c}")
                if c == 0:
                    # chunk 0 runs while x/w1 DMAs land: j-outer so each
                    # arriving k-pair tile is consumed immediately
                    for half in range(2):
                        ms = range(half * 4, half * 4 + 4)
                        pss = [PS()[:, :CHA] for _ in ms]
                        for mi, m in enumerate(ms):
                            nc.tensor.matmul(
                                pss[mi], b1_sb[:, m, :], ones_sb[:, :CHA],
                                start=True, stop=False, skip_group_check=True)
                        for j in range(DJ):
                            wv = pair(w1_k[j][:])
                            xv = pair(x_kc[j][0][:])
                            for mi, m in enumerate(ms):
                                nc.tensor.matmul(
                                    pss[mi], wv[:, :, m * 128:(m + 1) * 128],
                                    xv,
                                    start=False, stop=(j == DJ - 1),
                                    perf_mode=DR,
                                )
                            if j % 4 == 3:
                                yield c
                        for mi, m in enumerate(ms):
                            nc.vector.tensor_scalar(
                                z1c[:, m, :], pss[mi], 1.0 / SW1, 0.0,
                                mybir.AluOpType.mult, mybir.AluOpType.max)
                else:
                    for m in range(HID // 128):
                        ps = PS()[:, :CHA]
                        nc.tensor.matmul(
                            ps, b1_sb[:, m, :], ones_sb[:, :CHA],
                            start=True, stop=False, skip_group_check=True)
                        for j0 in range(0, DJ, 4):
                            for j in range(j0, j0 + 4):
                                nc.tensor.matmul(
                                    ps,
                                    pair(w1_k[j][:])[:, :, m * 128:(m + 1) * 128],
                                    pair(x_kc[j][c][:]),
                                    start=False, stop=(j == DJ - 1),
                                    perf_mode=DR,
                                )
                            yield c
                        nc.vector.tensor_scalar(
                            z1c[:, m, :], ps, 1.0 / SW1, 0.0,
                            mybir.AluOpType.mult, mybir.AluOpType.max)
                for m in range(H // 128):
                    ps = PS()[:, :CHA]
                    nc.tensor.matmul(
                        ps, b2_sb[:, m, :], ones_sb[:, :CHA],
                        start=True, stop=False, skip_group_check=True)
                    for j in range(HJ):
                        nc.tensor.matmul(
                            ps,
                            pair(w2_sb[:, j, :])[:, :, m * 128:(m + 1) * 128],
                            z1c[:, 2 * j:2 * j + 2, :],
                            start=False, stop=(j == HJ - 1),
                            perf_mode=DR,
                        )
                    nc.vector.tensor_scalar(
                        zc_[:, m, :], ps, 1.0 / SW2, 0.0,
                        mybir.AluOpType.mult, mybir.AluOpType.max)
                    yield c
                wihv = pair(cell_sb["en1"]["wih"][:])
                for m in range(G // 128):
                    ps = PS()[:, :CHA]
                    nc.tensor.matmul(
                        ps, wihv[:, :, m * 128:(m + 1) * 128], zc_[:],
                        start=True, stop=True, perf_mode=DR,
                    )
                    nc.vector.tensor_scalar_add(
                        gi1[c][:, m, :], ps, cell_sb["en1"]["bv"][:, m:m + 1])
                    if m % 2 == 1:
                        yield c + (m == G // 128 - 1)
            while True:
                yield NCHA + 1

        genA = phaseA_gen()
        a_done = [0]

        def fillA(n=1):
            for _ in range(n):
                a_done[0] = max(a_done[0], next(genA))

        def needA(chunks):
            while a_done[0] < chunks + 1:
                fillA()

        # ========== LSTM cell (single) ====================================
        def lstm_cell(nm, x_in, h_prev, c_prev, gi, htag, ctag,
                      h_out=None, c_out=None):
            e = cell_sb[nm]
            ps = PS()[:, :256]
            groups = []
            if gi is None:
                nc.tensor.matmul(ps, e["bB"][:], E_sb[:], start=True, stop=False)
                started = True
            else:
                started = False
            if x_in is not None:
                groups.append((pair(e["wih"][:]), pair(x_in)))
            groups.append((pair(e["whh"][:]), pair(h_prev)))
            ng = len(groups)
            for gidx, (wv, rv) in enumerate(groups):
                for m in range(8):
                    nc.tensor.matmul(
                        ps[:, m * 32:(m + 1) * 32],
                        wv[:, :, m * 128:(m + 1) * 128],
                        rv,
                        start=(not started and gidx == 0),
                        stop=(gidx == ng - 1),
                        perf_mode=DR,
                        skip_group_check=True,
                    )
            if gi is not None:
                pv = ps.rearrange("p (m s) -> p m s", s=32)
                nc.vector.tensor_add(pv, pv, gi)
            g = gates.tile([128, 256], DT.float32, tag="g", name=f"g{uid[0]}")
            nc.scalar.activation(g[:, 0:192], ps[:, 0:192], AF.Sigmoid,
                                 scale=1.0 / SWC)
            nc.scalar.activation(g[:, 192:256], ps[:, 192:256], AF.Tanh,
                                 scale=1.0 / SWC)
            # packed: i: 0..63, f: 64..127, o: 128..191, g: 192..255
            t1 = gates.tile([128, 64], DT.float32, tag="t1", name=f"t1{uid[0]}")
            nc.gpsimd.tensor_mul(t1[:], g[:, 0:64], g[:, 192:256])
            c2 = gates.tile([128, 64], DT.float32, tag="c2", name=f"c2{uid[0]}")
            nc.vector.tensor_mul(c2[:], g[:, 64:128], c_prev)
            if c_out is None:
                c_new = state.tile([128, 64], DT.float32, tag=ctag,
                                   name=f"{ctag}{uid[0]}")
                c_out = c_new[:]
            else:
                c_new = None
            nc.vector.tensor_add(c_out, c2[:], t1[:])
            th = gates.tile([128, 64], DT.float32, tag="th", name=f"th{uid[0]}")
            nc.scalar.activation(th[:], c_out, AF.Tanh)
            if h_out is None:
                h_new = state.tile([128, 64], DT.float8e4, tag=htag,
                                   name=f"{htag}{uid[0]}")
                h_out = h_new[:]
            else:
                h_new = None
            nc.vector.tensor_mul(h_out, g[:, 128:192], th[:])
            return h_out, c_out

        # ========== Paired LSTM cells (A = layer-2 cell at t, B = layer-1
        # cell at t+1; both read only pair_prev, so their 512-col gate psums
        # share one bank and the elementwise chain runs at double width) ====
        def lstm_pair(nmA, nmB, hp_prev, cp_prev, giB, htag, ctag):
            eA, eB = cell_sb[nmA], cell_sb[nmB]
            # one two-bank psum tile: A gates in bank 0 (cols 0:256), B gates
            # in bank 1 (cols 512:768). Matmuls stay within a single bank
            # each (start=True zeroing is bank-granular), but the sigmoid /
            # tanh ACTs read both banks in one strided instruction.
            uid[0] += 1
            ps2 = psum2.tile([128, 1024], DT.float32, tag="psp",
                             name=f"psp{uid[0]}")
            psA = ps2[:, 0:256]
            psB = ps2[:, 512:768]
            nc.tensor.matmul(psA, eA["bB"][:], E_sb[:],
                             start=True, stop=False, skip_group_check=True)
            bstart = False
            if giB is None:
                nc.tensor.matmul(psB, eB["bB"][:], E_sb[:],
                                 start=True, stop=False, skip_group_check=True)
            else:
                bstart = True
            xA = pair(hp_prev[:, 64:128])   # h of layer-1 cell at t
            hA = pair(hp_prev[:, 0:64])     # layer-2 cell's own h at t-1
            hB = pair(hp_prev[:, 64:128])   # layer-1 cell's own h at t
            for m in range(8):
                sl = slice(m * 32, (m + 1) * 32)
                nc.tensor.matmul(psA[:, sl],
                                 pair(eA["wih"][:])[:, :, m * 128:(m + 1) * 128],
                                 xA, start=False, stop=False,
                                 perf_mode=DR, skip_group_check=True)
            for m in range(8):
                sl = slice(m * 32, (m + 1) * 32)
                nc.tensor.matmul(psA[:, sl],
                                 pair(eA["whh"][:])[:, :, m * 128:(m + 1) * 128],
                                 hA, start=False, stop=True,
                                 perf_mode=DR, skip_group_check=True)
            for m in range(8):
                sl = slice(m * 32, (m + 1) * 32)
                nc.tensor.matmul(psB[:, sl],
                                 pair(eB["whh"][:])[:, :, m * 128:(m + 1) * 128],
                                 hB, start=bstart, stop=True,
                                 perf_mode=DR, skip_group_check=True)
            if giB is not None:
                pv = psB.rearrange("p (m s) -> p m s", s=32)
                nc.vector.tensor_add(pv, pv, giB)
            # gates tile packs both cells: [ifo|g] at 0 (A) and 256 (B)
            g = gates.tile([128, 512], DT.float32, tag="gp", name=f"gp{uid[0]}")
            cp_new = state.tile([128, 128], DT.float32, tag=ctag,
                                name=f"{ctag}{uid[0]}")
            th = gates.tile([128, 128], DT.float32, tag="thp",
                            name=f"thp{uid[0]}")
            hp_new = state.tile([128, 128], DT.float8e4, tag=htag,
                                name=f"{htag}{uid[0]}")
            gval = g.rearrange("p (two x) -> p two x", two=2)
            pv2 = ps2.rearrange("p (two x) -> p two x", two=2)
            nc.scalar.activation(gval[:, :, 0:192], pv2[:, :, 0:192],
                                 AF.Sigmoid, scale=1.0 / SWC)
            nc.scalar.activation(gval[:, :, 192:256], pv2[:, :, 192:256],
                                 AF.Tanh, scale=1.0 / SWC)
            # pair-wide elementwise on SBUF (strided [128, 2, 64] views)
            gv = g.rearrange("p (two x) -> p two x", two=2)
            cpv = cp_prev.rearrange("p (two s) -> p two s", two=2)
            t1 = gates.tile([128, 2, 64], DT.float32, tag="t1p",
                            name=f"t1p{uid[0]}")
            nc.gpsimd.tensor_mul(t1[:], gv[:, :, 0:64], gv[:, :, 192:256])
            c2 = gates.tile([128, 2, 64], DT.float32, tag="c2p",
                            name=f"c2p{uid[0]}")
            nc.vector.tensor_mul(c2[:], gv[:, :, 64:128], cpv)
            cnv = cp_new.rearrange("p (two s) -> p two s", two=2)
            nc.vector.tensor_add(cnv, c2[:], t1[:])
            nc.scalar.activation(th[:], cp_new[:], AF.Tanh)
            hnv = hp_new.rearrange("p (two s) -> p two s", two=2)
            nc.vector.tensor_mul(hnv, gv[:, :, 128:192],
                                 th.rearrange("p (two s) -> p two s", two=2))
            return hp_new, cp_new

        # ========== Phase D transition (callable mid-encoder) =============
        wd2_k = []
        pD_box = []

        def ensure_pD():
            if pD_box:
                return
            pA_ctx.close()  # free phase A SBUF before loading decoder weights
            pD = ctx.enter_context(tc.tile_pool(name="phaseD", bufs=1))
            pD_box.append(pD)
            for k in range(HID // 128):
                wk = pD.tile([128, D], DT.bfloat16, tag=f"wd2_{k}",
                             name=f"wd2_{k}")
                nc.sync.dma_start(wk[:], wd2T[k * 128:(k + 1) * 128, :])
                wd2_k.append(wk)

        # ========== Encoder recurrence, pipelined w/ phase A fillers ======
        # pair state: hp = [h2(t-1), h1(t)] (fp8), cp likewise (fp32)
        needA(0)
        h3s, c3s = [None] * S, [None] * S
        hp = state.tile([128, 128], DT.float8e4, tag="hp", name="hp_init")
        cp = state.tile([128, 128], DT.float32, tag="cp", name="cp_init")
        nc.vector.memset(hp[:, 0:64], 0.0)
        nc.vector.memset(cp[:, 0:64], 0.0)
        lstm_cell("en1", None, zh[:], zc[:], gi_ap(0), "h1", "c1",
                  h_out=hp[:, 64:128], c_out=cp[:, 64:128])
        for t in range(S):
            fillA(3)
            if t + 1 < S:
                needA((t + 1) // SPC)
                hp_new, cp_new = lstm_pair("en2", "en1", hp, cp,
                                           gi_ap(t + 1), "hp", "cp")
                h2t = hp_new[:, 0:64]
            else:
                h2t, _ = lstm_cell("en2", hp[:, 64:128], hp[:, 0:64],
                                   cp[:, 0:64], None, "h2", "c2")
            fillA(3)
            h3p = h3s[t - 1] if t else zh[:]
            c3p = c3s[t - 1] if t else zc[:]
            h3s[t], c3s[t] = lstm_cell("en3", h2t, h3p, c3p, None, "h3", "c3")
            if t + 1 < S:
                hp, cp = hp_new, cp_new
            if t == S - 5:
                needA(NCHA)  # drain phase A now so decoder weights can load
                ensure_pD()

        ensure_pD()
        pD = pD_box[0]

        def phaseD_gen(c):
            y1c = pD.tile([128, HID // 128, CHD], DT.bfloat16,
                          tag=f"y1_{c}", name=f"y1_{c}")
            for m in range(HID // 128):
                ps = PS()[:, :CHD]
                nc.tensor.matmul(
                    ps, bd1_sb[:, m, :], ones_sb[:, :CHD],
                    start=True, stop=False, skip_group_check=True)
                for k in range(H // 128):
                    nc.tensor.matmul(
                        ps, wd1_sb[:, k, m * 128:(m + 1) * 128],
                        h3all[c][:, k, :],
                        start=False, stop=(k == H // 128 - 1),
                    )
                nc.vector.tensor_scalar(
                    y1c[:, m, :], ps, 1.0, 0.0,
                    mybir.AluOpType.mult, mybir.AluOpType.max)
                if m % 2 == 1:
                    yield
            for m in range(D // 128):
                ps = PS()[:, :CHD]
                for k in range(HID // 128):
                    nc.tensor.matmul(
                        ps, wd2_k[k][:, m * 128:(m + 1) * 128],
                        y1c[:, k, :],
                        start=(k == 0), stop=(k == HID // 128 - 1),
                    )
                o_sb = outp.tile([128, CHD], DT.float32, tag="o",
                                 name=f"o{uid[0]}")
                nc.scalar.activation(o_sb[:], ps, AF.Tanh,
                                     bias=bd2_sb[:, m:m + 1])
                nc.sync.dma_start(
                    yT[m * 128:(m + 1) * 128, c * CHD:(c + 1) * CHD],
                    o_sb[:])
                yield

        genDs = [phaseD_gen(c) for c in range(NCHD)]
        d_ready = [0]   # decoder chunks whose h3all is complete

        def fillD(n=1):
            for _ in range(n):
                for c in range(d_ready[0]):
                    if next(genDs[c], None) is not None:
                        break

        # ========== Decoder recurrence, pipelined w/ phase D fillers ======
        d3s, f3s = [None] * F, [None] * F
        dp = state.tile([128, 128], DT.float8e4, tag="dp", name="dp_init")
        fp = state.tile([128, 128], DT.float32, tag="fp", name="fp_init")
        nc.vector.memset(dp[:, 0:64], 0.0)
        nc.vector.memset(fp[:, 0:64], 0.0)
        lstm_cell("de1", None, h3s[S - 1], zc[:], None, "d1", "e1",
                  h_out=dp[:, 64:128], c_out=fp[:, 64:128])
        for t in range(F):
            fillD(4)
            if t + 1 < F:
                dp_new, fp_new = lstm_pair("de2", "de1", dp, fp, None,
                                           "dp", "fp")
                d2t = dp_new[:, 0:64]
            else:
                d2t, _ = lstm_cell("de2", dp[:, 64:128], dp[:, 0:64],
                                   fp[:, 0:64], None, "d2", "e2")
            fillD(4)
            d3p = d3s[t - 1] if t else zh[:]
            f3p = f3s[t - 1] if t else zc[:]
            d3s[t], f3s[t] = lstm_cell("de3", d2t, d3p, f3p, None, "d3", "e3")
            if t + 1 < F:
                dp, fp = dp_new, fp_new
            c, o = divmod(t, CHD // BL)
            nc.vector.tensor_copy(
                h3all[c][:, :, o * BL:(o + 1) * BL],
                d3s[t].rearrange("p (k s) -> p k s", s=BL),
            )
            if o == CHD // BL - 1:
                d_ready[0] = c + 1
        # drain remaining phase D work
        for gd in genDs:
            for _ in gd:
                pass

    nsplit = _split_sync_waits(nc, limit=1)
    _log(f"split {nsplit} over-limit sync waits")
    return nc

# ---------------------------------------------------------------------------
# Host-side input prep
# ---------------------------------------------------------------------------
GATE_PERM = np.concatenate([
    np.arange(0, 2 * H),          # i, f
    np.arange(3 * H, 4 * H),      # o
    np.arange(2 * H, 3 * H),      # g
])


def _dr256(wT, scale):
    """[K, M] f32 (K = 256) -> DoubleRow fp8 layout [128, 2*M]:
    col i*M+m holds wT[128i+p, m] * scale."""
    K, M = wT.shape
    assert K == 256
    a = (wT * scale).reshape(2, 128, M).transpose(1, 0, 2).reshape(128, 2 * M)
    return np.ascontiguousarray(a).astype(F8NP)


def _drK(wT, scale):
    """[K, M] f32 (K = 256*J) -> [J*128, 2*M] fp8: row j*128+p,
    col i*M+m holds wT[256j+128i+p, m] * scale."""
    K, M = wT.shape
    J = K // 256
    a = (wT * scale).reshape(J, 2, 128, M).transpose(0, 2, 1, 3)
    return np.ascontiguousarray(a.reshape(J * 128, 2 * M)).astype(F8NP)


def prep_inputs(inputs):
    f32 = np.float32
    g = {k: np.asarray(v) for k, v in inputs.items()}
    F = int(np.asarray(g["future_step"]))

    shared = {}
    shared["w1T"] = _drK(np.asarray(g["fc_en1_w"].T, f32), SW1)
    b1B = np.zeros((128, HID // 128, 128), f32)
    b1B[0] = g["fc_en1_b"].astype(f32).reshape(HID // 128, 128) * SW1
    shared["b1B"] = b1B.reshape(128, HID).astype(BF16NP)
    shared["w2T"] = _drK(np.asarray(g["fc_en2_w"].T, f32), SW2)
    b2B = np.zeros((128, H // 128, 128), f32)
    b2B[0] = g["fc_en2_b"].astype(f32).reshape(H // 128, 128) * SW2
    shared["b2B"] = b2B.reshape(128, H).astype(BF16NP)
    for nm in CELLS:
        wih = g[nm + "_wih"][GATE_PERM]
        whh = g[nm + "_whh"][GATE_PERM]
        bsum = (g[nm + "_bih"] + g[nm + "_bhh"])[GATE_PERM].astype(f32) * SWC
        if nm != "de1":
            shared[nm + "_wihT"] = _dr256(np.asarray(wih.T, f32), SWC)
        shared[nm + "_whhT"] = _dr256(np.asarray(whh.T, f32), SWC)
        if nm == "en1":
            shared["en1_bv"] = np.ascontiguousarray(
                bsum.reshape(G // 128, 128).T)
        else:
            bB = np.zeros((128, 128), f32)
            bB[:G // 128, :] = bsum.reshape(G // 128, 128)
            shared[nm + "_bB"] = bB.astype(BF16NP)
    E = np.zeros((128, 256), f32)
    for j in range(8):
        E[j, j * 32:(j + 1) * 32] = 1.0
    shared["Em"] = E.astype(BF16NP)
    shared["wd1T"] = np.ascontiguousarray(g["fc_de1_w"].T).astype(BF16NP)
    bd1B = np.zeros((128, HID // 128, 128), f32)
    bd1B[0] = g["fc_de1_b"].astype(f32).reshape(HID // 128, 128)
    shared["bd1B"] = bd1B.reshape(128, HID).astype(BF16NP)
    shared["wd2T"] = np.ascontiguousarray(g["fc_de2_w"].T).astype(BF16NP)
    shared["bd2v"] = np.ascontiguousarray(
        g["fc_de2_b"].astype(f32).reshape(D // 128, 128).T)

    x = g["x"].astype(f32).reshape(S, B, D)
    NCHA, CHA = 4, SB // 4
    in_maps = []
    for c in range(NCORES):
        xc = x[:, c * BL:(c + 1) * BL, :].reshape(SB, D)   # row = t*BL + b
        # DoubleRow chunked layout: row j*128+p, col ch*2*CHA + i*CHA + s
        # holds x[feature 256j+128i+p, sample ch*CHA+s]
        xT = xc.T.reshape(DJ, 2, 128, NCHA, CHA).transpose(0, 2, 3, 1, 4)
        m = dict(shared)
        m["xT"] = np.ascontiguousarray(
            xT.reshape(DJ * 128, 2 * SB)).astype(F8NP)
        in_maps.append(m)
    return in_maps, F


# ---------------------------------------------------------------------------
# Execution via PJRT (axon), modeled on bass2jax.run_bass_via_pjrt
# ---------------------------------------------------------------------------
def run_spmd(nc, in_maps, n_timing=0):
    import jax
    from jax.experimental.shard_map import shard_map
    from jax.sharding import Mesh, NamedSharding, PartitionSpec

    from concourse import bass2jax

    bass2jax.install_neuronx_cc_hook()
    n_cores = len(in_maps)
    partition_name = nc.partition_id_tensor.name if nc.partition_id_tensor else None
    in_names, out_names, out_avals, zero_outs = [], [], [], []
    for alloc in nc.m.functions[0].allocations:
        if not isinstance(alloc, mybir.MemoryLocationSet):
            continue
        name = alloc.memorylocations[0].name
        if alloc.kind == "ExternalInput":
            if name != partition_name:
                in_names.append(name)
        elif alloc.kind == "ExternalOutput":
            out_names.append(name)
            shape = tuple(alloc.tensor_shape)
            dtype = mybir.dt.np(alloc.dtype)
            out_avals.append(jax.core.ShapedArray(shape, dtype))
            zero_outs.append(np.zeros(shape, dtype))
    n_params = len(in_names)
    all_in = in_names + out_names
    if partition_name is not None:
        all_in = all_in + [partition_name]
    all_in = tuple(all_in)

    def _bind(args):
        operands = list(args)
        if partition_name is not None:
            operands.append(bass2jax.partition_id_tensor())
        return bass2jax._bass_exec_p.bind(
            *operands,
            out_avals=tuple(out_avals),
            in_names=all_in,
            out_names=tuple(out_names),
            lowering_input_output_aliases=(),
            sim_require_finite=False,
            sim_require_nnan=False,
            nc=nc,
        )

    def _body(*args):
        return tuple(_bind(args))

    devices = jax.devices()[:n_cores]
    mesh = Mesh(np.asarray(devices), ("core",))
    pspec = PartitionSpec("core")
    in_specs = (pspec,) * (n_params + len(out_names))
    out_specs = (pspec,) * len(out_names)

    f1 = jax.jit(shard_map(_body, mesh=mesh, in_specs=in_specs,
                           out_specs=out_specs, check_rep=False))
    concat = [
        np.concatenate([np.asarray(in_maps[c][nm]) for c in range(n_cores)], axis=0)
        for nm in in_names
    ]
    concat += [np.concatenate([z] * n_cores, axis=0) for z in zero_outs]

    sharding = NamedSharding(mesh, pspec)
    t0 = time.perf_counter()
    dev_in = [jax.device_put(a, sharding) for a in concat]
    jax.block_until_ready(dev_in)
    _log(f"upload {sum(a.nbytes for a in concat)/1e6:.1f} MB in "
         f"{time.perf_counter()-t0:.2f}s")

    t0 = time.perf_counter()
    outs = jax.block_until_ready(f1(*dev_in))
    _log(f"first run (incl compile) {time.perf_counter()-t0:.1f}s")

    results = []
    np_outs = [np.asarray(o) for o in outs]
    for c in range(n_cores):
        r = {}
        for i, nm in enumerate(out_names):
            sh0 = out_avals[i].shape[0]
            r[nm] = np_outs[i][c * sh0:(c + 1) * sh0]
        results.append(r)

    wall = None
    if n_timing:
        ts = []
        for _ in range(n_timing):
            t0 = time.perf_counter()
            jax.block_until_ready(f1(*dev_in))
            ts.append(time.perf_counter() - t0)
        wall = min(ts)
        _log("wall per call ms: " + " ".join(f"{t*1e3:.2f}" for t in ts))
    return results, wall, (f1, dev_in)


def measure_hw_time(F, in_maps, nrep=9, reps=14, nblocks=6, block=14):
    """HW exec estimate: block design. Same-program bursts (no per-call NEFF
    swap inside a block), alternating blocks between the 1-rep and nrep-rep
    programs to cancel slow drift of the ~85ms RPC floor. Per-iter =
    (median over blocks of block-median wall) diff / (nrep - 1). The older
    interleaved-min protocol swaps NEFFs every call, which adds a
    program-size-dependent cost and ~200us of noise to the estimate."""
    import jax

    import numpy as _np

    nc1 = build_program(F, nrep=1)
    _, _, (f1, dev1) = run_spmd(nc1, in_maps)
    ncN = build_program(F, nrep=nrep)
    _, _, (fN, devN) = run_spmd(ncN, in_maps)
    meds = {1: [], nrep: []}
    for blk in range(nblocks):
        for key, f, dev in ((1, f1, dev1), (nrep, fN, devN)):
            ts = []
            for i in range(block):
                t0 = time.perf_counter()
                jax.block_until_ready(f(*dev))
                ts.append(time.perf_counter() - t0)
            meds[key].append(float(_np.median(ts[2:])))
    # adjacent 1-rep / nrep-rep blocks share slow drift: difference them
    # pairwise, then take the median over pairs
    diffs = [(bN - b1) / (nrep - 1)
             for b1, bN in zip(meds[1], meds[nrep])]
    per_iter = max(float(_np.median(diffs)), 0.0)
    _log("block medians 1: " + " ".join(f"{t*1e3:.2f}" for t in meds[1]))
    _log(f"block medians {nrep}: " + " ".join(f"{t*1e3:.2f}" for t in meds[nrep]))
    _log("paired us/iter: " + " ".join(f"{d*1e6:.0f}" for d in diffs))
    _log(f"measure: -> {per_iter*1e6:.1f}us/iter")
    return per_iter * 1e9


_LAST_TIMING = None


def kernel(**inputs) -> np.ndarray:
    t0 = time.perf_counter()
    in_maps, F = prep_inputs(inputs)
    _log(f"host prep {time.perf_counter()-t0:.2f}s")
    t0 = time.perf_counter()
    nc = build_program(F)
    _log(f"build+tile {time.perf_counter()-t0:.1f}s")
    results, _, _ = run_spmd(nc, in_maps)
    out = np.empty((F, B, 64, 64), np.float32)
    for c in range(NCORES):
        yT = results[c]["yT"]                      # [4096, F*32]
        y = yT.T.reshape(F, BL, 64, 64)
        out[:, c * BL:(c + 1) * BL] = y
    return out


# revision 27
# speedup vs baseline: 2.0301x; 1.0506x over previous
"""Trainium2 Bass kernel for nn_FC_LSTM (FC-LSTM encoder-decoder).

Strategy:
  - Data-parallel over batch: 256 samples -> 8 cores x 32 samples.
  - Feature-major layout on chip: activations stored [feature(part), sample(free)],
    weights host-transposed to [in_feat, out_feat] so every matmul is
    out[feat_out, samples] = W_T.T @ act with contraction on partitions.
  - All matmuls in fp8e4 (e4m3) with MatmulPerfMode.DoubleRow: each
    instruction contracts a 256-wide K (two 128 k-tiles packed in dim1 of
    both operands), doubling PE throughput and halving PE instruction
    count vs bf16. Weights are host-quantized with power-of-2 scales
    (max|w|*s = 128 < 240); the descale folds into the activation
    instruction's input scale (out = func(scale*psum + bias)).
  - Encoder FC stack (4096->1024->256) batched over all 20 timesteps (640
    samples per core). en1's input-side gate matmul (Wih @ z_t) is also
    precomputed densely for all t at gate scale 2^11.
  - LSTM recurrence: per step the layer-2 cell at t and the layer-1 cell at
    t+1 depend only on the previous step's pair state, so they are emitted
    as a PAIR: two PSUM banks (a shared bank trips psum start=True
    whole-bank zeroing), but one [128, 512] SBUF gates tile and
    double-width elementwise (sig/tanh ACT per bank, then pair-wide
    mul/add on DVE + Pool, one tanh(c) ACT, one fp8 h-mul). Gate psum col
    m*32+s = gate-feature 128m+p of sample s (scale 2^11); gate order
    host-permuted to [i, f, o, g]. Cell biases (x2^11) are injected with a
    single K=128 bf16 matmul against a constant indicator matrix; FC biases
    likewise ride in the psum via a bias-in-row-0 bf16 matmul against a
    ones column, so FC relu+descale is a single 2-op DVE tensor_scalar
    (no ACT involvement outside the cells and the output tanh).
    h state is stored fp8, c state fp32.
  - Decoder FC stack (256->1024->4096) batched over all future steps.
  - The FC phases are emitted as generators whose matmul blocks interleave
    into the recurrence as PE gap fillers; DMAs are emitted in first-use
    order (small bias/ones constants first: they gate FC psum-group
    starts).
  - measure_hw_time uses a block design: bursts of same-program calls
    (per-call NEFF swap costs otherwise contaminate the estimate),
    alternating blocks of the 1-rep and 9-rep programs, difference of
    median block-medians.
"""

import time
from contextlib import ExitStack

import ml_dtypes
import numpy as np

import concourse.bass as bass
import concourse.mybir as mybir
import concourse.tile as tile

BF16NP = ml_dtypes.bfloat16
F8NP = ml_dtypes.float8_e4m3
AF = mybir.ActivationFunctionType
DT = mybir.dt
DR = mybir.MatmulPerfMode.DoubleRow

S = 20          # encoder sequence length
B = 256         # global batch
NCORES = 8
BL = B // NCORES  # 32 samples per core
H = 256         # LSTM hidden
G = 4 * H       # 1024 gate features
D = 4096        # input feature dim (64*64)
HID = 1024      # FC hidden
SB = S * BL     # 640 encoder samples per core

# power-of-2 quant scales: max|w| * SW = 128 (e4m3 max finite = 240)
SW1 = 2.0 ** 13    # fc_en1 w ~ U(+-2^-6)
SW2 = 2.0 ** 12    # fc_en2 w ~ U(+-2^-5)
SWC = 2.0 ** 11    # cell weights ~ U(+-2^-4); gate psum scale
SWD1 = 2.0 ** 11   # fc_de1 w ~ U(+-2^-4)
SWD2 = 2.0 ** 12   # fc_de2 w ~ U(+-2^-5)

VERBOSE = True


def _log(*a):
    if VERBOSE:
        print("[kernel]", *a, flush=True)


# ---------------------------------------------------------------------------
# Workaround: walrus CoreV3 setupSyncWait allows only 1 sync wait on the
# TileContext exit Drain. Split its waits across multiple drain instructions.
# ---------------------------------------------------------------------------
def _patched_drain_and_barrier(self, tick_clock, wait_clock):
    nc = self.nc
    drain_inst = nc.sync.drain()
    wait_clock.add_sem_waits(
        drain_inst.ins, tile.ScopedClock({None: tick_clock.global_clock})
    )
    inst = drain_inst.ins
    si = inst.sync_info
    waits = list(si.on_wait) if si is not None and si.on_wait else []
    MAXW = 1
    if len(waits) > MAXW:
        si.on_wait = waits[:MAXW]
        for i in range(MAXW, len(waits), MAXW):
            d2 = nc.sync.drain()
            i2 = d2.ins
            si2 = i2.sync_info
            if si2 is None:
                i2.sync_info = type(si)(on_wait=waits[i : i + MAXW], on_update=[])
            else:
                si2.on_wait = list(si2.on_wait or []) + waits[i : i + MAXW]

    nc.all_engine_barrier()
    assert self.sems is not None
    popped = nc._tile_sem_poison_stack.pop()
    assert popped is self._sem_poison
    nc.clear_and_free_semaphores(list(self.sems.allocated().values()))
    nc.all_engine_barrier()


tile.TileContext._drain_and_barrier = _patched_drain_and_barrier


def _split_sync_waits(nc, limit=1):
    """walrus setupSyncWait rejects >2 sem waits per instruction: move excess
    waits onto same-engine NoOps spliced just before the instruction."""
    ctr = [0]
    SyncInfo = None
    for f in nc.m.functions:
        for bb in f.blocks:
            out = []
            for inst in bb.instructions:
                si = inst.sync_info
                waits = list(si.on_wait) if si is not None and si.on_wait else []
                if len(waits) > limit:
                    if SyncInfo is None:
                        SyncInfo = type(si)
                    extras = waits[: len(waits) - limit]
                    si.on_wait = waits[len(waits) - limit:]
                    for i in range(0, len(extras), limit):
                        ctr[0] += 1
                        nop = mybir.InstNoOp(name=f"ws_{ctr[0]}", ins=[], outs=[])
                        nop.engine = inst.engine
                        nop.sync_info = SyncInfo(
                            on_wait=extras[i : i + limit], on_update=[]
                        )
                        out.append(nop)
                out.append(inst)
            bb.instructions[:] = out
    return ctr[0]


# ---------------------------------------------------------------------------
# Program builder
# ---------------------------------------------------------------------------
CELLS = ["en1", "en2", "en3", "de1", "de2", "de3"]
DJ = D // 256       # 16 k-tile-pairs of the 4096 input dim
HJ = HID // 256     # 4 k-tile-pairs of the 1024 hidden dim


def build_program(F: int, nrep: int = 1) -> bass.Bass:
    FB = F * BL  # decoder samples per core
    assert FB <= 512 and FB % 2 == 0
    nc = bass.Bass()

    # --- DRAM tensors (all fp8 weight layouts are DoubleRow-paired:
    #     row j*128+p, col i*M+m holds W[256j+128i+p, m]) ---
    xT = nc.dram_tensor("xT", [DJ * 128, 2 * SB], DT.float8e4, kind="ExternalInput")
    w1T = nc.dram_tensor("w1T", [DJ * 128, 2 * HID], DT.float8e4, kind="ExternalInput")
    b1B = nc.dram_tensor("b1B", [128, HID], DT.bfloat16, kind="ExternalInput")
    w2T = nc.dram_tensor("w2T", [HJ * 128, 2 * H], DT.float8e4, kind="ExternalInput")
    b2B = nc.dram_tensor("b2B", [128, H], DT.bfloat16, kind="ExternalInput")
    cellT = {}
    for nm in CELLS:
        ins = {}
        if nm != "de1":
            ins["wih"] = nc.dram_tensor(f"{nm}_wihT", [128, 2 * G], DT.float8e4,
                                        kind="ExternalInput")
        ins["whh"] = nc.dram_tensor(f"{nm}_whhT", [128, 2 * G], DT.float8e4,
                                    kind="ExternalInput")
        ins["bB"] = nc.dram_tensor(f"{nm}_bB", [128, 128], DT.bfloat16,
                                   kind="ExternalInput")
        cellT[nm] = ins
    Em = nc.dram_tensor("Em", [128, 256], DT.bfloat16, kind="ExternalInput")
    # decoder head stays bf16: its pre-tanh output is tiny relative to the
    # summand magnitudes (heavy cancellation), so fp8 there blows the error
    # budget (measured: fp8 fc_de2 alone -> 3.3e-2 rel err).
    wd1T = nc.dram_tensor("wd1T", [H, HID], DT.bfloat16, kind="ExternalInput")
    bd1B = nc.dram_tensor("bd1B", [128, HID], DT.bfloat16, kind="ExternalInput")
    wd2T = nc.dram_tensor("wd2T", [HID, D], DT.bfloat16, kind="ExternalInput")
    bd2v = nc.dram_tensor("bd2v", [128, D // 128], DT.float32, kind="ExternalInput")
    yT = nc.dram_tensor("yT", [D, FB], DT.float32, kind="ExternalOutput")

    NCHA = 4                      # phase A chunks (5 encoder steps each)
    CHA = SB // NCHA              # 160 samples
    SPC = S // NCHA               # steps per chunk
    NCHD = 2                      # phase D chunks
    CHD = FB // NCHD

    def pair(ap):
        """[128, 2*N] fp8 AP -> [128, 2, N] DoubleRow operand view."""
        return ap.rearrange("p (i n) -> p i n", i=2)

    with tile.TileContext(nc) as tc:
     for rep in range(nrep):
      with ExitStack() as ctx:
        const = ctx.enter_context(tc.tile_pool(name="const", bufs=1))
        gi1p = ctx.enter_context(tc.tile_pool(name="gi1p", bufs=1))
        state = ctx.enter_context(tc.tile_pool(name="state", bufs=3))
        gates = ctx.enter_context(tc.tile_pool(name="gates", bufs=4))
        outp = ctx.enter_context(tc.tile_pool(name="outp", bufs=4))
        psum = ctx.enter_context(tc.tile_pool(name="psum", bufs=6, space="PSUM"))
        psum2 = ctx.enter_context(tc.tile_pool(name="psum2", bufs=1,
                                               space="PSUM"))

        uid = [0]

        def PS():
            uid[0] += 1
            return psum.tile([128, 512], DT.float32, tag="ps",
                             name=f"ps{uid[0]}")

        def dma_in2(pool, dram, tag):
            t = pool.tile(list(dram.shape), dram.dtype, tag=tag)
            nc.sync.dma_start(t[:], dram[:])
            return t

        def z_ap(t):
            c, o = divmod(t, SPC)
            return zs[c][:, :, o * BL:(o + 1) * BL]

        # ========== Phase A generator: per-k-tile weight/x DMAs, chunked ===
        pA_ctx = ExitStack()
        pA = pA_ctx.enter_context(tc.tile_pool(name="phaseA", bufs=1))

        # DMA emission ordered by first use: w1 + x chunk 0 feed the FC chain
        # immediately; encoder cell weights arrive next; remaining x chunks
        # stream during the early recurrence; decoder weights last.
        # tiny constants first: the FC1 bias matmuls (group starts) need
        # b1/ones immediately; don't queue them behind 5MB of w1/x stream
        b1_sb = const.tile([128, HID // 128, 128], DT.bfloat16, tag="b1B")
        nc.sync.dma_start(b1_sb[:], b1B.rearrange("p (m q) -> p m q", q=128))
        ones_sb = const.tile([128, 160], DT.bfloat16, tag="ones")
        nc.vector.memset(ones_sb[:], 1.0)
        b2_sb = const.tile([128, H // 128, 128], DT.bfloat16, tag="b2B")
        nc.sync.dma_start(b2_sb[:], b2B.rearrange("p (m q) -> p m q", q=128))
        x_kc = [[None] * NCHA for _ in range(DJ)]
        w1_k = []
        for j in range(DJ):
            wk = pA.tile([128, 2 * HID], DT.float8e4, tag=f"w1_{j}",
                         name=f"w1_{j}")
            nc.sync.dma_start(wk[:], w1T[j * 128:(j + 1) * 128, :])
            w1_k.append(wk)
            xk = pA.tile([128, 2 * CHA], DT.float8e4, tag=f"x{j}_0",
                         name=f"x{j}_0")
            nc.sync.dma_start(xk[:], xT[j * 128:(j + 1) * 128, 0:2 * CHA])
            x_kc[j][0] = xk

        # encoder-phase constants
        w2_sb = const.tile([128, HJ, 2 * H], DT.float8e4, tag="w2")
        nc.sync.dma_start(w2_sb[:], w2T.rearrange("(j p) c -> p j c", p=128))
        cell_sb = {}
        for nm in ["en1", "en2", "en3"]:
            e = {}
            e["wih"] = dma_in2(const, cellT[nm]["wih"], f"{nm}_wih")
            e["whh"] = dma_in2(const, cellT[nm]["whh"], f"{nm}_whh")
            e["bB"] = dma_in2(const, cellT[nm]["bB"], f"{nm}_bB")
            cell_sb[nm] = e
        E_sb = dma_in2(const, Em, "Em")

        # remaining x chunks
        for c in range(1, NCHA):
            for j in range(DJ):
                xk = pA.tile([128, 2 * CHA], DT.float8e4, tag=f"x{j}_{c}",
                             name=f"x{j}_{c}")
                nc.sync.dma_start(
                    xk[:], xT[j * 128:(j + 1) * 128, c * 2 * CHA:(c + 1) * 2 * CHA])
                x_kc[j][c] = xk

        # decoder-phase constants
        for nm in ["de1", "de2", "de3"]:
            e = {}
            if "wih" in cellT[nm]:
                e["wih"] = dma_in2(const, cellT[nm]["wih"], f"{nm}_wih")
            e["whh"] = dma_in2(const, cellT[nm]["whh"], f"{nm}_whh")
            e["bB"] = dma_in2(const, cellT[nm]["bB"], f"{nm}_bB")
            cell_sb[nm] = e
        wd1_sb = const.tile([128, H // 128, HID], DT.bfloat16, tag="wd1")
        nc.sync.dma_start(wd1_sb[:], wd1T.rearrange("(o p) m -> p o m", p=128))
        bd1_sb = const.tile([128, HID // 128, 128], DT.bfloat16, tag="bd1B")
        nc.sync.dma_start(bd1_sb[:], bd1B.rearrange("p (m q) -> p m q", q=128))
        bd2_sb = dma_in2(const, bd2v, "bd2v")
        zh = const.tile([128, 2 * BL], DT.float8e4, tag="zh")
        zc = const.tile([128, 2 * BL], DT.float32, tag="zc")
        nc.vector.memset(zh[:], 0.0)
        nc.vector.memset(zc[:], 0.0)
        h3all = [const.tile([128, H // 128, CHD], DT.bfloat16, tag=f"h3all{c}",
                            name=f"h3all{c}")
                 for c in range(NCHD)]
        zs = [gi1p.tile([128, 2, CHA], DT.float8e4, tag=f"zs{c}",
                        name=f"zs{c}")
              for c in range(NCHA)]

        def phaseA_gen():
            """Emits all of phase A; yields chunks_done after each MM block."""
            for c in range(NCHA):
                z1c = pA.tile([128, HID // 128, CHA], DT.float8e4,
                              tag=f"z1_{c}", name=f"z1_{c}")
                if c == 0:
                    # chunk 0 runs while x/w1 DMAs land: j-outer so each
                    # arriving k-pair tile is consumed immediately
                    for half in range(2):
                        ms = range(half * 4, half * 4 + 4)
                        pss = [PS()[:, :CHA] for _ in ms]
                        for mi, m in enumerate(ms):
                            nc.tensor.matmul(
                                pss[mi], b1_sb[:, m, :], ones_sb[:, :CHA],
                                start=True, stop=False, skip_group_check=True)
                        for j in range(DJ):
                            wv = pair(w1_k[j][:])
                            xv = pair(x_kc[j][0][:])
                            for mi, m in enumerate(ms):
                                nc.tensor.matmul(
                                    pss[mi], wv[:, :, m * 128:(m + 1) * 128],
                                    xv,
                                    start=False, stop=(j == DJ - 1),
                                    perf_mode=DR,
                                )
                            if j % 4 == 3:
                                yield c
                        for mi, m in enumerate(ms):
                            nc.vector.tensor_scalar(
                                z1c[:, m, :], pss[mi], 1.0 / SW1, 0.0,
                                mybir.AluOpType.mult, mybir.AluOpType.max)
                else:
                    for m in range(HID // 128):
                        ps = PS()[:, :CHA]
                        nc.tensor.matmul(
                            ps, b1_sb[:, m, :], ones_sb[:, :CHA],
                            start=True, stop=False, skip_group_check=True)
                        for j0 in range(0, DJ, 4):
                            for j in range(j0, j0 + 4):
                                nc.tensor.matmul(
                                    ps,
                                    pair(w1_k[j][:])[:, :, m * 128:(m + 1) * 128],
                                    pair(x_kc[j][c][:]),
                                    start=False, stop=(j == DJ - 1),
                                    perf_mode=DR,
                                )
                            yield c
                        nc.vector.tensor_scalar(
                            z1c[:, m, :], ps, 1.0 / SW1, 0.0,
                            mybir.AluOpType.mult, mybir.AluOpType.max)
                for m in range(H // 128):
                    ps = PS()[:, :CHA]
                    nc.tensor.matmul(
                        ps, b2_sb[:, m, :], ones_sb[:, :CHA],
                        start=True, stop=False, skip_group_check=True)
                    for j in range(HJ):
                        nc.tensor.matmul(
                            ps,
                            pair(w2_sb[:, j, :])[:, :, m * 128:(m + 1) * 128],
                            z1c[:, 2 * j:2 * j + 2, :],
                            start=False, stop=(j == HJ - 1),
                            perf_mode=DR,
                        )
                    nc.vector.tensor_scalar(
                        zs[c][:, m, :], ps, 1.0 / SW2, 0.0,
                        mybir.AluOpType.mult, mybir.AluOpType.max)
                    yield c + (m == H // 128 - 1)
            while True:
                yield NCHA + 1

        genA = phaseA_gen()
        a_done = [0]

        def fillA(n=1):
            for _ in range(n):
                a_done[0] = max(a_done[0], next(genA))

        def needA(chunks):
            while a_done[0] < chunks + 1:
                fillA()

        # ========== LSTM cell (single) ====================================
        def lstm_cell(nm, x_in, h_prev, c_prev, htag, ctag,
                      h_out=None, c_out=None):
            e = cell_sb[nm]
            ps = PS()[:, :256]
            nc.tensor.matmul(ps, e["bB"][:], E_sb[:], start=True, stop=False)
            groups = []
            if x_in is not None:
                groups.append((pair(e["wih"][:]), x_in))
            groups.append((pair(e["whh"][:]), h_prev))
            ng = len(groups)
            for gidx, (wv, rv) in enumerate(groups):
                for m in range(8):
                    nc.tensor.matmul(
                        ps[:, m * 32:(m + 1) * 32],
                        wv[:, :, m * 128:(m + 1) * 128],
                        rv,
                        start=False,
                        stop=(gidx == ng - 1),
                        perf_mode=DR,
                        skip_group_check=True,
                    )
            g = gates.tile([128, 256], DT.float32, tag="g", name=f"g{uid[0]}")
            nc.scalar.activation(g[:, 0:192], ps[:, 0:192], AF.Sigmoid,
                                 scale=1.0 / SWC)
            nc.scalar.activation(g[:, 192:256], ps[:, 192:256], AF.Tanh,
                                 scale=1.0 / SWC)
            # packed: i: 0..63, f: 64..127, o: 128..191, g: 192..255
            t1 = gates.tile([128, 64], DT.float32, tag="t1", name=f"t1{uid[0]}")
            nc.gpsimd.tensor_mul(t1[:], g[:, 0:64], g[:, 192:256])
            c2 = gates.tile([128, 64], DT.float32, tag="c2", name=f"c2{uid[0]}")
            nc.vector.tensor_mul(c2[:], g[:, 64:128], c_prev)
            if c_out is None:
                c_new = state.tile([128, 64], DT.float32, tag=ctag,
                                   name=f"{ctag}{uid[0]}")
                c_out = c_new[:]
            else:
                c_new = None
            nc.vector.tensor_add(c_out, c2[:], t1[:])
            th = gates.tile([128, 64], DT.float32, tag="th", name=f"th{uid[0]}")
            nc.scalar.activation(th[:], c_out, AF.Tanh)
            if h_out is None:
                h_new = state.tile([128, 64], DT.float8e4, tag=htag,
                                   name=f"{htag}{uid[0]}")
                h_out = h_new[:]
            else:
                h_new = None
            nc.vector.tensor_mul(h_out, g[:, 128:192], th[:])
            return h_out, c_out

        # ========== Paired LSTM cells (A = layer-2 cell at t, B = layer-1
        # cell at t+1; both read only pair_prev, so their 512-col gate psums
        # share one bank and the elementwise chain runs at double width) ====
        def lstm_pair(nmA, nmB, hp_prev, cp_prev, xB, htag, ctag):
            eA, eB = cell_sb[nmA], cell_sb[nmB]
            # one two-bank psum tile: A gates in bank 0 (cols 0:256), B gates
            # in bank 1 (cols 512:768). Matmuls stay within a single bank
            # each (start=True zeroing is bank-granular), but the sigmoid /
            # tanh ACTs read both banks in one strided instruction.
            uid[0] += 1
            ps2 = psum2.tile([128, 1024], DT.float32, tag="psp",
                             name=f"psp{uid[0]}")
            psA = ps2[:, 0:256]
            psB = ps2[:, 512:768]
            nc.tensor.matmul(psA, eA["bB"][:], E_sb[:],
                             start=True, stop=False, skip_group_check=True)
            nc.tensor.matmul(psB, eB["bB"][:], E_sb[:],
                             start=True, stop=False, skip_group_check=True)
            xA = pair(hp_prev[:, 64:128])   # h of layer-1 cell at t
            hA = pair(hp_prev[:, 0:64])     # layer-2 cell's own h at t-1
            hB = pair(hp_prev[:, 64:128])   # layer-1 cell's own h at t
            for m in range(8):
                sl = slice(m * 32, (m + 1) * 32)
                nc.tensor.matmul(psA[:, sl],
                                 pair(eA["wih"][:])[:, :, m * 128:(m + 1) * 128],
                                 xA, start=False, stop=False,
                                 perf_mode=DR, skip_group_check=True)
            for m in range(8):
                sl = slice(m * 32, (m + 1) * 32)
                nc.tensor.matmul(psA[:, sl],
                                 pair(eA["whh"][:])[:, :, m * 128:(m + 1) * 128],
                                 hA, start=False, stop=True,
                                 perf_mode=DR, skip_group_check=True)
            if xB is not None:
                for m in range(8):
                    sl = slice(m * 32, (m + 1) * 32)
                    nc.tensor.matmul(psB[:, sl],
                                     pair(eB["wih"][:])[:, :, m * 128:(m + 1) * 128],
                                     xB, start=False, stop=False,
                                     perf_mode=DR, skip_group_check=True)
            for m in range(8):
                sl = slice(m * 32, (m + 1) * 32)
                nc.tensor.matmul(psB[:, sl],
                                 pair(eB["whh"][:])[:, :, m * 128:(m + 1) * 128],
                                 hB, start=False, stop=True,
                                 perf_mode=DR, skip_group_check=True)
            # gates tile packs both cells: [ifo|g] at 0 (A) and 256 (B)
            g = gates.tile([128, 512], DT.float32, tag="gp", name=f"gp{uid[0]}")
            cp_new = state.tile([128, 128], DT.float32, tag=ctag,
                                name=f"{ctag}{uid[0]}")
            th = gates.tile([128, 128], DT.float32, tag="thp",
                            name=f"thp{uid[0]}")
            hp_new = state.tile([128, 128], DT.float8e4, tag=htag,
                                name=f"{htag}{uid[0]}")
            gval = g.rearrange("p (two x) -> p two x", two=2)
            pv2 = ps2.rearrange("p (two x) -> p two x", two=2)
            nc.scalar.activation(gval[:, :, 0:192], pv2[:, :, 0:192],
                                 AF.Sigmoid, scale=1.0 / SWC)
            nc.scalar.activation(gval[:, :, 192:256], pv2[:, :, 192:256],
                                 AF.Tanh, scale=1.0 / SWC)
            # pair-wide elementwise on SBUF (strided [128, 2, 64] views)
            gv = g.rearrange("p (two x) -> p two x", two=2)
            cpv = cp_prev.rearrange("p (two s) -> p two s", two=2)
            t1 = gates.tile([128, 2, 64], DT.float32, tag="t1p",
                            name=f"t1p{uid[0]}")
            nc.gpsimd.tensor_mul(t1[:], gv[:, :, 0:64], gv[:, :, 192:256])
            c2 = gates.tile([128, 2, 64], DT.float32, tag="c2p",
                            name=f"c2p{uid[0]}")
            nc.vector.tensor_mul(c2[:], gv[:, :, 64:128], cpv)
            cnv = cp_new.rearrange("p (two s) -> p two s", two=2)
            nc.vector.tensor_add(cnv, c2[:], t1[:])
            nc.scalar.activation(th[:], cp_new[:], AF.Tanh)
            hnv = hp_new.rearrange("p (two s) -> p two s", two=2)
            nc.vector.tensor_mul(hnv, gv[:, :, 128:192],
                                 th.rearrange("p (two s) -> p two s", two=2))
            return hp_new, cp_new

        # ========== Phase D transition (callable mid-encoder) =============
        wd2_k = []
        pD_box = []

        def ensure_pD():
            if pD_box:
                return
            pA_ctx.close()  # free phase A SBUF before loading decoder weights
            pD = ctx.enter_context(tc.tile_pool(name="phaseD", bufs=1))
            pD_box.append(pD)
            for k in range(HID // 128):
                wk = pD.tile([128, D], DT.bfloat16, tag=f"wd2_{k}",
                             name=f"wd2_{k}")
                nc.sync.dma_start(wk[:], wd2T[k * 128:(k + 1) * 128, :])
                wd2_k.append(wk)

        # ========== Encoder recurrence, pipelined w/ phase A fillers ======
        # pair state: hp = [h2(t-1), h1(t)] (fp8), cp likewise (fp32)
        needA(0)
        h3s, c3s = [None] * S, [None] * S
        hp = state.tile([128, 128], DT.float8e4, tag="hp", name="hp_init")
        cp = state.tile([128, 128], DT.float32, tag="cp", name="cp_init")
        nc.vector.memset(hp[:, 0:64], 0.0)
        nc.vector.memset(cp[:, 0:64], 0.0)
        lstm_cell("en1", z_ap(0), pair(zh[:]), zc[:], "h1", "c1",
                  h_out=hp[:, 64:128], c_out=cp[:, 64:128])
        for t in range(S):
            fillA(3)
            if t + 1 < S:
                needA((t + 1) // SPC)
                hp_new, cp_new = lstm_pair("en2", "en1", hp, cp,
                                           z_ap(t + 1), "hp", "cp")
                h2t = hp_new[:, 0:64]
            else:
                h2t, _ = lstm_cell("en2", pair(hp[:, 64:128]),
                                   pair(hp[:, 0:64]), cp[:, 0:64], "h2", "c2")
            fillA(3)
            h3p = h3s[t - 1] if t else zh[:]
            c3p = c3s[t - 1] if t else zc[:]
            h3s[t], c3s[t] = lstm_cell("en3", pair(h2t), pair(h3p), c3p,
                                       "h3", "c3")
            if t + 1 < S:
                hp, cp = hp_new, cp_new
            if t == S - 5:
                needA(NCHA)  # drain phase A now so decoder weights can load
                ensure_pD()

        ensure_pD()
        pD = pD_box[0]

        def phaseD_gen(c):
            y1c = pD.tile([128, HID // 128, CHD], DT.bfloat16,
                          tag=f"y1_{c}", name=f"y1_{c}")
            for m in range(HID // 128):
                ps = PS()[:, :CHD]
                nc.tensor.matmul(
                    ps, bd1_sb[:, m, :], ones_sb[:, :CHD],
                    start=True, stop=False, skip_group_check=True)
                for k in range(H // 128):
                    nc.tensor.matmul(
                        ps, wd1_sb[:, k, m * 128:(m + 1) * 128],
                        h3all[c][:, k, :],
                        start=False, stop=(k == H // 128 - 1),
                    )
                nc.vector.tensor_scalar(
                    y1c[:, m, :], ps, 1.0, 0.0,
                    mybir.AluOpType.mult, mybir.AluOpType.max)
                if m % 2 == 1:
                    yield
            for m in range(D // 128):
                ps = PS()[:, :CHD]
                for k in range(HID // 128):
                    nc.tensor.matmul(
                        ps, wd2_k[k][:, m * 128:(m + 1) * 128],
                        y1c[:, k, :],
                        start=(k == 0), stop=(k == HID // 128 - 1),
                    )
                o_sb = outp.tile([128, CHD], DT.float32, tag="o",
                                 name=f"o{uid[0]}")
                nc.scalar.activation(o_sb[:], ps, AF.Tanh,
                                     bias=bd2_sb[:, m:m + 1])
                nc.sync.dma_start(
                    yT[m * 128:(m + 1) * 128, c * CHD:(c + 1) * CHD],
                    o_sb[:])
                yield

        genDs = [phaseD_gen(c) for c in range(NCHD)]
        d_ready = [0]   # decoder chunks whose h3all is complete

        def fillD(n=1):
            for _ in range(n):
                for c in range(d_ready[0]):
                    if next(genDs[c], None) is not None:
                        break

        # ========== Decoder recurrence, pipelined w/ phase D fillers ======
        d3s, f3s = [None] * F, [None] * F
        dp = state.tile([128, 128], DT.float8e4, tag="dp", name="dp_init")
        fp = state.tile([128, 128], DT.float32, tag="fp", name="fp_init")
        nc.vector.memset(dp[:, 0:64], 0.0)
        nc.vector.memset(fp[:, 0:64], 0.0)
        lstm_cell("de1", None, pair(h3s[S - 1]), zc[:], "d1", "e1",
                  h_out=dp[:, 64:128], c_out=fp[:, 64:128])
        for t in range(F):
            fillD(4)
            if t + 1 < F:
                dp_new, fp_new = lstm_pair("de2", "de1", dp, fp, None,
                                           "dp", "fp")
                d2t = dp_new[:, 0:64]
            else:
                d2t, _ = lstm_cell("de2", pair(dp[:, 64:128]),
                                   pair(dp[:, 0:64]), fp[:, 0:64], "d2", "e2")
            fillD(4)
            d3p = d3s[t - 1] if t else zh[:]
            f3p = f3s[t - 1] if t else zc[:]
            d3s[t], f3s[t] = lstm_cell("de3", pair(d2t), pair(d3p), f3p,
                                       "d3", "e3")
            if t + 1 < F:
                dp, fp = dp_new, fp_new
            c, o = divmod(t, CHD // BL)
            nc.vector.tensor_copy(
                h3all[c][:, :, o * BL:(o + 1) * BL],
                d3s[t].rearrange("p (k s) -> p k s", s=BL),
            )
            if o == CHD // BL - 1:
                d_ready[0] = c + 1
        # drain remaining phase D work
        for gd in genDs:
            for _ in gd:
                pass

    nsplit = _split_sync_waits(nc, limit=1)
    _log(f"split {nsplit} over-limit sync waits")
    return nc

# ---------------------------------------------------------------------------
# Host-side input prep
# ---------------------------------------------------------------------------
GATE_PERM = np.concatenate([
    np.arange(0, 2 * H),          # i, f
    np.arange(3 * H, 4 * H),      # o
    np.arange(2 * H, 3 * H),      # g
])


def _dr256(wT, scale):
    """[K, M] f32 (K = 256) -> DoubleRow fp8 layout [128, 2*M]:
    col i*M+m holds wT[128i+p, m] * scale."""
    K, M = wT.shape
    assert K == 256
    a = (wT * scale).reshape(2, 128, M).transpose(1, 0, 2).reshape(128, 2 * M)
    return np.ascontiguousarray(a).astype(F8NP)


def _drK(wT, scale):
    """[K, M] f32 (K = 256*J) -> [J*128, 2*M] fp8: row j*128+p,
    col i*M+m holds wT[256j+128i+p, m] * scale."""
    K, M = wT.shape
    J = K // 256
    a = (wT * scale).reshape(J, 2, 128, M).transpose(0, 2, 1, 3)
    return np.ascontiguousarray(a.reshape(J * 128, 2 * M)).astype(F8NP)


def prep_inputs(inputs):
    f32 = np.float32
    g = {k: np.asarray(v) for k, v in inputs.items()}
    F = int(np.asarray(g["future_step"]))

    shared = {}
    shared["w1T"] = _drK(np.asarray(g["fc_en1_w"].T, f32), SW1)
    b1B = np.zeros((128, HID // 128, 128), f32)
    b1B[0] = g["fc_en1_b"].astype(f32).reshape(HID // 128, 128) * SW1
    shared["b1B"] = b1B.reshape(128, HID).astype(BF16NP)
    shared["w2T"] = _drK(np.asarray(g["fc_en2_w"].T, f32), SW2)
    b2B = np.zeros((128, H // 128, 128), f32)
    b2B[0] = g["fc_en2_b"].astype(f32).reshape(H // 128, 128) * SW2
    shared["b2B"] = b2B.reshape(128, H).astype(BF16NP)
    for nm in CELLS:
        wih = g[nm + "_wih"][GATE_PERM]
        whh = g[nm + "_whh"][GATE_PERM]
        bsum = (g[nm + "_bih"] + g[nm + "_bhh"])[GATE_PERM].astype(f32) * SWC
        if nm != "de1":
            shared[nm + "_wihT"] = _dr256(np.asarray(wih.T, f32), SWC)
        shared[nm + "_whhT"] = _dr256(np.asarray(whh.T, f32), SWC)
        bB = np.zeros((128, 128), f32)
        bB[:G // 128, :] = bsum.reshape(G // 128, 128)
        shared[nm + "_bB"] = bB.astype(BF16NP)
    E = np.zeros((128, 256), f32)
    for j in range(8):
        E[j, j * 32:(j + 1) * 32] = 1.0
    shared["Em"] = E.astype(BF16NP)
    shared["wd1T"] = np.ascontiguousarray(g["fc_de1_w"].T).astype(BF16NP)
    bd1B = np.zeros((128, HID // 128, 128), f32)
    bd1B[0] = g["fc_de1_b"].astype(f32).reshape(HID // 128, 128)
    shared["bd1B"] = bd1B.reshape(128, HID).astype(BF16NP)
    shared["wd2T"] = np.ascontiguousarray(g["fc_de2_w"].T).astype(BF16NP)
    shared["bd2v"] = np.ascontiguousarray(
        g["fc_de2_b"].astype(f32).reshape(D // 128, 128).T)

    x = g["x"].astype(f32).reshape(S, B, D)
    NCHA, CHA = 4, SB // 4
    in_maps = []
    for c in range(NCORES):
        xc = x[:, c * BL:(c + 1) * BL, :].reshape(SB, D)   # row = t*BL + b
        # DoubleRow chunked layout: row j*128+p, col ch*2*CHA + i*CHA + s
        # holds x[feature 256j+128i+p, sample ch*CHA+s]
        xT = xc.T.reshape(DJ, 2, 128, NCHA, CHA).transpose(0, 2, 3, 1, 4)
        m = dict(shared)
        m["xT"] = np.ascontiguousarray(
            xT.reshape(DJ * 128, 2 * SB)).astype(F8NP)
        in_maps.append(m)
    return in_maps, F


# ---------------------------------------------------------------------------
# Execution via PJRT (axon), modeled on bass2jax.run_bass_via_pjrt
# ---------------------------------------------------------------------------
def run_spmd(nc, in_maps, n_timing=0):
    import jax
    from jax.experimental.shard_map import shard_map
    from jax.sharding import Mesh, NamedSharding, PartitionSpec

    from concourse import bass2jax

    bass2jax.install_neuronx_cc_hook()
    n_cores = len(in_maps)
    partition_name = nc.partition_id_tensor.name if nc.partition_id_tensor else None
    in_names, out_names, out_avals, zero_outs = [], [], [], []
    for alloc in nc.m.functions[0].allocations:
        if not isinstance(alloc, mybir.MemoryLocationSet):
            continue
        name = alloc.memorylocations[0].name
        if alloc.kind == "ExternalInput":
            if name != partition_name:
                in_names.append(name)
        elif alloc.kind == "ExternalOutput":
            out_names.append(name)
            shape = tuple(alloc.tensor_shape)
            dtype = mybir.dt.np(alloc.dtype)
            out_avals.append(jax.core.ShapedArray(shape, dtype))
            zero_outs.append(np.zeros(shape, dtype))
    n_params = len(in_names)
    all_in = in_names + out_names
    if partition_name is not None:
        all_in = all_in + [partition_name]
    all_in = tuple(all_in)

    def _bind(args):
        operands = list(args)
        if partition_name is not None:
            operands.append(bass2jax.partition_id_tensor())
        return bass2jax._bass_exec_p.bind(
            *operands,
            out_avals=tuple(out_avals),
            in_names=all_in,
            out_names=tuple(out_names),
            lowering_input_output_aliases=(),
            sim_require_finite=False,
            sim_require_nnan=False,
            nc=nc,
        )

    def _body(*args):
        return tuple(_bind(args))

    devices = jax.devices()[:n_cores]
    mesh = Mesh(np.asarray(devices), ("core",))
    pspec = PartitionSpec("core")
    in_specs = (pspec,) * (n_params + len(out_names))
    out_specs = (pspec,) * len(out_names)

    f1 = jax.jit(shard_map(_body, mesh=mesh, in_specs=in_specs,
                           out_specs=out_specs, check_rep=False))
    concat = [
        np.concatenate([np.asarray(in_maps[c][nm]) for c in range(n_cores)], axis=0)
        for nm in in_names
    ]
    concat += [np.concatenate([z] * n_cores, axis=0) for z in zero_outs]

    sharding = NamedSharding(mesh, pspec)
    t0 = time.perf_counter()
    dev_in = [jax.device_put(a, sharding) for a in concat]
    jax.block_until_ready(dev_in)
    _log(f"upload {sum(a.nbytes for a in concat)/1e6:.1f} MB in "
         f"{time.perf_counter()-t0:.2f}s")

    t0 = time.perf_counter()
    outs = jax.block_until_ready(f1(*dev_in))
    _log(f"first run (incl compile) {time.perf_counter()-t0:.1f}s")

    results = []
    np_outs = [np.asarray(o) for o in outs]
    for c in range(n_cores):
        r = {}
        for i, nm in enumerate(out_names):
            sh0 = out_avals[i].shape[0]
            r[nm] = np_outs[i][c * sh0:(c + 1) * sh0]
        results.append(r)

    wall = None
    if n_timing:
        ts = []
        for _ in range(n_timing):
            t0 = time.perf_counter()
            jax.block_until_ready(f1(*dev_in))
            ts.append(time.perf_counter() - t0)
        wall = min(ts)
        _log("wall per call ms: " + " ".join(f"{t*1e3:.2f}" for t in ts))
    return results, wall, (f1, dev_in)


def measure_hw_time(F, in_maps, nrep=9, reps=14, nblocks=6, block=14):
    """HW exec estimate: block design. Same-program bursts (no per-call NEFF
    swap inside a block), alternating blocks between the 1-rep and nrep-rep
    programs to cancel slow drift of the ~85ms RPC floor. Per-iter =
    (median over blocks of block-median wall) diff / (nrep - 1). The older
    interleaved-min protocol swaps NEFFs every call, which adds a
    program-size-dependent cost and ~200us of noise to the estimate."""
    import jax

    import numpy as _np

    nc1 = build_program(F, nrep=1)
    _, _, (f1, dev1) = run_spmd(nc1, in_maps)
    ncN = build_program(F, nrep=nrep)
    _, _, (fN, devN) = run_spmd(ncN, in_maps)
    meds = {1: [], nrep: []}
    for blk in range(nblocks):
        for key, f, dev in ((1, f1, dev1), (nrep, fN, devN)):
            ts = []
            for i in range(block):
                t0 = time.perf_counter()
                jax.block_until_ready(f(*dev))
                ts.append(time.perf_counter() - t0)
            meds[key].append(float(_np.median(ts[2:])))
    # adjacent 1-rep / nrep-rep blocks share slow drift: difference them
    # pairwise, then take the median over pairs
    diffs = [(bN - b1) / (nrep - 1)
             for b1, bN in zip(meds[1], meds[nrep])]
    per_iter = max(float(_np.median(diffs)), 0.0)
    _log("block medians 1: " + " ".join(f"{t*1e3:.2f}" for t in meds[1]))
    _log(f"block medians {nrep}: " + " ".join(f"{t*1e3:.2f}" for t in meds[nrep]))
    _log("paired us/iter: " + " ".join(f"{d*1e6:.0f}" for d in diffs))
    _log(f"measure: -> {per_iter*1e6:.1f}us/iter")
    return per_iter * 1e9


_LAST_TIMING = None


def kernel(**inputs) -> np.ndarray:
    t0 = time.perf_counter()
    in_maps, F = prep_inputs(inputs)
    _log(f"host prep {time.perf_counter()-t0:.2f}s")
    t0 = time.perf_counter()
    nc = build_program(F)
    _log(f"build+tile {time.perf_counter()-t0:.1f}s")
    results, _, _ = run_spmd(nc, in_maps)
    out = np.empty((F, B, 64, 64), np.float32)
    for c in range(NCORES):
        yT = results[c]["yT"]                      # [4096, F*32]
        y = yT.T.reshape(F, BL, 64, 64)
        out[:, c * BL:(c + 1) * BL] = y
    return out


# revision 29
# speedup vs baseline: 3.2382x; 1.5951x over previous
"""Trainium2 Bass kernel for nn_FC_LSTM (FC-LSTM encoder-decoder).

Strategy:
  - Data-parallel over batch: 256 samples -> 8 cores x 32 samples.
  - Feature-major layout on chip: activations stored [feature(part), sample(free)],
    weights host-transposed to [in_feat, out_feat] so every matmul is
    out[feat_out, samples] = W_T.T @ act with contraction on partitions.
  - All matmuls in fp8e4 (e4m3) with MatmulPerfMode.DoubleRow: each
    instruction contracts a 256-wide K (two 128 k-tiles packed in dim1 of
    both operands), doubling PE throughput and halving PE instruction
    count vs bf16. Weights are host-quantized with power-of-2 scales
    (max|w|*s = 128 < 240); the descale folds into the activation
    instruction's input scale (out = func(scale*psum + bias)).
  - Encoder FC stack (4096->1024->256) batched over all 20 timesteps (640
    samples per core). en1's input-side gate matmul (Wih @ z_t) is also
    precomputed densely for all t at gate scale 2^11.
  - LSTM recurrence: per step the layer-2 cell at t and the layer-1 cell at
    t+1 depend only on the previous step's pair state, so they are emitted
    as a PAIR: two PSUM banks (a shared bank trips psum start=True
    whole-bank zeroing), but one [128, 512] SBUF gates tile and
    double-width elementwise (sig/tanh ACT per bank, then pair-wide
    mul/add on DVE + Pool, one tanh(c) ACT, one fp8 h-mul). Gate psum col
    m*32+s = gate-feature 128m+p of sample s (scale 2^11); gate order
    host-permuted to [i, f, o, g]. Cell biases (x2^11) are injected with a
    single K=128 bf16 matmul against a constant indicator matrix; FC biases
    likewise ride in the psum via a bias-in-row-0 bf16 matmul against a
    ones column, so FC relu+descale is a single 2-op DVE tensor_scalar
    (no ACT involvement outside the cells and the output tanh).
    h state is stored fp8, c state fp32.
  - Decoder FC stack (256->1024->4096) batched over all future steps.
  - The FC phases are emitted as generators whose matmul blocks interleave
    into the recurrence as PE gap fillers; DMAs are emitted in first-use
    order (small bias/ones constants first: they gate FC psum-group
    starts).
  - measure_hw_time uses a block design: bursts of same-program calls
    (per-call NEFF swap costs otherwise contaminate the estimate),
    alternating blocks of the 1-rep and 9-rep programs, difference of
    median block-medians.
"""

import time
from contextlib import ExitStack

import ml_dtypes
import numpy as np

import concourse.bass as bass
import concourse.mybir as mybir
import concourse.tile as tile

BF16NP = ml_dtypes.bfloat16
F8NP = ml_dtypes.float8_e4m3
AF = mybir.ActivationFunctionType
DT = mybir.dt
DR = mybir.MatmulPerfMode.DoubleRow

S = 20          # encoder sequence length
B = 256         # global batch
NCORES = 8
BL = B // NCORES  # 32 samples per core
H = 256         # LSTM hidden
G = 4 * H       # 1024 gate features
D = 4096        # input feature dim (64*64)
HID = 1024      # FC hidden
SB = S * BL     # 640 encoder samples per core

# power-of-2 quant scales: max|w| * SW = 128 (e4m3 max finite = 240)
SW1 = 2.0 ** 13    # fc_en1 w ~ U(+-2^-6)
SW2 = 2.0 ** 12    # fc_en2 w ~ U(+-2^-5)
SWC = 2.0 ** 11    # cell weights ~ U(+-2^-4); gate psum scale
SWD1 = 2.0 ** 11   # fc_de1 w ~ U(+-2^-4)
SWD2 = 2.0 ** 12   # fc_de2 w ~ U(+-2^-5)

VERBOSE = True


def _log(*a):
    if VERBOSE:
        print("[kernel]", *a, flush=True)


# ---------------------------------------------------------------------------
# Workaround: walrus CoreV3 setupSyncWait allows only 1 sync wait on the
# TileContext exit Drain. Split its waits across multiple drain instructions.
# ---------------------------------------------------------------------------
def _patched_drain_and_barrier(self, tick_clock, wait_clock):
    nc = self.nc
    drain_inst = nc.sync.drain()
    wait_clock.add_sem_waits(
        drain_inst.ins, tile.ScopedClock({None: tick_clock.global_clock})
    )
    inst = drain_inst.ins
    si = inst.sync_info
    waits = list(si.on_wait) if si is not None and si.on_wait else []
    MAXW = 1
    if len(waits) > MAXW:
        si.on_wait = waits[:MAXW]
        for i in range(MAXW, len(waits), MAXW):
            d2 = nc.sync.drain()
            i2 = d2.ins
            si2 = i2.sync_info
            if si2 is None:
                i2.sync_info = type(si)(on_wait=waits[i : i + MAXW], on_update=[])
            else:
                si2.on_wait = list(si2.on_wait or []) + waits[i : i + MAXW]

    nc.all_engine_barrier()
    assert self.sems is not None
    popped = nc._tile_sem_poison_stack.pop()
    assert popped is self._sem_poison
    nc.clear_and_free_semaphores(list(self.sems.allocated().values()))
    nc.all_engine_barrier()


tile.TileContext._drain_and_barrier = _patched_drain_and_barrier


def _split_sync_waits(nc, limit=1):
    """walrus setupSyncWait rejects >2 sem waits per instruction: move excess
    waits onto same-engine NoOps spliced just before the instruction."""
    ctr = [0]
    SyncInfo = None
    for f in nc.m.functions:
        for bb in f.blocks:
            out = []
            for inst in bb.instructions:
                si = inst.sync_info
                waits = list(si.on_wait) if si is not None and si.on_wait else []
                if len(waits) > limit:
                    if SyncInfo is None:
                        SyncInfo = type(si)
                    extras = waits[: len(waits) - limit]
                    si.on_wait = waits[len(waits) - limit:]
                    for i in range(0, len(extras), limit):
                        ctr[0] += 1
                        nop = mybir.InstNoOp(name=f"ws_{ctr[0]}", ins=[], outs=[])
                        nop.engine = inst.engine
                        nop.sync_info = SyncInfo(
                            on_wait=extras[i : i + limit], on_update=[]
                        )
                        out.append(nop)
                out.append(inst)
            bb.instructions[:] = out
    return ctr[0]


# ---------------------------------------------------------------------------
# Program builder
# ---------------------------------------------------------------------------
CELLS = ["en1", "en2", "en3", "de1", "de2", "de3"]
DJ = D // 256       # 16 k-tile-pairs of the 4096 input dim
HJ = HID // 256     # 4 k-tile-pairs of the 1024 hidden dim


def build_program(F: int, nrep: int = 1) -> bass.Bass:
    FB = F * BL  # decoder samples per core
    assert FB <= 512 and FB % 2 == 0
    nc = bass.Bass()

    # --- DRAM tensors (all fp8 weight layouts are DoubleRow-paired:
    #     row j*128+p, col i*M+m holds W[256j+128i+p, m]) ---
    xT = nc.dram_tensor("xT", [DJ * 128, 2 * SB], DT.float8e4, kind="ExternalInput")
    w1T = nc.dram_tensor("w1T", [DJ * 128, 2 * HID], DT.float8e4, kind="ExternalInput")
    b1B = nc.dram_tensor("b1B", [128, HID], DT.bfloat16, kind="ExternalInput")
    w2T = nc.dram_tensor("w2T", [HJ * 128, 2 * H], DT.float8e4, kind="ExternalInput")
    b2B = nc.dram_tensor("b2B", [128, H], DT.bfloat16, kind="ExternalInput")
    cellT = {}
    for nm in CELLS:
        ins = {}
        if nm != "de1":
            ins["wih"] = nc.dram_tensor(f"{nm}_wihT", [128, 2 * G], DT.float8e4,
                                        kind="ExternalInput")
        ins["whh"] = nc.dram_tensor(f"{nm}_whhT", [128, 2 * G], DT.float8e4,
                                    kind="ExternalInput")
        ins["bB"] = nc.dram_tensor(f"{nm}_bB", [128, 128], DT.bfloat16,
                                   kind="ExternalInput")
        cellT[nm] = ins
    Em = nc.dram_tensor("Em", [128, 256], DT.bfloat16, kind="ExternalInput")
    # decoder head stays bf16: its pre-tanh output is tiny relative to the
    # summand magnitudes (heavy cancellation), so fp8 there blows the error
    # budget (measured: fp8 fc_de2 alone -> 3.3e-2 rel err).
    wd1T = nc.dram_tensor("wd1T", [H, HID], DT.bfloat16, kind="ExternalInput")
    bd1B = nc.dram_tensor("bd1B", [128, HID], DT.bfloat16, kind="ExternalInput")
    wd2T = nc.dram_tensor("wd2T", [HID, D], DT.bfloat16, kind="ExternalInput")
    bd2v = nc.dram_tensor("bd2v", [128, D // 128], DT.float32, kind="ExternalInput")
    yT = nc.dram_tensor("yT", [D, FB], DT.float32, kind="ExternalOutput")

    NCHA = 4                      # phase A chunks (5 encoder steps each)
    CHA = SB // NCHA              # 160 samples
    SPC = S // NCHA               # steps per chunk
    NCHD = 2                      # phase D chunks
    CHD = FB // NCHD

    def pair(ap):
        """[128, 2*N] fp8 AP -> [128, 2, N] DoubleRow operand view."""
        return ap.rearrange("p (i n) -> p i n", i=2)

    with tile.TileContext(nc) as tc:
     for rep in range(nrep):
      with ExitStack() as ctx:
        const = ctx.enter_context(tc.tile_pool(name="const", bufs=1))
        gi1p = ctx.enter_context(tc.tile_pool(name="gi1p", bufs=1))
        state = ctx.enter_context(tc.tile_pool(name="state", bufs=3))
        gates = ctx.enter_context(tc.tile_pool(name="gates", bufs=4))
        outp = ctx.enter_context(tc.tile_pool(name="outp", bufs=4))
        psum = ctx.enter_context(tc.tile_pool(name="psum", bufs=6, space="PSUM"))
        psum2 = ctx.enter_context(tc.tile_pool(name="psum2", bufs=1,
                                               space="PSUM"))

        uid = [0]

        def PS():
            uid[0] += 1
            return psum.tile([128, 512], DT.float32, tag="ps",
                             name=f"ps{uid[0]}")

        def dma_in2(pool, dram, tag):
            t = pool.tile(list(dram.shape), dram.dtype, tag=tag)
            nc.sync.dma_start(t[:], dram[:])
            return t

        def z_ap(t):
            c, o = divmod(t, SPC)
            return zs[c][:, :, o * BL:(o + 1) * BL]

        # ========== Phase A generator: per-k-tile weight/x DMAs, chunked ===
        pA_ctx = ExitStack()
        pA = pA_ctx.enter_context(tc.tile_pool(name="phaseA", bufs=1))

        # DMA emission ordered by first use: w1 + x chunk 0 feed the FC chain
        # immediately; encoder cell weights arrive next; remaining x chunks
        # stream during the early recurrence; decoder weights last.
        # tiny constants first: the FC1 bias matmuls (group starts) need
        # b1/ones immediately; don't queue them behind 5MB of w1/x stream
        b1_sb = const.tile([128, HID // 128, 128], DT.bfloat16, tag="b1B")
        nc.sync.dma_start(b1_sb[:], b1B.rearrange("p (m q) -> p m q", q=128))
        ones_sb = const.tile([128, 160], DT.bfloat16, tag="ones")
        nc.vector.memset(ones_sb[:], 1.0)
        b2_sb = const.tile([128, H // 128, 128], DT.bfloat16, tag="b2B")
        nc.sync.dma_start(b2_sb[:], b2B.rearrange("p (m q) -> p m q", q=128))
        x_kc = [[None] * NCHA for _ in range(DJ)]
        w1_k = []
        for j in range(DJ):
            wk = pA.tile([128, 2 * HID], DT.float8e4, tag=f"w1_{j}",
                         name=f"w1_{j}")
            nc.sync.dma_start(wk[:], w1T[j * 128:(j + 1) * 128, :])
            w1_k.append(wk)
            xk = pA.tile([128, 2 * CHA], DT.float8e4, tag=f"x{j}_0",
                         name=f"x{j}_0")
            nc.sync.dma_start(xk[:], xT[j * 128:(j + 1) * 128, 0:2 * CHA])
            x_kc[j][0] = xk

        # encoder-phase constants
        w2_sb = const.tile([128, HJ, 2 * H], DT.float8e4, tag="w2")
        nc.sync.dma_start(w2_sb[:], w2T.rearrange("(j p) c -> p j c", p=128))
        cell_sb = {}
        for nm in ["en1", "en2", "en3"]:
            e = {}
            e["wih"] = dma_in2(const, cellT[nm]["wih"], f"{nm}_wih")
            e["whh"] = dma_in2(const, cellT[nm]["whh"], f"{nm}_whh")
            e["bB"] = dma_in2(const, cellT[nm]["bB"], f"{nm}_bB")
            cell_sb[nm] = e
        E_sb = dma_in2(const, Em, "Em")

        # remaining x chunks
        for c in range(1, NCHA):
            for j in range(DJ):
                xk = pA.tile([128, 2 * CHA], DT.float8e4, tag=f"x{j}_{c}",
                             name=f"x{j}_{c}")
                nc.sync.dma_start(
                    xk[:], xT[j * 128:(j + 1) * 128, c * 2 * CHA:(c + 1) * 2 * CHA])
                x_kc[j][c] = xk

        # decoder-phase constants
        for nm in ["de1", "de2", "de3"]:
            e = {}
            if "wih" in cellT[nm]:
                e["wih"] = dma_in2(const, cellT[nm]["wih"], f"{nm}_wih")
            e["whh"] = dma_in2(const, cellT[nm]["whh"], f"{nm}_whh")
            e["bB"] = dma_in2(const, cellT[nm]["bB"], f"{nm}_bB")
            cell_sb[nm] = e
        wd1_sb = const.tile([128, H // 128, HID], DT.bfloat16, tag="wd1")
        nc.sync.dma_start(wd1_sb[:], wd1T.rearrange("(o p) m -> p o m", p=128))
        bd1_sb = const.tile([128, HID // 128, 128], DT.bfloat16, tag="bd1B")
        nc.sync.dma_start(bd1_sb[:], bd1B.rearrange("p (m q) -> p m q", q=128))
        bd2_sb = dma_in2(const, bd2v, "bd2v")
        zh = const.tile([128, 2 * BL], DT.float8e4, tag="zh")
        zc = const.tile([128, 2 * BL], DT.float32, tag="zc")
        nc.vector.memset(zh[:], 0.0)
        nc.vector.memset(zc[:], 0.0)
        h3all = [const.tile([128, H // 128, CHD], DT.bfloat16, tag=f"h3all{c}",
                            name=f"h3all{c}")
                 for c in range(NCHD)]
        zs = [gi1p.tile([128, 2, CHA], DT.float8e4, tag=f"zs{c}",
                        name=f"zs{c}")
              for c in range(NCHA)]

        def phaseA_gen():
            """Emits all of phase A; yields chunks_done after each MM block."""
            for c in range(NCHA):
                z1c = pA.tile([128, HID // 128, CHA], DT.float8e4,
                              tag=f"z1_{c}", name=f"z1_{c}")
                if c == 0:
                    # chunk 0 runs while x/w1 DMAs land: j-outer so each
                    # arriving k-pair tile is consumed immediately
                    for half in range(2):
                        ms = range(half * 4, half * 4 + 4)
                        pss = [PS()[:, :CHA] for _ in ms]
                        for mi, m in enumerate(ms):
                            nc.tensor.matmul(
                                pss[mi], b1_sb[:, m, :], ones_sb[:, :CHA],
                                start=True, stop=False, skip_group_check=True)
                        for j in range(DJ):
                            wv = pair(w1_k[j][:])
                            xv = pair(x_kc[j][0][:])
                            for mi, m in enumerate(ms):
                                nc.tensor.matmul(
                                    pss[mi], wv[:, :, m * 128:(m + 1) * 128],
                                    xv,
                                    start=False, stop=(j == DJ - 1),
                                    perf_mode=DR,
                                )
                            if j % 4 == 3:
                                yield c
                        for mi, m in enumerate(ms):
                            nc.vector.tensor_scalar(
                                z1c[:, m, :], pss[mi], 1.0 / SW1, 0.0,
                                mybir.AluOpType.mult, mybir.AluOpType.max)
                else:
                    for m in range(HID // 128):
                        ps = PS()[:, :CHA]
                        nc.tensor.matmul(
                            ps, b1_sb[:, m, :], ones_sb[:, :CHA],
                            start=True, stop=False, skip_group_check=True)
                        for j0 in range(0, DJ, 4):
                            for j in range(j0, j0 + 4):
                                nc.tensor.matmul(
                                    ps,
                                    pair(w1_k[j][:])[:, :, m * 128:(m + 1) * 128],
                                    pair(x_kc[j][c][:]),
                                    start=False, stop=(j == DJ - 1),
                                    perf_mode=DR,
                                )
                            yield c
                        nc.vector.tensor_scalar(
                            z1c[:, m, :], ps, 1.0 / SW1, 0.0,
                            mybir.AluOpType.mult, mybir.AluOpType.max)
                for m in range(H // 128):
                    ps = PS()[:, :CHA]
                    nc.tensor.matmul(
                        ps, b2_sb[:, m, :], ones_sb[:, :CHA],
                        start=True, stop=False, skip_group_check=True)
                    for j in range(HJ):
                        nc.tensor.matmul(
                            ps,
                            pair(w2_sb[:, j, :])[:, :, m * 128:(m + 1) * 128],
                            z1c[:, 2 * j:2 * j + 2, :],
                            start=False, stop=(j == HJ - 1),
                            perf_mode=DR,
                        )
                    nc.vector.tensor_scalar(
                        zs[c][:, m, :], ps, 1.0 / SW2, 0.0,
                        mybir.AluOpType.mult, mybir.AluOpType.max)
                    yield c + (m == H // 128 - 1)
            while True:
                yield NCHA + 1

        genA = phaseA_gen()
        a_done = [0]

        def fillA(n=1):
            for _ in range(n):
                a_done[0] = max(a_done[0], next(genA))

        def needA(chunks):
            while a_done[0] < chunks + 1:
                fillA()

        # ========== LSTM cell (single) ====================================
        def lstm_cell(nm, x_in, h_prev, c_prev, htag, ctag,
                      h_out=None, c_out=None):
            e = cell_sb[nm]
            ps = PS()[:, :256]
            nc.tensor.matmul(ps, e["bB"][:], E_sb[:], start=True, stop=False)
            groups = []
            if x_in is not None:
                groups.append((pair(e["wih"][:]), x_in))
            groups.append((pair(e["whh"][:]), h_prev))
            ng = len(groups)
            for gidx, (wv, rv) in enumerate(groups):
                for m in range(8):
                    nc.tensor.matmul(
                        ps[:, m * 32:(m + 1) * 32],
                        wv[:, :, m * 128:(m + 1) * 128],
                        rv,
                        start=False,
                        stop=(gidx == ng - 1),
                        perf_mode=DR,
                        skip_group_check=True,
                    )
            g = gates.tile([128, 256], DT.float32, tag="g", name=f"g{uid[0]}")
            nc.scalar.activation(g[:, 0:192], ps[:, 0:192], AF.Sigmoid,
                                 scale=1.0 / SWC)
            nc.scalar.activation(g[:, 192:256], ps[:, 192:256], AF.Tanh,
                                 scale=1.0 / SWC)
            # packed: i: 0..63, f: 64..127, o: 128..191, g: 192..255
            t1 = gates.tile([128, 64], DT.float32, tag="t1", name=f"t1{uid[0]}")
            nc.gpsimd.tensor_mul(t1[:], g[:, 0:64], g[:, 192:256])
            c2 = gates.tile([128, 64], DT.float32, tag="c2", name=f"c2{uid[0]}")
            nc.vector.tensor_mul(c2[:], g[:, 64:128], c_prev)
            if c_out is None:
                c_new = state.tile([128, 64], DT.float32, tag=ctag,
                                   name=f"{ctag}{uid[0]}")
                c_out = c_new[:]
            else:
                c_new = None
            nc.vector.tensor_add(c_out, c2[:], t1[:])
            th = gates.tile([128, 64], DT.float32, tag="th", name=f"th{uid[0]}")
            nc.scalar.activation(th[:], c_out, AF.Tanh)
            if h_out is None:
                h_new = state.tile([128, 64], DT.float8e4, tag=htag,
                                   name=f"{htag}{uid[0]}")
                h_out = h_new[:]
            else:
                h_new = None
            nc.vector.tensor_mul(h_out, g[:, 128:192], th[:])
            return h_out, c_out

        # ========== Paired LSTM cells (A = layer-2 cell at t, B = layer-1
        # cell at t+1; both read only pair_prev, so their 512-col gate psums
        # share one bank and the elementwise chain runs at double width) ====
        def lstm_pair(nmA, nmB, hp_prev, cp_prev, xB, htag, ctag):
            eA, eB = cell_sb[nmA], cell_sb[nmB]
            # one two-bank psum tile: A gates in bank 0 (cols 0:256), B gates
            # in bank 1 (cols 512:768). Matmuls stay within a single bank
            # each (start=True zeroing is bank-granular), but the sigmoid /
            # tanh ACTs read both banks in one strided instruction.
            uid[0] += 1
            ps2 = psum2.tile([128, 1024], DT.float32, tag="psp",
                             name=f"psp{uid[0]}")
            psA = ps2[:, 0:256]
            psB = ps2[:, 512:768]
            nc.tensor.matmul(psA, eA["bB"][:], E_sb[:],
                             start=True, stop=False, skip_group_check=True)
            nc.tensor.matmul(psB, eB["bB"][:], E_sb[:],
                             start=True, stop=False, skip_group_check=True)
            xA = pair(hp_prev[:, 64:128])   # h of layer-1 cell at t
            hA = pair(hp_prev[:, 0:64])     # layer-2 cell's own h at t-1
            hB = pair(hp_prev[:, 64:128])   # layer-1 cell's own h at t
            for m in range(8):
                sl = slice(m * 32, (m + 1) * 32)
                nc.tensor.matmul(psA[:, sl],
                                 pair(eA["wih"][:])[:, :, m * 128:(m + 1) * 128],
                                 xA, start=False, stop=False,
                                 perf_mode=DR, skip_group_check=True)
            for m in range(8):
                sl = slice(m * 32, (m + 1) * 32)
                nc.tensor.matmul(psA[:, sl],
                                 pair(eA["whh"][:])[:, :, m * 128:(m + 1) * 128],
                                 hA, start=False, stop=True,
                                 perf_mode=DR, skip_group_check=True)
            if xB is not None:
                for m in range(8):
                    sl = slice(m * 32, (m + 1) * 32)
                    nc.tensor.matmul(psB[:, sl],
                                     pair(eB["wih"][:])[:, :, m * 128:(m + 1) * 128],
                                     xB, start=False, stop=False,
                                     perf_mode=DR, skip_group_check=True)
            for m in range(8):
                sl = slice(m * 32, (m + 1) * 32)
                nc.tensor.matmul(psB[:, sl],
                                 pair(eB["whh"][:])[:, :, m * 128:(m + 1) * 128],
                                 hB, start=False, stop=True,
                                 perf_mode=DR, skip_group_check=True)
            # gates tile packs both cells: [ifo|g] at 0 (A) and 256 (B)
            g = gates.tile([128, 512], DT.float32, tag="gp", name=f"gp{uid[0]}")
            cp_new = state.tile([128, 128], DT.float32, tag=ctag,
                                name=f"{ctag}{uid[0]}")
            th = gates.tile([128, 128], DT.float32, tag="thp",
                            name=f"thp{uid[0]}")
            hp_new = state.tile([128, 128], DT.float8e4, tag=htag,
                                name=f"{htag}{uid[0]}")
            gval = g.rearrange("p (two x) -> p two x", two=2)
            pv2 = ps2.rearrange("p (two x) -> p two x", two=2)
            nc.scalar.activation(gval[:, :, 0:192], pv2[:, :, 0:192],
                                 AF.Sigmoid, scale=1.0 / SWC)
            nc.scalar.activation(gval[:, :, 192:256], pv2[:, :, 192:256],
                                 AF.Tanh, scale=1.0 / SWC)
            # pair-wide elementwise on SBUF (strided [128, 2, 64] views)
            gv = g.rearrange("p (two x) -> p two x", two=2)
            cpv = cp_prev.rearrange("p (two s) -> p two s", two=2)
            t1 = gates.tile([128, 2, 64], DT.float32, tag="t1p",
                            name=f"t1p{uid[0]}")
            nc.gpsimd.tensor_mul(t1[:], gv[:, :, 0:64], gv[:, :, 192:256])
            c2 = gates.tile([128, 2, 64], DT.float32, tag="c2p",
                            name=f"c2p{uid[0]}")
            nc.vector.tensor_mul(c2[:], gv[:, :, 64:128], cpv)
            cnv = cp_new.rearrange("p (two s) -> p two s", two=2)
            nc.vector.tensor_add(cnv, c2[:], t1[:])
            nc.scalar.activation(th[:], cp_new[:], AF.Tanh)
            hnv = hp_new.rearrange("p (two s) -> p two s", two=2)
            nc.vector.tensor_mul(hnv, gv[:, :, 128:192],
                                 th.rearrange("p (two s) -> p two s", two=2))
            return hp_new, cp_new

        # ========== Phase D transition (callable mid-encoder) =============
        wd2_k = []
        pD_box = []

        def ensure_pD():
            if pD_box:
                return
            pA_ctx.close()  # free phase A SBUF before loading decoder weights
            pD = ctx.enter_context(tc.tile_pool(name="phaseD", bufs=1))
            pD_box.append(pD)
            for k in range(HID // 128):
                wk = pD.tile([128, D], DT.bfloat16, tag=f"wd2_{k}",
                             name=f"wd2_{k}")
                nc.sync.dma_start(wk[:], wd2T[k * 128:(k + 1) * 128, :])
                wd2_k.append(wk)

        # ========== Encoder recurrence, pipelined w/ phase A fillers ======
        # pair state: hp = [h2(t-1), h1(t)] (fp8), cp likewise (fp32)
        needA(0)
        h3s, c3s = [None] * S, [None] * S
        hp = state.tile([128, 128], DT.float8e4, tag="hp", name="hp_init")
        cp = state.tile([128, 128], DT.float32, tag="cp", name="cp_init")
        nc.vector.memset(hp[:, 0:64], 0.0)
        nc.vector.memset(cp[:, 0:64], 0.0)
        lstm_cell("en1", z_ap(0), pair(zh[:]), zc[:], "h1", "c1",
                  h_out=hp[:, 64:128], c_out=cp[:, 64:128])
        for t in range(S):
            fillA(3)
            if t + 1 < S:
                needA((t + 1) // SPC)
                hp_new, cp_new = lstm_pair("en2", "en1", hp, cp,
                                           z_ap(t + 1), "hp", "cp")
                h2t = hp_new[:, 0:64]
            else:
                h2t, _ = lstm_cell("en2", pair(hp[:, 64:128]),
                                   pair(hp[:, 0:64]), cp[:, 0:64], "h2", "c2")
            fillA(3)
            h3p = h3s[t - 1] if t else zh[:]
            c3p = c3s[t - 1] if t else zc[:]
            h3s[t], c3s[t] = lstm_cell("en3", pair(h2t), pair(h3p), c3p,
                                       "h3", "c3")
            if t + 1 < S:
                hp, cp = hp_new, cp_new
            if t == S - 5:
                needA(NCHA)  # drain phase A now so decoder weights can load
                ensure_pD()

        ensure_pD()
        pD = pD_box[0]

        def phaseD_gen(c):
            y1c = pD.tile([128, HID // 128, CHD], DT.bfloat16,
                          tag=f"y1_{c}", name=f"y1_{c}")
            for m in range(HID // 128):
                ps = PS()[:, :CHD]
                nc.tensor.matmul(
                    ps, bd1_sb[:, m, :], ones_sb[:, :CHD],
                    start=True, stop=False, skip_group_check=True)
                for k in range(H // 128):
                    nc.tensor.matmul(
                        ps, wd1_sb[:, k, m * 128:(m + 1) * 128],
                        h3all[c][:, k, :],
                        start=False, stop=(k == H // 128 - 1),
                    )
                nc.vector.tensor_scalar(
                    y1c[:, m, :], ps, 1.0, 0.0,
                    mybir.AluOpType.mult, mybir.AluOpType.max)
                if m % 2 == 1:
                    yield
            for m in range(D // 128):
                ps = PS()[:, :CHD]
                for k in range(HID // 128):
                    nc.tensor.matmul(
                        ps, wd2_k[k][:, m * 128:(m + 1) * 128],
                        y1c[:, k, :],
                        start=(k == 0), stop=(k == HID // 128 - 1),
                    )
                o_sb = outp.tile([128, CHD], DT.float32, tag="o",
                                 name=f"o{uid[0]}")
                nc.scalar.activation(o_sb[:], ps, AF.Tanh,
                                     bias=bd2_sb[:, m:m + 1])
                nc.sync.dma_start(
                    yT[m * 128:(m + 1) * 128, c * CHD:(c + 1) * CHD],
                    o_sb[:])
                yield

        genDs = [phaseD_gen(c) for c in range(NCHD)]
        d_ready = [0]   # decoder chunks whose h3all is complete

        def fillD(n=1):
            for _ in range(n):
                for c in range(d_ready[0]):
                    if next(genDs[c], None) is not None:
                        break

        # ========== Decoder recurrence, pipelined w/ phase D fillers ======
        d3s, f3s = [None] * F, [None] * F
        dp = state.tile([128, 128], DT.float8e4, tag="dp", name="dp_init")
        fp = state.tile([128, 128], DT.float32, tag="fp", name="fp_init")
        nc.vector.memset(dp[:, 0:64], 0.0)
        nc.vector.memset(fp[:, 0:64], 0.0)
        lstm_cell("de1", None, pair(h3s[S - 1]), zc[:], "d1", "e1",
                  h_out=dp[:, 64:128], c_out=fp[:, 64:128])
        for t in range(F):
            fillD(4)
            if t + 1 < F:
                dp_new, fp_new = lstm_pair("de2", "de1", dp, fp, None,
                                           "dp", "fp")
                d2t = dp_new[:, 0:64]
            else:
                d2t, _ = lstm_cell("de2", pair(dp[:, 64:128]),
                                   pair(dp[:, 0:64]), fp[:, 0:64], "d2", "e2")
            fillD(4)
            d3p = d3s[t - 1] if t else zh[:]
            f3p = f3s[t - 1] if t else zc[:]
            d3s[t], f3s[t] = lstm_cell("de3", pair(d2t), pair(d3p), f3p,
                                       "d3", "e3")
            if t + 1 < F:
                dp, fp = dp_new, fp_new
            c, o = divmod(t, CHD // BL)
            nc.gpsimd.tensor_copy(
                h3all[c][:, :, o * BL:(o + 1) * BL],
                d3s[t].rearrange("p (k s) -> p k s", s=BL),
            )
            if o == CHD // BL - 1:
                d_ready[0] = c + 1
        # drain remaining phase D work
        for gd in genDs:
            for _ in gd:
                pass

    nsplit = _split_sync_waits(nc, limit=1)
    _log(f"split {nsplit} over-limit sync waits")
    return nc

# ---------------------------------------------------------------------------
# Host-side input prep
# ---------------------------------------------------------------------------
GATE_PERM = np.concatenate([
    np.arange(0, 2 * H),          # i, f
    np.arange(3 * H, 4 * H),      # o
    np.arange(2 * H, 3 * H),      # g
])


def _dr256(wT, scale):
    """[K, M] f32 (K = 256) -> DoubleRow fp8 layout [128, 2*M]:
    col i*M+m holds wT[128i+p, m] * scale."""
    K, M = wT.shape
    assert K == 256
    a = (wT * scale).reshape(2, 128, M).transpose(1, 0, 2).reshape(128, 2 * M)
    return np.ascontiguousarray(a).astype(F8NP)


def _drK(wT, scale):
    """[K, M] f32 (K = 256*J) -> [J*128, 2*M] fp8: row j*128+p,
    col i*M+m holds wT[256j+128i+p, m] * scale."""
    K, M = wT.shape
    J = K // 256
    a = (wT * scale).reshape(J, 2, 128, M).transpose(0, 2, 1, 3)
    return np.ascontiguousarray(a.reshape(J * 128, 2 * M)).astype(F8NP)


def prep_inputs(inputs):
    f32 = np.float32
    g = {k: np.asarray(v) for k, v in inputs.items()}
    F = int(np.asarray(g["future_step"]))

    shared = {}
    shared["w1T"] = _drK(np.asarray(g["fc_en1_w"].T, f32), SW1)
    b1B = np.zeros((128, HID // 128, 128), f32)
    b1B[0] = g["fc_en1_b"].astype(f32).reshape(HID // 128, 128) * SW1
    shared["b1B"] = b1B.reshape(128, HID).astype(BF16NP)
    shared["w2T"] = _drK(np.asarray(g["fc_en2_w"].T, f32), SW2)
    b2B = np.zeros((128, H // 128, 128), f32)
    b2B[0] = g["fc_en2_b"].astype(f32).reshape(H // 128, 128) * SW2
    shared["b2B"] = b2B.reshape(128, H).astype(BF16NP)
    for nm in CELLS:
        wih = g[nm + "_wih"][GATE_PERM]
        whh = g[nm + "_whh"][GATE_PERM]
        bsum = (g[nm + "_bih"] + g[nm + "_bhh"])[GATE_PERM].astype(f32) * SWC
        if nm != "de1":
            shared[nm + "_wihT"] = _dr256(np.asarray(wih.T, f32), SWC)
        shared[nm + "_whhT"] = _dr256(np.asarray(whh.T, f32), SWC)
        bB = np.zeros((128, 128), f32)
        bB[:G // 128, :] = bsum.reshape(G // 128, 128)
        shared[nm + "_bB"] = bB.astype(BF16NP)
    E = np.zeros((128, 256), f32)
    for j in range(8):
        E[j, j * 32:(j + 1) * 32] = 1.0
    shared["Em"] = E.astype(BF16NP)
    shared["wd1T"] = np.ascontiguousarray(g["fc_de1_w"].T).astype(BF16NP)
    bd1B = np.zeros((128, HID // 128, 128), f32)
    bd1B[0] = g["fc_de1_b"].astype(f32).reshape(HID // 128, 128)
    shared["bd1B"] = bd1B.reshape(128, HID).astype(BF16NP)
    shared["wd2T"] = np.ascontiguousarray(g["fc_de2_w"].T).astype(BF16NP)
    shared["bd2v"] = np.ascontiguousarray(
        g["fc_de2_b"].astype(f32).reshape(D // 128, 128).T)

    x = g["x"].astype(f32).reshape(S, B, D)
    NCHA, CHA = 4, SB // 4
    in_maps = []
    for c in range(NCORES):
        xc = x[:, c * BL:(c + 1) * BL, :].reshape(SB, D)   # row = t*BL + b
        # DoubleRow chunked layout: row j*128+p, col ch*2*CHA + i*CHA + s
        # holds x[feature 256j+128i+p, sample ch*CHA+s]
        xT = xc.T.reshape(DJ, 2, 128, NCHA, CHA).transpose(0, 2, 3, 1, 4)
        m = dict(shared)
        m["xT"] = np.ascontiguousarray(
            xT.reshape(DJ * 128, 2 * SB)).astype(F8NP)
        in_maps.append(m)
    return in_maps, F


# ---------------------------------------------------------------------------
# Execution via PJRT (axon), modeled on bass2jax.run_bass_via_pjrt
# ---------------------------------------------------------------------------
def run_spmd(nc, in_maps, n_timing=0):
    import jax
    from jax.experimental.shard_map import shard_map
    from jax.sharding import Mesh, NamedSharding, PartitionSpec

    from concourse import bass2jax

    bass2jax.install_neuronx_cc_hook()
    n_cores = len(in_maps)
    partition_name = nc.partition_id_tensor.name if nc.partition_id_tensor else None
    in_names, out_names, out_avals, zero_outs = [], [], [], []
    for alloc in nc.m.functions[0].allocations:
        if not isinstance(alloc, mybir.MemoryLocationSet):
            continue
        name = alloc.memorylocations[0].name
        if alloc.kind == "ExternalInput":
            if name != partition_name:
                in_names.append(name)
        elif alloc.kind == "ExternalOutput":
            out_names.append(name)
            shape = tuple(alloc.tensor_shape)
            dtype = mybir.dt.np(alloc.dtype)
            out_avals.append(jax.core.ShapedArray(shape, dtype))
            zero_outs.append(np.zeros(shape, dtype))
    n_params = len(in_names)
    all_in = in_names + out_names
    if partition_name is not None:
        all_in = all_in + [partition_name]
    all_in = tuple(all_in)

    def _bind(args):
        operands = list(args)
        if partition_name is not None:
            operands.append(bass2jax.partition_id_tensor())
        return bass2jax._bass_exec_p.bind(
            *operands,
            out_avals=tuple(out_avals),
            in_names=all_in,
            out_names=tuple(out_names),
            lowering_input_output_aliases=(),
            sim_require_finite=False,
            sim_require_nnan=False,
            nc=nc,
        )

    def _body(*args):
        return tuple(_bind(args))

    devices = jax.devices()[:n_cores]
    mesh = Mesh(np.asarray(devices), ("core",))
    pspec = PartitionSpec("core")
    in_specs = (pspec,) * (n_params + len(out_names))
    out_specs = (pspec,) * len(out_names)

    f1 = jax.jit(shard_map(_body, mesh=mesh, in_specs=in_specs,
                           out_specs=out_specs, check_rep=False))
    concat = [
        np.concatenate([np.asarray(in_maps[c][nm]) for c in range(n_cores)], axis=0)
        for nm in in_names
    ]
    concat += [np.concatenate([z] * n_cores, axis=0) for z in zero_outs]

    sharding = NamedSharding(mesh, pspec)
    t0 = time.perf_counter()
    dev_in = [jax.device_put(a, sharding) for a in concat]
    jax.block_until_ready(dev_in)
    _log(f"upload {sum(a.nbytes for a in concat)/1e6:.1f} MB in "
         f"{time.perf_counter()-t0:.2f}s")

    t0 = time.perf_counter()
    outs = jax.block_until_ready(f1(*dev_in))
    _log(f"first run (incl compile) {time.perf_counter()-t0:.1f}s")

    results = []
    np_outs = [np.asarray(o) for o in outs]
    for c in range(n_cores):
        r = {}
        for i, nm in enumerate(out_names):
            sh0 = out_avals[i].shape[0]
            r[nm] = np_outs[i][c * sh0:(c + 1) * sh0]
        results.append(r)

    wall = None
    if n_timing:
        ts = []
        for _ in range(n_timing):
            t0 = time.perf_counter()
            jax.block_until_ready(f1(*dev_in))
            ts.append(time.perf_counter() - t0)
        wall = min(ts)
        _log("wall per call ms: " + " ".join(f"{t*1e3:.2f}" for t in ts))
    return results, wall, (f1, dev_in)


def measure_hw_time(F, in_maps, nrep=9, reps=14, nblocks=6, block=14):
    """HW exec estimate: block design. Same-program bursts (no per-call NEFF
    swap inside a block), alternating blocks between the 1-rep and nrep-rep
    programs to cancel slow drift of the ~85ms RPC floor. Per-iter =
    (median over blocks of block-median wall) diff / (nrep - 1). The older
    interleaved-min protocol swaps NEFFs every call, which adds a
    program-size-dependent cost and ~200us of noise to the estimate."""
    import jax

    import numpy as _np

    nc1 = build_program(F, nrep=1)
    _, _, (f1, dev1) = run_spmd(nc1, in_maps)
    ncN = build_program(F, nrep=nrep)
    _, _, (fN, devN) = run_spmd(ncN, in_maps)
    meds = {1: [], nrep: []}
    for blk in range(nblocks):
        for key, f, dev in ((1, f1, dev1), (nrep, fN, devN)):
            ts = []
            for i in range(block):
                t0 = time.perf_counter()
                jax.block_until_ready(f(*dev))
                ts.append(time.perf_counter() - t0)
            meds[key].append(float(_np.median(ts[2:])))
    # adjacent 1-rep / nrep-rep blocks share slow drift: difference them
    # pairwise, then take the median over pairs
    diffs = [(bN - b1) / (nrep - 1)
             for b1, bN in zip(meds[1], meds[nrep])]
    per_iter = max(float(_np.median(diffs)), 0.0)
    _log("block medians 1: " + " ".join(f"{t*1e3:.2f}" for t in meds[1]))
    _log(f"block medians {nrep}: " + " ".join(f"{t*1e3:.2f}" for t in meds[nrep]))
    _log("paired us/iter: " + " ".join(f"{d*1e6:.0f}" for d in diffs))
    _log(f"measure: -> {per_iter*1e6:.1f}us/iter")
    return per_iter * 1e9


_LAST_TIMING = None


def kernel(**inputs) -> np.ndarray:
    t0 = time.perf_counter()
    in_maps, F = prep_inputs(inputs)
    _log(f"host prep {time.perf_counter()-t0:.2f}s")
    t0 = time.perf_counter()
    nc = build_program(F)
    _log(f"build+tile {time.perf_counter()-t0:.1f}s")
    results, _, _ = run_spmd(nc, in_maps)
    out = np.empty((F, B, 64, 64), np.float32)
    for c in range(NCORES):
        yT = results[c]["yT"]                      # [4096, F*32]
        y = yT.T.reshape(F, BL, 64, 64)
        out[:, c * BL:(c + 1) * BL] = y
    return out
